# revision 1
# baseline (speedup 1.0000x reference)
"""Trainium2 Bass kernel for the DHMNN gnn_message_passing problem.

kernel(**inputs) takes FULL unsharded inputs (as produced by setup_inputs()),
shards across 8 NeuronCores internally, runs one SPMD Bass/Tile program, and
returns the full (Pe, Se, De) outputs.

Sharding: vertices / hyperedges / global nodes block-sharded 8 ways, weights
replicated. All segment reductions (graphnorm, GAT softmax+aggregate, score
segment-means) are done by sorting rows by segment on the host and
aggregating 128-row chunks on the PE via one-hot matmuls built with
DVE is_equal. Row gathers use dma_gather from bf16 tables in DRAM built with
AllGather collectives. The 6 segment-means in _score reduce algebraically to
4 (segment mean of x and x**2), and the segment-max in the GAT softmax is
dropped (mathematically invariant).
"""

import math
import numpy as np

P = 128


class Cfg:
    def __init__(self, NG, NV, NE, E, NT, NH, NGRAPH):
        self.NG, self.NV, self.NE, self.E, self.NT, self.NH = NG, NV, NE, E, NT, NH
        self.NGRAPH = NGRAPH
        self.DG, self.DL, self.H, self.HEADS, self.HID = 256, 128, 256, 4, 256
        self.D = self.H // self.HEADS
        self.Q = 0.5
        self.NCORES = 8
        self.VR = NV // self.NCORES
        self.ER = NE // self.NCORES
        self.GR = NG // self.NCORES
        self.VTILES = math.ceil(self.VR / P)
        self.ETILES = math.ceil(self.ER / P)
        self.GTILES = math.ceil(self.GR / P)
        self.VPAD = self.VTILES * P
        self.GHALF = (self.NCORES // 2) * self.VPAD
        self.NGT = NG // P            # global seq tiles (MHA keys)
        self.QT = self.GR // P        # own query tiles
        self.CPP = 6                  # gather chunks per dma_gather piece


FULL = Cfg(NG=4096, NV=50000, NE=50000, E=300000, NT=300000, NH=300000,
           NGRAPH=64)
SMALL = Cfg(NG=1024, NV=10000, NE=10000, E=60000, NT=60000, NH=60000,
            NGRAPH=64)

# ------------------------------------------------------------- layout helpers


def vhat(cfg, v):
    v = np.asarray(v)
    c, r = v // cfg.VR, v % cfg.VR
    p, t = r % P, r // P
    return c * cfg.VPAD + p * cfg.VTILES + t


def ghat(cfg, g):
    g = np.asarray(g)
    c, r = g // cfg.GR, g % cfg.GR
    p, t = r % P, r // P
    return c * cfg.GR + p * cfg.GTILES + t


def col_layout_f32(vals, lo, n_rows, ntiles):
    """(128, ntiles) f32: [p, t] = vals[lo + t*128 + p] (pad 0)."""
    arr = np.zeros(ntiles * P, np.float32)
    v = np.asarray(vals[lo:lo + n_rows], np.float32)
    arr[:len(v)] = v
    return np.ascontiguousarray(arr.reshape(ntiles, P).T)


def row_bcast(vec, n=P):
    return np.ascontiguousarray(
        np.tile(np.asarray(vec, np.float32).reshape(1, -1), (n, 1)))


def col_of(vec, nblk):
    """per-partition column layout: (128, nblk)[p, b] = vec[b*128+p]."""
    v = np.asarray(vec, np.float32).reshape(nblk, P)
    return np.ascontiguousarray(v.T)


# --------------------------------------------------------------- preprocessing


def _tile_groups(seg, core_lo, core_hi, ntiles):
    """Sorted row ids (and rebased seg ids) per output tile for segments in
    [core_lo, core_hi)."""
    order = np.argsort(seg, kind="stable")
    seg_s = seg[order]
    groups, rebased = [], []
    for o in range(ntiles):
        s0 = core_lo + P * o
        s1 = min(core_lo + P * (o + 1), core_hi)
        j0 = np.searchsorted(seg_s, s0)
        j1 = np.searchsorted(seg_s, s1)
        groups.append(order[j0:j1])
        rebased.append((seg_s[j0:j1] - s0).astype(np.float32))
    return groups, rebased


def _assemble(groups_per_core, shared_cnt, idx_fn, cpp):
    """Chunk-major per-core lists with shared (max over cores) per-tile chunk
    counts, padded to whole pieces."""
    ntiles = len(shared_cnt)
    total_chunks = int(shared_cnt.sum())
    npieces = max(1, math.ceil(total_chunks / cpp))
    tot_rows = npieces * cpp * P
    out = []
    for rows_tiles, seg_tiles in groups_per_core:
        idx = np.zeros(tot_rows, np.int64)
        seg = np.full(tot_rows, -1.0, np.float32)
        pos = 0
        for o in range(ntiles):
            rows, sg = rows_tiles[o], seg_tiles[o]
            n = len(rows)
            idx[pos:pos + n] = idx_fn(rows)
            seg[pos:pos + n] = sg
            pos += int(shared_cnt[o]) * P
        out.append(dict(idx=idx, seg=seg))
    return out, npieces


def idx_i16(idx):
    """(128, n/16) int16 wrapped layout, replicated across partition groups."""
    idx = np.asarray(idx, np.int64)
    n = len(idx)
    assert n % 16 == 0
    arr = np.zeros((16, n // 16), np.int64)
    arr[np.arange(n) % 16, np.arange(n) // 16] = idx
    assert arr.max() < 32768 and arr.min() >= 0
    return np.ascontiguousarray(np.tile(arr.astype(np.int16), (8, 1)))


def seg_cols(seg):
    """(128, nchunks) f32; column k = chunk k's rebased seg ids."""
    nch = len(seg) // P
    return np.ascontiguousarray(seg.reshape(nch, P).T)


def preprocess(cfg, C_vertex, C_edge, T_vertex, H_vertex, T_edge, H_edge,
               e_index):
    c64 = lambda x: np.asarray(x, np.int64)
    C_vertex, C_edge = c64(C_vertex), c64(C_edge)
    T_vertex, H_vertex = c64(T_vertex), c64(H_vertex)
    T_edge, H_edge, e_index = c64(T_edge), c64(H_edge), c64(e_index)

    cnt_T = np.bincount(T_edge, minlength=cfg.NE)
    cnt_H = np.bincount(H_edge, minlength=cfg.NE)
    recip_T = (1.0 / np.maximum(cnt_T, 1)).astype(np.float32)
    recip_H = (1.0 / np.maximum(cnt_H, 1)).astype(np.float32)
    gcnt = np.bincount(C_edge, minlength=cfg.NGRAPH)
    recip_G = (1.0 / np.maximum(gcnt, 1)).astype(np.float32)

    src2 = np.concatenate([e_index[0], np.arange(cfg.NV)])
    dst2 = np.concatenate([e_index[1], np.arange(cfg.NV)])

    meta = dict(lists={}, npieces={}, shared_cnt={})

    def groups_for(seg_arr, core_n, ntiles):
        out = []
        for c in range(cfg.NCORES):
            out.append(_tile_groups(seg_arr, c * core_n, (c + 1) * core_n,
                                    ntiles))
        return out

    def split_groups(groups, tab_fn, half, ntiles):
        """Restrict row groups to one table half (by tab_fn(row) < GHALF)."""
        res = []
        for c in range(cfg.NCORES):
            g_rows, g_segs = [], []
            for o in range(ntiles):
                rows, sg = groups[c][0][o], groups[c][1][o]
                tab = tab_fn(rows)
                sel = (tab < cfg.GHALF) if half == 0 else (tab >= cfg.GHALF)
                g_rows.append(rows[sel])
                g_segs.append(sg[sel])
            res.append((g_rows, g_segs))
        return res

    def shared_counts(groups, ntiles):
        cnts = np.zeros((cfg.NCORES, ntiles), np.int64)
        for c in range(cfg.NCORES):
            for o in range(ntiles):
                cnts[c, o] = math.ceil(len(groups[c][0][o]) / P)
        return cnts.max(0)

    def emit(name, groups, idx_fn, ntiles):
        shared = shared_counts(groups, ntiles)
        lists, npieces = _assemble(groups, shared, idx_fn, cfg.CPP)
        meta["lists"][name] = lists
        meta["npieces"][name] = npieces
        meta["shared_cnt"][name] = shared

    for side, (seg, gl) in dict(T=(T_edge, T_vertex),
                                H=(H_edge, H_vertex)).items():
        gg = C_vertex[gl]
        groups = groups_for(seg, cfg.ER, cfg.ETILES)
        emit(f"{side}_G", groups, lambda rows, gg=gg: ghat(cfg, gg[rows]),
             cfg.ETILES)
        tab_fn = lambda rows, gl=gl: vhat(cfg, gl[rows])
        for half, sfx in ((0, "lo"), (1, "hi")):
            sub = split_groups(groups, tab_fn, half, cfg.ETILES)
            off = 0 if half == 0 else cfg.GHALF
            emit(f"{side}_L{sfx}", sub,
                 lambda rows, off=off, tab_fn=tab_fn: tab_fn(rows) - off,
                 cfg.ETILES)

    # GAT: rows grouped by dst; gather xw|a_s by src (lo/hi) and mirror a_d
    # lists (same row order, indexed by dst)
    groups = groups_for(dst2, cfg.VR, cfg.VTILES)
    tab_fn = lambda rows: vhat(cfg, src2[rows])
    for half, sfx in ((0, "lo"), (1, "hi")):
        sub = split_groups(groups, tab_fn, half, cfg.VTILES)
        off = 0 if half == 0 else cfg.GHALF
        emit(f"GAT_X{sfx}", sub,
             lambda rows, off=off: vhat(cfg, src2[rows]) - off, cfg.VTILES)
        ad_fn = lambda rows: vhat(cfg, dst2[rows])
        shared = meta["shared_cnt"][f"GAT_X{sfx}"]
        ad_lists, _ = _assemble(sub, shared, ad_fn, cfg.CPP)
        for c in range(cfg.NCORES):
            ad_lists[c]["idx"] = np.maximum(
                ad_lists[c]["idx"] - c * cfg.VPAD, 0)
        meta["lists"][f"GAT_AD{sfx}"] = ad_lists
        meta["npieces"][f"GAT_AD{sfx}"] = meta["npieces"][f"GAT_X{sfx}"]

    percore = []
    for c in range(cfg.NCORES):
        md = {}
        md["recip_T"] = col_layout_f32(recip_T, c * cfg.ER, cfg.ER, cfg.ETILES)
        md["recip_H"] = col_layout_f32(recip_H, c * cfg.ER, cfg.ER, cfg.ETILES)
        ce = np.full(cfg.VPAD, -1.0, np.float32)
        ce[:cfg.VR] = C_edge[c * cfg.VR:(c + 1) * cfg.VR]
        md["cedge"] = np.ascontiguousarray(ce.reshape(cfg.VTILES, P).T)
        percore.append(md)
    meta["percore"] = percore
    meta["recip_G"] = recip_G
    return meta


# ----------------------------------------------------------- input preparation

LIST_KINDS = ["T_G", "T_Llo", "T_Lhi", "H_G", "H_Llo", "H_Lhi",
              "GAT_Xlo", "GAT_Xhi"]


def prepare_inputs(cfg, inputs, meta):
    f32 = np.float32
    H, DG, DL = cfg.H, cfg.DG, cfg.DL
    Xg = np.asarray(inputs["Xg"], f32)
    Xl = np.asarray(inputs["Xl"], f32)
    Xe = np.asarray(inputs["Xe"], f32)

    s = {}
    s["XgT"] = np.ascontiguousarray(Xg.T.reshape(2, P, -1).transpose(1, 0, 2))
    s["iota"] = row_bcast(np.arange(P, dtype=f32))
    s["ident"] = np.eye(P, dtype=f32)
    s["lingWT"] = np.ascontiguousarray(
        np.asarray(inputs["ling_W"], f32).T.reshape(2, P, H).transpose(1, 0, 2))
    s["lingb_col"] = col_of(inputs["ling_b"], 2)
    s["WinT"] = np.ascontiguousarray(
        np.asarray(inputs["mha_in_W"], f32).T.reshape(2, P, 3 * H)
        .transpose(1, 0, 2))
    s["Winb_col"] = col_of(inputs["mha_in_b"], 6)
    s["WoutT"] = np.ascontiguousarray(
        np.asarray(inputs["mha_out_W"], f32).T.reshape(2, P, H).transpose(1, 0, 2))
    s["Woutb_row"] = row_bcast(inputs["mha_out_b"])
    s["normgw_row"] = row_bcast(inputs["normg_w"])
    s["normgb_row"] = row_bcast(inputs["normg_b"])
    s["linlWT"] = np.ascontiguousarray(np.asarray(inputs["linl_W"], f32).T)
    s["linlb_row"] = row_bcast(inputs["linl_b"])
    ms = np.asarray(inputs["gn_ms"], f32)
    s["ms_row"] = row_bcast(ms)
    s["ms2_row"] = row_bcast(2.0 * ms - ms * ms)
    s["gnw_row"] = row_bcast(inputs["gn_w"])
    s["gnb_row"] = row_bcast(inputs["gn_b"])
    rg = np.zeros((P, 1), f32)
    rg[:cfg.NGRAPH, 0] = meta["recip_G"]
    s["recipG_col"] = rg
    s["gatWT"] = np.ascontiguousarray(
        np.asarray(inputs["gat_W"], f32).T.reshape(2, P, H).transpose(1, 0, 2))
    s["attsrc_row"] = row_bcast(np.asarray(inputs["gat_att_src"], f32)
                                .reshape(-1))
    s["attdst_row"] = row_bcast(np.asarray(inputs["gat_att_dst"], f32)
                                .reshape(-1))
    s["gatb_row"] = row_bcast(inputs["gat_b"])
    s["swg_row"] = row_bcast(np.asarray(inputs["lingS_W"], f32)[0])
    s["swl_row"] = row_bcast(np.asarray(inputs["linlS_W"], f32)[0])
    s["sbg_col"] = np.full((P, 1), np.asarray(inputs["lingS_b"], f32)
                           .reshape(-1)[0], f32)
    s["sbl_col"] = np.full((P, 1), np.asarray(inputs["linlS_b"], f32)
                           .reshape(-1)[0], f32)
    s["DWTg"] = np.ascontiguousarray(
        np.asarray(inputs["lingD_W"], f32).T.reshape(2, P, H).transpose(1, 0, 2))
    s["DWTl"] = np.ascontiguousarray(
        np.asarray(inputs["linlD_W"], f32).T.reshape(2, P, H).transpose(1, 0, 2))
    s["W1T"] = np.ascontiguousarray(
        np.asarray(inputs["mlp_W1"], f32).T.reshape(2, P, cfg.HID).transpose(1, 0, 2))
    s["b1_row"] = row_bcast(inputs["mlp_b1"])
    s["lnw_row"] = row_bcast(inputs["mlp_ln_w"])
    s["lnb_row"] = row_bcast(inputs["mlp_ln_b"])
    s["w2_row"] = row_bcast(np.asarray(inputs["mlp_W2"], f32)[0])
    s["b2_col"] = np.full((P, 1), np.asarray(inputs["mlp_b2"], f32)
                          .reshape(-1)[0], f32)

    in_maps = []
    for c in range(cfg.NCORES):
        md = meta["percore"][c]
        m = dict(s)
        m["XgT_own"] = np.ascontiguousarray(
            Xg[c * cfg.GR:(c + 1) * cfg.GR].T.reshape(2, P, -1)
            .transpose(1, 0, 2))
        xlp = np.zeros((cfg.VPAD, DL), f32)
        xlp[:cfg.VR] = Xl[c * cfg.VR:(c + 1) * cfg.VR]
        m["XlT"] = np.ascontiguousarray(xlp.T)
        xep = np.zeros((cfg.VPAD, DG), f32)
        xep[:cfg.ER] = Xe[c * cfg.ER:(c + 1) * cfg.ER]
        m["XeT"] = np.ascontiguousarray(xep.T.reshape(2, P, -1).transpose(1, 0, 2))
        m["recip_T"], m["recip_H"] = md["recip_T"], md["recip_H"]
        m["cedge"] = md["cedge"]
        for kind in LIST_KINDS:
            lst = meta["lists"][kind][c]
            m[f"idx_{kind}"] = idx_i16(lst["idx"])
            m[f"seg_{kind}"] = seg_cols(lst["seg"])
        for sfx in ("lo", "hi"):
            m[f"idx_GAT_AD{sfx}"] = idx_i16(
                meta["lists"][f"GAT_AD{sfx}"][c]["idx"])
        in_maps.append(m)
    return in_maps


# --------------------------------------------------------------------- builder


def build(cfg, meta, maxphase=99, reps=1):
    from contextlib import ExitStack
    import concourse.bacc as bacc
    import concourse.mybir as mybir
    import concourse.tile as tile

    f32 = mybir.dt.float32
    bf16 = mybir.dt.bfloat16
    i16 = mybir.dt.int16
    AF = mybir.ActivationFunctionType
    OP = mybir.AluOpType
    AX = mybir.AxisListType

    H, DG, DL, HEADS = cfg.H, cfg.DG, cfg.DL, cfg.HEADS
    NGRAPH, VTILES, ETILES, CPP = cfg.NGRAPH, cfg.VTILES, cfg.ETILES, cfg.CPP
    NGT, QT, GR = cfg.NGT, cfg.QT, cfg.GR

    nc = bacc.Bacc("TRN2", target_bir_lowering=False, debug=False,
                   num_devices=cfg.NCORES)
    RG = [list(range(cfg.NCORES))]

    def din(name, shape, dtype=f32):
        return nc.dram_tensor(name, list(shape), dtype, kind="ExternalInput")

    D_IN = {}
    for name, shape, dt in [
        ("XgT", (P, 2, cfg.NG), f32), ("XgT_own", (P, 2, GR), f32),
        ("XlT", (DL, cfg.VPAD), f32), ("XeT", (P, 2, cfg.VPAD), f32),
        ("iota", (P, P), f32), ("ident", (P, P), f32),
        ("lingWT", (P, 2, H), f32), ("lingb_col", (P, 2), f32),
        ("WinT", (P, 2, 3 * H), f32), ("Winb_col", (P, 6), f32),
        ("WoutT", (P, 2, H), f32), ("Woutb_row", (P, H), f32),
        ("normgw_row", (P, H), f32), ("normgb_row", (P, H), f32),
        ("linlWT", (DL, H), f32), ("linlb_row", (P, H), f32),
        ("ms_row", (P, H), f32), ("ms2_row", (P, H), f32),
        ("gnw_row", (P, H), f32), ("gnb_row", (P, H), f32),
        ("recipG_col", (P, 1), f32),
        ("gatWT", (P, 2, H), f32), ("attsrc_row", (P, H), f32),
        ("attdst_row", (P, H), f32), ("gatb_row", (P, H), f32),
        ("swg_row", (P, 4 * H), f32), ("swl_row", (P, 4 * H), f32),
        ("sbg_col", (P, 1), f32), ("sbl_col", (P, 1), f32),
        ("DWTg", (P, 2, H), f32), ("DWTl", (P, 2, H), f32),
        ("W1T", (P, 2, cfg.HID), f32), ("b1_row", (P, cfg.HID), f32),
        ("lnw_row", (P, cfg.HID), f32), ("lnb_row", (P, cfg.HID), f32),
        ("w2_row", (P, cfg.HID), f32), ("b2_col", (P, 1), f32),
        ("recip_T", (P, ETILES), f32), ("recip_H", (P, ETILES), f32),
        ("cedge", (P, VTILES), f32),
    ]:
        D_IN[name] = din(name, shape, dt)
    npieces = meta["npieces"]
    for kind in LIST_KINDS:
        npc = npieces[kind]
        D_IN[f"idx_{kind}"] = din(f"idx_{kind}", (P, npc * CPP * P // 16), i16)
        D_IN[f"seg_{kind}"] = din(f"seg_{kind}", (P, npc * CPP), f32)
    for sfx in ("lo", "hi"):
        npc = npieces[f"GAT_X{sfx}"]
        D_IN[f"idx_GAT_AD{sfx}"] = din(f"idx_GAT_AD{sfx}",
                                       (P, npc * CPP * P // 16), i16)

    Pe_out = nc.dram_tensor("Pe_out", [P, ETILES], f32, kind="ExternalOutput")
    Se_out = nc.dram_tensor("Se_out", [P, ETILES], f32, kind="ExternalOutput")
    De_out = nc.dram_tensor("De_out", [P, ETILES], f32, kind="ExternalOutput")

    gn_in = nc.dram_tensor("gn_in", [NGRAPH, 2 * H], f32)
    gn_out = nc.dram_tensor("gn_out", [NGRAPH, 2 * H], f32, addr_space="Shared")
    xw_in = nc.dram_tensor("xw_in", [P, VTILES, 384], bf16)
    XWtab = nc.dram_tensor("XWtab", [cfg.NCORES * cfg.VPAD, 384], bf16,
                           addr_space="Shared")
    ad_loc = nc.dram_tensor("ad_loc", [cfg.VPAD, P], bf16)
    g_in = nc.dram_tensor("g_in", [P, cfg.GTILES, 2 * H], bf16)
    Gtab = nc.dram_tensor("Gtab", [cfg.NG, 2 * H], bf16, addr_space="Shared")
    l_in = nc.dram_tensor("l_in", [P, VTILES, 2 * H], bf16)
    Ltab = nc.dram_tensor("Ltab", [cfg.NCORES * cfg.VPAD, 2 * H], bf16,
                          addr_space="Shared")

    GHALF = cfg.GHALF

    with tile.TileContext(nc) as tc, ExitStack() as top:
        const = top.enter_context(tc.tile_pool(name="const", bufs=1))
        persist = top.enter_context(tc.tile_pool(name="persist", bufs=1))
        for _rep in range(reps):
            mid = ExitStack()
            loc = mid.enter_context(tc.tile_pool(name="loc", bufs=1))

            CT = {}

            def cget(name, pool=None):
                if name not in CT:
                    d = D_IN[name]
                    t = (pool or const).tile(list(d.shape), d.dtype, tag=name)
                    nc.sync.dma_start(t[:], d[:])
                    CT[name] = t
                return CT[name]

            iota_t = cget("iota")
            ident_t = cget("ident")
            pe_stage = persist.tile([P, ETILES], f32, tag="pe_stage")
            se_stage = persist.tile([P, ETILES], f32, tag="se_stage")
            de_stage = persist.tile([P, ETILES], f32, tag="de_stage")
            nc.vector.memset(pe_stage[:], 0.0)
            nc.vector.memset(se_stage[:], 0.0)
            nc.vector.memset(de_stage[:], 0.0)
            eps_col = const.tile([P, 1], f32, tag="eps_col")
            nc.vector.memset(eps_col[:], 1e-5)
            half_col = const.tile([P, 1], f32, tag="half_col")
            nc.vector.memset(half_col[:], 0.5)

            def onehot(pool, segc, dtype, ncols=P, tag="oh"):
                oh = pool.tile([P, ncols], dtype, tag=tag)
                nc.vector.tensor_scalar(out=oh[:], in0=iota_t[:, :ncols],
                                        scalar1=segc, scalar2=None,
                                        op0=OP.is_equal)
                return oh

            class GatherList:
                def __init__(self, kind, table_ap, elem, pool, cpool,
                             dtype=bf16, idx_kind=None):
                    self.kind = kind
                    self.table_ap = table_ap
                    self.elem = elem
                    self.pool = pool
                    self.dtype = dtype
                    ik = idx_kind or kind
                    self.npc = npieces[ik if ik in npieces else kind]
                    d_idx = D_IN[f"idx_{kind}"]
                    self.idx_t = cpool.tile(list(d_idx.shape), i16,
                                            tag=f"idx_{kind}")
                    nc.sync.dma_start(self.idx_t[:], d_idx[:])
                    if f"seg_{kind}" in D_IN:
                        d_seg = D_IN[f"seg_{kind}"]
                        self.seg_t = cpool.tile(list(d_seg.shape), f32,
                                                tag=f"seg_{kind}")
                        nc.sync.dma_start(self.seg_t[:], d_seg[:])
                    else:
                        self.seg_t = None
                    self.k = 0
                    self.cur = None

                def chunk(self):
                    p, s = divmod(self.k, CPP)
                    if s == 0:
                        self.cur = self.pool.tile([P, CPP, self.elem], self.dtype,
                                                  tag=f"gb_{self.kind}")
                        nidx = CPP * P
                        nc.gpsimd.dma_gather(
                            self.cur[:], self.table_ap,
                            self.idx_t[:, p * (nidx // 16):(p + 1) * (nidx // 16)],
                            nidx, nidx, self.elem)
                    rows = self.cur[:, s, :]
                    segc = (self.seg_t[:, self.k:self.k + 1]
                            if self.seg_t is not None else None)
                    self.k += 1
                    return rows, segc

            # ==================================================== phase 1: linl+gn
            if maxphase >= 1:
             with tc.tile_pool(name="p1", bufs=1) as p1, \
                 tc.tile_pool(name="w1", bufs=2) as w1, \
                 tc.tile_pool(name="ps1", bufs=1, space="PSUM") as ps1, \
                 tc.tile_pool(name="ps1g", bufs=1, space="PSUM") as ps1g:
                linlWT_t = cget("linlWT", p1)
                linlb_t = cget("linlb_row", p1)
                cedge_t = cget("cedge", p1)
                gn_ps = ps1g.tile([NGRAPH, 2 * H], f32, tag="gn")
                xlT_t = p1.tile([DL, cfg.VPAD], f32, tag="xlT")
                nc.sync.dma_start(xlT_t[:], D_IN["XlT"][:, :])
                for t in range(VTILES):
                    mm = ps1.tile([P, H], f32, tag="mm1")
                    nc.tensor.matmul(mm[:], lhsT=xlT_t[:, t * P:(t + 1) * P],
                                     rhs=linlWT_t[:], start=True, stop=True)
                    sc2 = w1.tile([P, 2 * H], f32, tag="sc2")
                    nc.vector.tensor_tensor(out=sc2[:, :H], in0=mm[:],
                                            in1=linlb_t[:], op=OP.add)
                    nc.scalar.activation(sc2[:, :H], sc2[:, :H], AF.Tanh)
                    nc.scalar.activation(sc2[:, H:], sc2[:, :H], AF.Square)
                    oh = onehot(w1, cedge_t[:, t:t + 1], f32, NGRAPH, tag="ohg")
                    nc.tensor.matmul(gn_ps[:], lhsT=oh[:], rhs=sc2[:],
                                     start=(t == 0), stop=(t == VTILES - 1))
                gn_sb = w1.tile([NGRAPH, 2 * H], f32, tag="gnsb")
                nc.vector.tensor_copy(gn_sb[:], gn_ps[:])
                nc.sync.dma_start(gn_in[:, :], gn_sb[:])
                nc.gpsimd.collective_compute(
                    "AllReduce", OP.add, replica_groups=RG,
                    ins=[gn_in[:, :]], outs=[gn_out[:, :]])
                gn_glob = w1.tile([NGRAPH, 2 * H], f32, tag="gnglob")
                nc.sync.dma_start(gn_glob[:], gn_out[:, :])

                # per-graph affine: x*A + B
                rgc = cget("recipG_col")
                AB = p1.tile([NGRAPH, 2 * H], f32, tag="AB")
                mean_t = w1.tile([NGRAPH, H], f32, tag="gmean")
                nc.vector.tensor_scalar(out=mean_t[:], in0=gn_glob[:, :H],
                                        scalar1=rgc[:NGRAPH, :], scalar2=None,
                                        op0=OP.mult)
                ex2_t = w1.tile([NGRAPH, H], f32, tag="gex2")
                nc.vector.tensor_scalar(out=ex2_t[:], in0=gn_glob[:, H:],
                                        scalar1=rgc[:NGRAPH, :], scalar2=None,
                                        op0=OP.mult)
                var_t = w1.tile([NGRAPH, H], f32, tag="gvar")
                nc.vector.tensor_tensor(out=var_t[:], in0=mean_t[:],
                                        in1=mean_t[:], op=OP.mult)
                nc.vector.tensor_tensor(out=var_t[:], in0=var_t[:],
                                        in1=cget("ms2_row")[:NGRAPH, :],
                                        op=OP.mult)
                nc.vector.tensor_tensor(out=var_t[:], in0=ex2_t[:], in1=var_t[:],
                                        op=OP.subtract)
                sq_t = w1.tile([NGRAPH, H], f32, tag="gsq")
                nc.scalar.activation(sq_t[:], var_t[:], AF.Sqrt,
                                     bias=eps_col[:NGRAPH, :])
                rstd_t = w1.tile([NGRAPH, H], f32, tag="grstd")
                nc.vector.reciprocal(rstd_t[:], sq_t[:])
                nc.vector.tensor_tensor(out=AB[:, :H], in0=rstd_t[:],
                                        in1=cget("gnw_row")[:NGRAPH, :],
                                        op=OP.mult)
                t2 = w1.tile([NGRAPH, H], f32, tag="gt2")
                nc.vector.tensor_tensor(out=t2[:], in0=mean_t[:], in1=AB[:, :H],
                                        op=OP.mult)
                nc.vector.tensor_tensor(out=t2[:], in0=t2[:],
                                        in1=cget("ms_row")[:NGRAPH, :],
                                        op=OP.mult)
                nc.vector.tensor_tensor(out=AB[:, H:],
                                        in0=cget("gnb_row")[:NGRAPH, :],
                                        in1=t2[:], op=OP.subtract)

                # ------------------------------------------ phase 1b: gn apply, xw
                gatWT_t = cget("gatWT", p1)
                attsrc_t = cget("attsrc_row", p1)
                attdst_t = cget("attdst_row", p1)
                xl1n_store = loc.tile([P, VTILES, H], f32, tag="xl1n")
                xwst = p1.tile([P, VTILES, 384], bf16, tag="xwst")
                adst = p1.tile([P, VTILES, P], bf16, tag="adst")
                nc.vector.memset(xwst[:], 0.0)
                nc.vector.memset(adst[:], 0.0)
                for t in range(VTILES):
                    oh = onehot(w1, cedge_t[:, t:t + 1], f32, NGRAPH, tag="ohg")
                    ohT_ps = ps1.tile([NGRAPH, P], f32, tag="ohTps")
                    nc.tensor.transpose(ohT_ps[:], oh[:], ident_t[:])
                    ohT = w1.tile([NGRAPH, P], f32, tag="ohT")
                    nc.vector.tensor_copy(ohT[:], ohT_ps[:])
                    abrows = ps1.tile([P, 2 * H], f32, tag="abrows")
                    nc.tensor.matmul(abrows[:], lhsT=ohT[:], rhs=AB[:],
                                     start=True, stop=True)
                    mm2 = ps1.tile([P, H], f32, tag="mm1")
                    nc.tensor.matmul(mm2[:], lhsT=xlT_t[:, t * P:(t + 1) * P],
                                     rhs=linlWT_t[:], start=True, stop=True)
                    xl1r = w1.tile([P, H], f32, tag="xl1r")
                    nc.vector.tensor_tensor(out=xl1r[:], in0=mm2[:],
                                            in1=linlb_t[:], op=OP.add)
                    nc.scalar.activation(xl1r[:], xl1r[:], AF.Tanh)
                    xl1n = xl1n_store[:, t, :]
                    nc.vector.tensor_tensor(out=xl1n, in0=xl1r[:],
                                            in1=abrows[:, :H], op=OP.mult)
                    nc.vector.tensor_tensor(out=xl1n, in0=xl1n, in1=abrows[:, H:],
                                            op=OP.add)
                    xnT_ps = ps1.tile([P, 2, P], f32, tag="xnTps")
                    nc.tensor.transpose(xnT_ps[:, 0, :], xl1n[:, :P], ident_t[:])
                    nc.tensor.transpose(xnT_ps[:, 1, :], xl1n[:, P:], ident_t[:])
                    xnT = w1.tile([P, 2, P], f32, tag="xnT")
                    nc.vector.tensor_copy(xnT[:], xnT_ps[:])
                    xw_ps = ps1.tile([P, H], f32, tag="xwps")
                    for k in range(2):
                        nc.tensor.matmul(xw_ps[:], lhsT=xnT[:, k, :],
                                         rhs=gatWT_t[:, k, :],
                                         start=(k == 0), stop=(k == 1))
                    scr = w1.tile([P, H], f32, tag="scrxw")
                    asd = w1.tile([P, 8], f32, tag="asd")
                    nc.vector.tensor_tensor(out=scr[:], in0=xw_ps[:],
                                            in1=attsrc_t[:], op=OP.mult)
                    nc.vector.tensor_reduce(
                        out=asd[:, 0:HEADS],
                        in_=scr[:].rearrange("p (h d) -> p h d", h=HEADS),
                        axis=AX.X, op=OP.add)
                    nc.vector.tensor_tensor(out=scr[:], in0=xw_ps[:],
                                            in1=attdst_t[:], op=OP.mult)
                    nc.vector.tensor_reduce(
                        out=asd[:, HEADS:2 * HEADS],
                        in_=scr[:].rearrange("p (h d) -> p h d", h=HEADS),
                        axis=AX.X, op=OP.add)
                    nc.vector.tensor_copy(xwst[:, t, :H], xw_ps[:])
                    nc.vector.tensor_copy(xwst[:, t, H:H + HEADS], asd[:, 0:HEADS])
                    nc.vector.tensor_copy(adst[:, t, 0:HEADS],
                                          asd[:, HEADS:2 * HEADS])
                nc.sync.dma_start(xw_in[:, :, :], xwst[:])
                nc.gpsimd.collective_compute(
                    "AllGather", OP.bypass, replica_groups=RG,
                    ins=[xw_in[:, :, :]], outs=[XWtab[:, :]])
                nc.sync.dma_start(
                    ad_loc[:].rearrange("(p t) d -> p (t d)", p=P), adst[:])

            # ======================================================== phase 2: MHA
            if maxphase >= 2:
             with tc.tile_pool(name="mha", bufs=1) as mp, \
                 tc.tile_pool(name="wm", bufs=3) as wm:
                lingWT_t = cget("lingWT", mp)
                lingb_t = cget("lingb_col", mp)
                WinT_t = cget("WinT", mp)
                WinT_bf = mp.tile([P, 2, 3 * H], bf16, tag="WinT_bf")
                nc.vector.tensor_copy(WinT_bf[:], WinT_t[:])
                Winb_t = cget("Winb_col", mp)
                NCH = cfg.NG // 512

                xg1T = mp.tile([P, 2, cfg.NG], bf16, tag="xg1T")
                with tc.tile_pool(name="xgp", bufs=1) as xgp, \
                     tc.tile_pool(name="psx", bufs=2, space="PSUM") as psx:
                    xgT_t = xgp.tile([P, 2, cfg.NG], f32, tag="xgT")
                    nc.sync.dma_start(xgT_t[:], D_IN["XgT"][:, :, :])
                    for fb in range(2):
                        for ch in range(NCH):
                            mm = psx.tile([P, 512], f32, tag="mmg")
                            for k in range(2):
                                nc.tensor.matmul(
                                    mm[:],
                                    lhsT=lingWT_t[:, k, fb * P:(fb + 1) * P],
                                    rhs=xgT_t[:, k, ch * 512:(ch + 1) * 512],
                                    start=(k == 0), stop=(k == 1))
                            nc.scalar.activation(
                                xg1T[:, fb, ch * 512:(ch + 1) * 512], mm[:],
                                AF.Tanh, bias=lingb_t[:, fb:fb + 1])
                    # own q block: from XgT_own
                    xg1T_own = mp.tile([P, 2, GR], f32, tag="xg1T_own")
                    xgTo_t = xgp.tile([P, 2, GR], f32, tag="xgTo")
                    nc.sync.dma_start(xgTo_t[:], D_IN["XgT_own"][:, :, :])
                    for fb in range(2):
                        mm = psx.tile([P, GR], f32, tag="mmgo")
                        for k in range(2):
                            nc.tensor.matmul(
                                mm[:], lhsT=lingWT_t[:, k, fb * P:(fb + 1) * P],
                                rhs=xgTo_t[:, k, :], start=(k == 0), stop=(k == 1))
                        nc.scalar.activation(xg1T_own[:, fb, :], mm[:], AF.Tanh,
                                             bias=lingb_t[:, fb:fb + 1])

                kT = mp.tile([P, 2, cfg.NG], bf16, tag="kT")
                vext = mp.tile([P, NGT, HEADS, 65], bf16, tag="vext")
                nc.vector.memset(vext[:], 1.0)
                qT_own = mp.tile([P, 2, GR], bf16, tag="qT_own")
                xg1own = mp.tile([P, QT, DG], f32, tag="xg1own")
                with tc.tile_pool(name="psk", bufs=1, space="PSUM") as psk:
                    # q (own)
                    for fb in range(2):
                        mm = psk.tile([P, GR], f32, tag="qmm")
                        for k in range(2):
                            nc.tensor.matmul(
                                mm[:], lhsT=WinT_t[:, k, fb * P:(fb + 1) * P],
                                rhs=xg1T_own[:, k, :], start=(k == 0),
                                stop=(k == 1))
                        nc.scalar.activation(qT_own[:, fb, :], mm[:], AF.Identity,
                                             bias=Winb_t[:, fb:fb + 1])
                    # k (full)
                    for fb in range(2):
                        for ch in range(NCH):
                            mm = psk.tile([P, 512], f32, tag="kmm")
                            for k in range(2):
                                nc.tensor.matmul(
                                    mm[:],
                                    lhsT=WinT_bf[:, k, (2 + fb) * P:(3 + fb) * P],
                                    rhs=xg1T[:, k, ch * 512:(ch + 1) * 512],
                                    start=(k == 0), stop=(k == 1))
                            nc.scalar.activation(
                                kT[:, fb, ch * 512:(ch + 1) * 512], mm[:],
                                AF.Identity, bias=Winb_t[:, 2 + fb:3 + fb])
                    # v (full) -> transposed into vext rows
                    for fb in range(2):
                        for ch in range(NCH):
                            mm = psk.tile([P, 512], f32, tag="vmm")
                            for k in range(2):
                                nc.tensor.matmul(
                                    mm[:],
                                    lhsT=WinT_bf[:, k, (4 + fb) * P:(5 + fb) * P],
                                    rhs=xg1T[:, k, ch * 512:(ch + 1) * 512],
                                    start=(k == 0), stop=(k == 1))
                            vsb = wm.tile([P, 512], f32, tag="vsb")
                            nc.scalar.activation(vsb[:], mm[:], AF.Identity,
                                                 bias=Winb_t[:, 4 + fb:5 + fb])
                            for hh in range(2):
                                h = fb * 2 + hh
                                for j in range(4):
                                    kt = ch * 4 + j
                                    tp = psk.tile([P, 64], f32, tag="vtp")
                                    nc.tensor.transpose(
                                        tp[:],
                                        vsb[hh * 64:(hh + 1) * 64,
                                            j * P:(j + 1) * P],
                                        ident_t[hh * 64:(hh + 1) * 64,
                                                hh * 64:(hh + 1) * 64])
                                    nc.vector.tensor_copy(
                                        vext[:, kt, h, :64], tp[:])
                    # residual rows (own)
                    for qt in range(QT):
                        for k in range(2):
                            tp = psk.tile([P, P], f32, tag="xg1ownT")
                            nc.tensor.transpose(
                                tp[:], xg1T_own[:, k, qt * P:(qt + 1) * P],
                                ident_t[:])
                            nc.vector.tensor_copy(
                                xg1own[:, qt, k * P:(k + 1) * P], tp[:])

                attnout = mp.tile([P, QT, H], f32, tag="attnout")
                with tc.tile_pool(name="expp", bufs=1) as expp, \
                     tc.tile_pool(name="psS", bufs=2, space="PSUM") as psS, \
                     tc.tile_pool(name="psAV", bufs=1, space="PSUM") as psAV:
                    HNGT = NGT // 2
                    for h in range(HEADS):
                        kT_h = kT[64 * (h % 2):64 * (h % 2) + 64, h // 2, :]
                        qT_h = qT_own[64 * (h % 2):64 * (h % 2) + 64, h // 2, :]
                        avs = []
                        for qt in range(QT):
                            av_t = psAV.tile([P, 65], f32, tag=f"av{qt}")
                            avs.append(av_t)
                        for half in range(2):
                            expS = expp.tile([P, HNGT, GR], bf16, tag="expS")
                            for kt in range(HNGT):
                                ktg = half * HNGT + kt
                                s_ps = psS.tile([P, GR], f32, tag="sps")
                                nc.tensor.matmul(
                                    s_ps[:], lhsT=kT_h[:, ktg * P:(ktg + 1) * P],
                                    rhs=qT_h[:, :], start=True, stop=True)
                                nc.scalar.activation(expS[:, kt, :], s_ps[:],
                                                     AF.Exp, scale=0.125)
                            for qt in range(QT):
                                for kt in range(HNGT):
                                    ktg = half * HNGT + kt
                                    nc.tensor.matmul(
                                        avs[qt][:],
                                        lhsT=expS[:, kt, qt * P:(qt + 1) * P],
                                        rhs=vext[:, ktg, h, :],
                                        start=(ktg == 0), stop=(ktg == NGT - 1))
                        for qt in range(QT):
                            rc = wm.tile([P, 1], f32, tag="avrc")
                            nc.vector.reciprocal(rc[:], avs[qt][:, 64:65])
                            nc.vector.tensor_scalar(
                                out=attnout[:, qt, h * 64:(h + 1) * 64],
                                in0=avs[qt][:, :64], scalar1=rc[:], scalar2=None,
                                op0=OP.mult)

                # out proj + residual + LN + tanh -> staging
                WoutT_t = cget("WoutT", mp)
                gst = mp.tile([P, cfg.GTILES, 2 * H], bf16, tag="gst")
                with tc.tile_pool(name="pso", bufs=2, space="PSUM") as pso:
                    for qt in range(QT):
                        aT = wm.tile([P, 2, P], f32, tag="aT")
                        for k in range(2):
                            tp = pso.tile([P, P], f32, tag="aTps")
                            nc.tensor.transpose(
                                tp[:], attnout[:, qt, k * P:(k + 1) * P],
                                ident_t[:])
                            nc.vector.tensor_copy(aT[:, k, :], tp[:])
                        o_ps = pso.tile([P, H], f32, tag="ops")
                        for k in range(2):
                            nc.tensor.matmul(o_ps[:], lhsT=aT[:, k, :],
                                             rhs=WoutT_t[:, k, :],
                                             start=(k == 0), stop=(k == 1))
                        xs = wm.tile([P, H], f32, tag="xs")
                        nc.vector.tensor_tensor(out=xs[:], in0=o_ps[:],
                                                in1=cget("Woutb_row", mp)[:],
                                                op=OP.add)
                        nc.vector.tensor_tensor(out=xs[:], in0=xs[:],
                                                in1=xg1own[:, qt, :], op=OP.add)
                        ssum = wm.tile([P, 1], f32, tag="ssum")
                        nc.vector.tensor_reduce(out=ssum[:], in_=xs[:], axis=AX.X,
                                                op=OP.add)
                        mu = wm.tile([P, 1], f32, tag="mu")
                        nc.scalar.mul(mu[:], ssum[:], 1.0 / H)
                        xc = wm.tile([P, H], f32, tag="xc")
                        nc.vector.tensor_scalar(out=xc[:], in0=xs[:],
                                                scalar1=mu[:], scalar2=None,
                                                op0=OP.subtract)
                        scr = wm.tile([P, H], f32, tag="lnscr")
                        ss2 = wm.tile([P, 1], f32, tag="ss2")
                        nc.scalar.activation(scr[:], xc[:], AF.Square,
                                             accum_out=ss2[:])
                        sq = wm.tile([P, 1], f32, tag="sqt")
                        nc.scalar.activation(sq[:], ss2[:], AF.Sqrt,
                                             bias=eps_col[:], scale=1.0 / H)
                        rstd = wm.tile([P, 1], f32, tag="rstd")
                        nc.vector.reciprocal(rstd[:], sq[:])
                        xn = wm.tile([P, H], f32, tag="xn")
                        nc.vector.tensor_scalar(out=xn[:], in0=xc[:],
                                                scalar1=rstd[:], scalar2=None,
                                                op0=OP.mult)
                        nc.vector.tensor_tensor(out=xn[:], in0=xn[:],
                                                in1=cget("normgw_row", mp)[:],
                                                op=OP.mult)
                        nc.vector.tensor_tensor(out=xn[:], in0=xn[:],
                                                in1=cget("normgb_row", mp)[:],
                                                op=OP.add)
                        xg2 = wm.tile([P, H], f32, tag="xg2")
                        nc.scalar.activation(xg2[:], xn[:], AF.Tanh)
                        nc.vector.tensor_copy(gst[:, qt, :H], xg2[:])
                        nc.scalar.activation(gst[:, qt, H:], xg2[:], AF.Square)
                nc.sync.dma_start(g_in[:, :, :], gst[:])
                nc.gpsimd.collective_compute(
                    "AllGather", OP.bypass, replica_groups=RG,
                    ins=[g_in[:, :, :]], outs=[Gtab[:, :]])

            # ======================================================== phase 3: GAT
            if maxphase >= 3:
             with tc.tile_pool(name="gatp", bufs=1) as gp, \
                 tc.tile_pool(name="wg", bufs=3) as wg, \
                 tc.tile_pool(name="gbuf", bufs=2) as gbuf, \
                 tc.tile_pool(name="psg", bufs=2, space="PSUM") as psg:
                gatb_t = cget("gatb_row", gp)
                xlo = GatherList("GAT_Xlo", XWtab[:GHALF, :], 384, gbuf, gp)
                xhi = GatherList("GAT_Xhi", XWtab[GHALF:, :], 384, gbuf, gp)
                adlo = GatherList("GAT_ADlo", ad_loc[:, :], P, gbuf, gp,
                                  idx_kind="GAT_Xlo")
                adhi = GatherList("GAT_ADhi", ad_loc[:, :], P, gbuf, gp,
                                  idx_kind="GAT_Xhi")
                xl2st = gp.tile([P, VTILES, 2 * H], bf16, tag="xl2st")
                cnt_lo = meta["shared_cnt"]["GAT_Xlo"]
                cnt_hi = meta["shared_cnt"]["GAT_Xhi"]
                ex_pid, ex_tile = {}, {}
                for ot in range(VTILES):
                    acc = psg.tile([P, 260], f32, tag="gatacc")
                    total = int(cnt_lo[ot]) + int(cnt_hi[ot])
                    done = 0
                    for gl, adl, cnt in ((xlo, adlo, int(cnt_lo[ot])),
                                         (xhi, adhi, int(cnt_hi[ot]))):
                        for _ in range(cnt):
                            rows, segc = gl.chunk()
                            adrows, _ = adl.chunk()
                            pid = (gl.k - 1) // CPP
                            if ex_pid.get(gl.kind) != pid:
                                ex_pid[gl.kind] = pid
                                ext = wg.tile([P, CPP, HEADS], f32,
                                              tag=f"ex_{gl.kind}")
                                nc.vector.tensor_tensor(
                                    out=ext[:], in0=gl.cur[:, :, H:H + HEADS],
                                    in1=adl.cur[:, :, 0:HEADS], op=OP.add)
                                t02 = wg.tile([P, CPP, HEADS], f32, tag="t02")
                                nc.vector.tensor_scalar(out=t02[:], in0=ext[:],
                                                        scalar1=0.2, scalar2=None,
                                                        op0=OP.mult)
                                nc.vector.tensor_tensor(out=ext[:], in0=t02[:],
                                                        in1=ext[:], op=OP.max)
                                nc.scalar.activation(ext[:], ext[:], AF.Exp)
                                ex_tile[gl.kind] = ext
                            s = (gl.k - 1) % CPP
                            ex = ex_tile[gl.kind][:, s, :]
                            rhs = wg.tile([P, 260], bf16, tag="gatrhs")
                            for h in range(HEADS):
                                nc.vector.tensor_scalar(
                                    out=rhs[:, h * 64:(h + 1) * 64],
                                    in0=rows[:, h * 64:(h + 1) * 64],
                                    scalar1=ex[:, h:h + 1], scalar2=None,
                                    op0=OP.mult)
                            nc.vector.tensor_copy(rhs[:, H:], ex[:])
                            oh = onehot(wg, segc, bf16, tag="ohgat")
                            nc.tensor.matmul(acc[:], lhsT=oh[:], rhs=rhs[:],
                                             start=(done == 0),
                                             stop=(done == total - 1))
                            done += 1
                    dene = wg.tile([P, HEADS], f32, tag="dene")
                    nc.vector.tensor_scalar(out=dene[:], in0=acc[:, H:],
                                            scalar1=1e-20, scalar2=None,
                                            op0=OP.add)
                    rc = wg.tile([P, HEADS], f32, tag="gatrc")
                    nc.vector.reciprocal(rc[:], dene[:])
                    xl2 = wg.tile([P, H], f32, tag="xl2")
                    for h in range(HEADS):
                        nc.vector.tensor_scalar(
                            out=xl2[:, h * 64:(h + 1) * 64],
                            in0=acc[:, h * 64:(h + 1) * 64],
                            scalar1=rc[:, h:h + 1], scalar2=None, op0=OP.mult)
                    nc.vector.tensor_tensor(out=xl2[:], in0=xl2[:], in1=gatb_t[:],
                                            op=OP.add)
                    nc.vector.tensor_tensor(out=xl2[:], in0=xl2[:],
                                            in1=xl1n_store[:, ot, :], op=OP.add)
                    xl2t = wg.tile([P, H], f32, tag="xl2t")
                    nc.scalar.activation(xl2t[:], xl2[:], AF.Tanh)
                    nc.vector.tensor_copy(xl2st[:, ot, :H], xl2t[:])
                    nc.scalar.activation(xl2st[:, ot, H:], xl2t[:],
                                         AF.Square)
                nc.sync.dma_start(l_in[:, :, :], xl2st[:])
                nc.gpsimd.collective_compute(
                    "AllGather", OP.bypass, replica_groups=RG,
                    ins=[l_in[:, :, :]], outs=[Ltab[:, :]])

            mid.close()

            # ======================================================== phase 4: MLP
            if maxphase >= 4:
             with tc.tile_pool(name="mlpp", bufs=1) as mlpp, \
                 tc.tile_pool(name="wp", bufs=3) as wp, \
                 tc.tile_pool(name="psm", bufs=2, space="PSUM") as psm:
                W1T_t = cget("W1T", mlpp)
                b1_t = cget("b1_row", mlpp)
                lnw_t = cget("lnw_row", mlpp)
                lnb_t = cget("lnb_row", mlpp)
                w2_t = cget("w2_row", mlpp)
                b2_t = cget("b2_col", mlpp)
                xeT_t = mlpp.tile([P, 2, cfg.VPAD], f32, tag="xeT")
                nc.sync.dma_start(xeT_t[:], D_IN["XeT"][:, :, :])
                for t in range(ETILES):
                    mm = psm.tile([P, cfg.HID], f32, tag="mmp")
                    for k in range(2):
                        nc.tensor.matmul(mm[:],
                                         lhsT=xeT_t[:, k, t * P:(t + 1) * P],
                                         rhs=W1T_t[:, k, :],
                                         start=(k == 0), stop=(k == 1))
                    h1 = wp.tile([P, cfg.HID], f32, tag="h1")
                    nc.vector.tensor_tensor(out=h1[:], in0=mm[:], in1=b1_t[:],
                                            op=OP.add)
                    nc.scalar.activation(h1[:], h1[:], AF.Relu)
                    ssum = wp.tile([P, 1], f32, tag="psum1")
                    nc.vector.tensor_reduce(out=ssum[:], in_=h1[:], axis=AX.X,
                                            op=OP.add)
                    mu = wp.tile([P, 1], f32, tag="pmu")
                    nc.scalar.mul(mu[:], ssum[:], 1.0 / cfg.HID)
                    xc = wp.tile([P, cfg.HID], f32, tag="pxc")
                    nc.vector.tensor_scalar(out=xc[:], in0=h1[:], scalar1=mu[:],
                                            scalar2=None, op0=OP.subtract)
                    scr = wp.tile([P, cfg.HID], f32, tag="pscr")
                    ss2 = wp.tile([P, 1], f32, tag="pss2")
                    nc.scalar.activation(scr[:], xc[:], AF.Square,
                                         accum_out=ss2[:])
                    sq = wp.tile([P, 1], f32, tag="psq")
                    nc.scalar.activation(sq[:], ss2[:], AF.Sqrt,
                                         bias=eps_col[:], scale=1.0 / cfg.HID)
                    rstd = wp.tile([P, 1], f32, tag="prstd")
                    nc.vector.reciprocal(rstd[:], sq[:])
                    xn = wp.tile([P, cfg.HID], f32, tag="pxn")
                    nc.vector.tensor_scalar(out=xn[:], in0=xc[:], scalar1=rstd[:],
                                            scalar2=None, op0=OP.mult)
                    nc.vector.tensor_tensor(out=xn[:], in0=xn[:], in1=lnw_t[:],
                                            op=OP.mult)
                    nc.vector.tensor_tensor(out=xn[:], in0=xn[:], in1=lnb_t[:],
                                            op=OP.add)
                    z = wp.tile([P, 1], f32, tag="pz")
                    nc.vector.tensor_tensor(out=scr[:], in0=xn[:], in1=w2_t[:],
                                            op=OP.mult)
                    nc.vector.tensor_reduce(out=z[:], in_=scr[:], axis=AX.X,
                                            op=OP.add)


# revision 24
# speedup vs baseline: 2.0212x; 2.0212x over previous
"""Trainium2 Bass kernel for the DHMNN gnn_message_passing problem.

kernel(**inputs) takes FULL unsharded inputs, shards across 8 NeuronCores,
runs one SPMD Bass/Tile program, returns full (Pe, Se, De).

v2 design (vs baseline):
- Tables carry [x | q_u | q_v] (258 cols bf16) instead of [x | x^2] (512):
  the Se terms sum(u*x^2) are precomputed per vertex before the AllGather,
  halving AllGather bytes, gather DMA and score matmul width.
- Se computed via z = rT*A + rH*B + rT^2*C + rH^2*D + rT*rH*E with
  host-combined weight rows (u=sw0+sw2, v=sw1+sw3, w=sw2+sw3).
- De cosine is scale-invariant: segment sums used directly (no recip).
- Sqrt/Sigmoid batched at the end -> ~5 activation-table loads total.
- GAT att_src/att_dst folded into the gatW matmul (264-col weights).
- MLP LayerNorm folded algebraically into two staged columns.
- Score G-branch runs before/during the Ltab AllGather; CPP=12.
"""

import math
import numpy as np

P = 128


class Cfg:
    def __init__(self, NG, NV, NE, E, NT, NH, NGRAPH):
        self.NG, self.NV, self.NE, self.E, self.NT, self.NH = NG, NV, NE, E, NT, NH
        self.NGRAPH = NGRAPH
        self.DG, self.DL, self.H, self.HEADS, self.HID = 256, 128, 256, 4, 256
        self.D = self.H // self.HEADS
        self.Q = 0.5
        self.NCORES = 8
        self.VR = NV // self.NCORES
        self.ER = NE // self.NCORES
        self.GR = NG // self.NCORES
        self.VTILES = math.ceil(self.VR / P)
        self.ETILES = math.ceil(self.ER / P)
        self.GTILES = math.ceil(self.GR / P)
        self.VPAD = self.VTILES * P
        self.GHALF = (self.NCORES // 2) * self.VPAD
        self.NGT = NG // P            # global seq tiles (MHA keys)
        self.QT = self.GR // P        # own query tiles
        self.CPP = 8                  # gather chunks per dma_gather piece


FULL = Cfg(NG=4096, NV=50000, NE=50000, E=300000, NT=300000, NH=300000,
           NGRAPH=64)
SMALL = Cfg(NG=1024, NV=10000, NE=10000, E=60000, NT=60000, NH=60000,
            NGRAPH=64)

# ------------------------------------------------------------- layout helpers


def vhat(cfg, v):
    v = np.asarray(v)
    c, r = v // cfg.VR, v % cfg.VR
    p, t = r % P, r // P
    return c * cfg.VPAD + p * cfg.VTILES + t


def ghat(cfg, g):
    g = np.asarray(g)
    c, r = g // cfg.GR, g % cfg.GR
    p, t = r % P, r // P
    return c * cfg.GR + p * cfg.GTILES + t


def col_layout_f32(vals, lo, n_rows, ntiles):
    """(128, ntiles) f32: [p, t] = vals[lo + t*128 + p] (pad 0)."""
    arr = np.zeros(ntiles * P, np.float32)
    v = np.asarray(vals[lo:lo + n_rows], np.float32)
    arr[:len(v)] = v
    return np.ascontiguousarray(arr.reshape(ntiles, P).T)


def row_bcast(vec, n=P):
    return np.ascontiguousarray(
        np.tile(np.asarray(vec, np.float32).reshape(1, -1), (n, 1)))


def col_of(vec, nblk):
    v = np.asarray(vec, np.float32).reshape(nblk, P)
    return np.ascontiguousarray(v.T)


# --------------------------------------------------------------- preprocessing


def _tile_groups(seg, core_lo, core_hi, ntiles):
    order = np.argsort(seg, kind="stable")
    seg_s = seg[order]
    groups, rebased = [], []
    for o in range(ntiles):
        s0 = core_lo + P * o
        s1 = min(core_lo + P * (o + 1), core_hi)
        j0 = np.searchsorted(seg_s, s0)
        j1 = np.searchsorted(seg_s, s1)
        groups.append(order[j0:j1])
        rebased.append((seg_s[j0:j1] - s0).astype(np.float32))
    return groups, rebased


def _assemble(groups_per_core, shared_cnt, idx_fn, cpp):
    ntiles = len(shared_cnt)
    total_chunks = int(shared_cnt.sum())
    npieces = max(1, math.ceil(total_chunks / cpp))
    tot_rows = npieces * cpp * P
    out = []
    for rows_tiles, seg_tiles in groups_per_core:
        idx = np.zeros(tot_rows, np.int64)
        seg = np.full(tot_rows, -1.0, np.float32)
        pos = 0
        for o in range(ntiles):
            rows, sg = rows_tiles[o], seg_tiles[o]
            n = len(rows)
            idx[pos:pos + n] = idx_fn(rows)
            seg[pos:pos + n] = sg
            pos += int(shared_cnt[o]) * P
        out.append(dict(idx=idx, seg=seg))
    return out, npieces


def idx_i16(idx):
    """(128, n/16) int16 wrapped layout, replicated across partition groups."""
    idx = np.asarray(idx, np.int64)
    n = len(idx)
    assert n % 16 == 0
    arr = np.zeros((16, n // 16), np.int64)
    arr[np.arange(n) % 16, np.arange(n) // 16] = idx
    assert arr.max() < 32768 and arr.min() >= 0
    return np.ascontiguousarray(np.tile(arr.astype(np.int16), (8, 1)))


def seg_cols(seg):
    nch = len(seg) // P
    return np.ascontiguousarray(seg.reshape(nch, P).T)


def preprocess(cfg, C_vertex, C_edge, T_vertex, H_vertex, T_edge, H_edge,
               e_index):
    c64 = lambda x: np.asarray(x, np.int64)
    C_vertex, C_edge = c64(C_vertex), c64(C_edge)
    T_vertex, H_vertex = c64(T_vertex), c64(H_vertex)
    T_edge, H_edge, e_index = c64(T_edge), c64(H_edge), c64(e_index)

    cnt_T = np.bincount(T_edge, minlength=cfg.NE)
    cnt_H = np.bincount(H_edge, minlength=cfg.NE)
    recip_T = (1.0 / np.maximum(cnt_T, 1)).astype(np.float32)
    recip_H = (1.0 / np.maximum(cnt_H, 1)).astype(np.float32)
    gcnt = np.bincount(C_edge, minlength=cfg.NGRAPH)
    recip_G = (1.0 / np.maximum(gcnt, 1)).astype(np.float32)

    src2 = np.concatenate([e_index[0], np.arange(cfg.NV)])
    dst2 = np.concatenate([e_index[1], np.arange(cfg.NV)])

    meta = dict(lists={}, npieces={}, shared_cnt={})

    def groups_for(seg_arr, core_n, ntiles):
        out = []
        for c in range(cfg.NCORES):
            out.append(_tile_groups(seg_arr, c * core_n, (c + 1) * core_n,
                                    ntiles))
        return out

    def split_groups(groups, tab_fn, half, ntiles):
        res = []
        for c in range(cfg.NCORES):
            g_rows, g_segs = [], []
            for o in range(ntiles):
                rows, sg = groups[c][0][o], groups[c][1][o]
                tab = tab_fn(rows)
                sel = (tab < cfg.GHALF) if half == 0 else (tab >= cfg.GHALF)
                g_rows.append(rows[sel])
                g_segs.append(sg[sel])
            res.append((g_rows, g_segs))
        return res

    def shared_counts(groups, ntiles):
        cnts = np.zeros((cfg.NCORES, ntiles), np.int64)
        for c in range(cfg.NCORES):
            for o in range(ntiles):
                cnts[c, o] = math.ceil(len(groups[c][0][o]) / P)
        return cnts.max(0)

    def emit(name, groups, idx_fn, ntiles):
        shared = shared_counts(groups, ntiles)
        lists, npieces = _assemble(groups, shared, idx_fn, cfg.CPP)
        meta["lists"][name] = lists
        meta["npieces"][name] = npieces
        meta["shared_cnt"][name] = shared

    for side, (seg, gl) in dict(T=(T_edge, T_vertex),
                                H=(H_edge, H_vertex)).items():
        gg = C_vertex[gl]
        groups = groups_for(seg, cfg.ER, cfg.ETILES)
        emit(f"{side}_G", groups, lambda rows, gg=gg: ghat(cfg, gg[rows]),
             cfg.ETILES)
        tab_fn = lambda rows, gl=gl: vhat(cfg, gl[rows])
        for half, sfx in ((0, "lo"), (1, "hi")):
            sub = split_groups(groups, tab_fn, half, cfg.ETILES)
            off = 0 if half == 0 else cfg.GHALF
            emit(f"{side}_L{sfx}", sub,
                 lambda rows, off=off, tab_fn=tab_fn: tab_fn(rows) - off,
                 cfg.ETILES)

    groups = groups_for(dst2, cfg.VR, cfg.VTILES)
    tab_fn = lambda rows: vhat(cfg, src2[rows])
    for half, sfx in ((0, "lo"), (1, "hi")):
        sub = split_groups(groups, tab_fn, half, cfg.VTILES)
        off = 0 if half == 0 else cfg.GHALF
        emit(f"GAT_X{sfx}", sub,
             lambda rows, off=off: vhat(cfg, src2[rows]) - off, cfg.VTILES)
        ad_fn = lambda rows: vhat(cfg, dst2[rows])
        shared = meta["shared_cnt"][f"GAT_X{sfx}"]
        ad_lists, _ = _assemble(sub, shared, ad_fn, cfg.CPP)
        for c in range(cfg.NCORES):
            ad_lists[c]["idx"] = np.maximum(
                ad_lists[c]["idx"] - c * cfg.VPAD, 0)
        meta["lists"][f"GAT_AD{sfx}"] = ad_lists
        meta["npieces"][f"GAT_AD{sfx}"] = meta["npieces"][f"GAT_X{sfx}"]

    percore = []
    for c in range(cfg.NCORES):
        md = {}
        rT = col_layout_f32(recip_T, c * cfg.ER, cfg.ER, cfg.ETILES)
        rH = col_layout_f32(recip_H, c * cfg.ER, cfg.ER, cfg.ETILES)
        # rcols order matches zbuf [C, D, E, A, B]
        md["rcols"] = np.ascontiguousarray(
            np.stack([rT * rT, rH * rH, rT * rH, rT, rH], axis=1))
        ce = np.full(cfg.VPAD, -1.0, np.float32)
        ce[:cfg.VR] = C_edge[c * cfg.VR:(c + 1) * cfg.VR]
        md["cedge"] = np.ascontiguousarray(ce.reshape(cfg.VTILES, P).T)
        percore.append(md)
    meta["percore"] = percore
    meta["recip_G"] = recip_G
    return meta


# ----------------------------------------------------------- input preparation

LIST_KINDS = ["T_G", "T_Llo", "T_Lhi", "H_G", "H_Llo", "H_Lhi",
              "GAT_Xlo", "GAT_Xhi"]


def prepare_inputs(cfg, inputs, meta):
    f32 = np.float32
    H, DG, DL, HEADS, D = cfg.H, cfg.DG, cfg.DL, cfg.HEADS, cfg.D
    Xg = np.asarray(inputs["Xg"], f32)
    Xl = np.asarray(inputs["Xl"], f32)
    Xe = np.asarray(inputs["Xe"], f32)

    s = {}
    s["XgT"] = np.ascontiguousarray(Xg.T.reshape(2, P, -1).transpose(1, 0, 2))
    s["iota"] = row_bcast(np.arange(P, dtype=f32))
    s["ident"] = np.eye(P, dtype=f32)
    s["lingWT"] = np.ascontiguousarray(
        np.asarray(inputs["ling_W"], f32).T.reshape(2, P, H).transpose(1, 0, 2))
    s["lingb_col"] = col_of(inputs["ling_b"], 2)
    s["WinT"] = np.ascontiguousarray(
        np.asarray(inputs["mha_in_W"], f32).T.reshape(2, P, 3 * H)
        .transpose(1, 0, 2))
    s["Winb_col"] = col_of(inputs["mha_in_b"], 6)
    s["WoutT"] = np.ascontiguousarray(
        np.asarray(inputs["mha_out_W"], f32).T.reshape(2, P, H).transpose(1, 0, 2))
    s["Woutb_row"] = row_bcast(inputs["mha_out_b"])
    s["normgw_row"] = row_bcast(inputs["normg_w"])
    s["normgb_row"] = row_bcast(inputs["normg_b"])
    s["linlWT"] = np.ascontiguousarray(np.asarray(inputs["linl_W"], f32).T)
    s["linlb_row"] = row_bcast(inputs["linl_b"])
    ms = np.asarray(inputs["gn_ms"], f32)
    s["ms_row"] = row_bcast(ms)
    s["ms2_row"] = row_bcast(2.0 * ms - ms * ms)
    s["gnw_row"] = row_bcast(inputs["gn_w"])
    s["gnb_row"] = row_bcast(inputs["gn_b"])
    rg = np.zeros((P, 1), f32)
    rg[:cfg.NGRAPH, 0] = meta["recip_G"]
    s["recipG_col"] = rg

    # GAT weights extended with folded attention vectors:
    # xw = x @ W.T ; a_s = sum_d xw[h*64+d]*att_src[h,d] = x @ WAs[:,h]
    gw = np.asarray(inputs["gat_W"], f32)            # [H, H] (out, in)
    asrc = np.asarray(inputs["gat_att_src"], f32)    # [HEADS, D]
    adst = np.asarray(inputs["gat_att_dst"], f32)
    WAs = np.einsum("hdk,hd->kh", gw.reshape(HEADS, D, H), asrc)  # [H, HEADS]
    WAd = np.einsum("hdk,hd->kh", gw.reshape(HEADS, D, H), adst)
    gatx = np.concatenate([gw.T, WAs, WAd], axis=1)  # [H(in), 264]
    s["gatWTx"] = np.ascontiguousarray(
        gatx.reshape(2, P, 264).transpose(1, 0, 2))
    s["gatb_row"] = row_bcast(inputs["gat_b"])

    # score weight rows, order [u, v, wu, wv, w2m] per branch
    def score_rows(SW):
        sw = np.asarray(SW, f32).reshape(4, H)
        u, v, w = sw[0] + sw[2], sw[1] + sw[3], sw[2] + sw[3]
        return np.ascontiguousarray(np.stack(
            [row_bcast(u), row_bcast(v), row_bcast(w - u), row_bcast(w - v),
             row_bcast(-2.0 * w)], axis=1))
    s["scoreW_g"] = score_rows(inputs["lingS_W"])
    s["scoreW_l"] = score_rows(inputs["linlS_W"])
    s["sbg_col"] = np.full((P, 1), np.asarray(inputs["lingS_b"], f32)
                           .reshape(-1)[0], f32)
    s["sbl_col"] = np.full((P, 1), np.asarray(inputs["linlS_b"], f32)
                           .reshape(-1)[0], f32)
    s["DWTg"] = np.ascontiguousarray(
        np.asarray(inputs["lingD_W"], f32).T.reshape(2, P, H).transpose(1, 0, 2))
    s["DWTl"] = np.ascontiguousarray(
        np.asarray(inputs["linlD_W"], f32).T.reshape(2, P, H).transpose(1, 0, 2))

    # MLP folds
    s["W1T"] = np.ascontiguousarray(
        np.asarray(inputs["mlp_W1"], f32).T.reshape(2, P, cfg.HID).transpose(1, 0, 2))
    s["b1_row"] = row_bcast(inputs["mlp_b1"])
    lnw = np.asarray(inputs["mlp_ln_w"], f32)
    lnb = np.asarray(inputs["mlp_ln_b"], f32)
    w2 = np.asarray(inputs["mlp_W2"], f32).reshape(-1)
    wl = lnw * w2
    s["wl_row"] = row_bcast(wl)
    s["woh_col"] = np.full((P, 1), float(wl.sum()) / cfg.HID, f32)
    c1 = float((w2 * lnb).sum()) + float(np.asarray(inputs["mlp_b2"], f32)
                                         .reshape(-1)[0])
    s["c1b2_col"] = np.full((P, 1), c1, f32)

    in_maps = []
    for c in range(cfg.NCORES):
        md = meta["percore"][c]
        m = dict(s)
        m["XgT_own"] = np.ascontiguousarray(
            Xg[c * cfg.GR:(c + 1) * cfg.GR].T.reshape(2, P, -1)
            .transpose(1, 0, 2))
        xlp = np.zeros((cfg.VPAD, DL), f32)
        xlp[:cfg.VR] = Xl[c * cfg.VR:(c + 1) * cfg.VR]
        m["XlT"] = np.ascontiguousarray(xlp.T)
        xep = np.zeros((cfg.VPAD, DG), f32)
        xep[:cfg.ER] = Xe[c * cfg.ER:(c + 1) * cfg.ER]
        m["XeT"] = np.ascontiguousarray(xep.T.reshape(2, P, -1).transpose(1, 0, 2))
        m["rcols"] = md["rcols"]
        m["cedge"] = md["cedge"]
        for kind in LIST_KINDS:
            lst = meta["lists"][kind][c]
            m[f"idx_{kind}"] = idx_i16(lst["idx"])
            m[f"seg_{kind}"] = seg_cols(lst["seg"])
        for sfx in ("lo", "hi"):
            m[f"idx_GAT_AD{sfx}"] = idx_i16(
                meta["lists"][f"GAT_AD{sfx}"][c]["idx"])
        in_maps.append(m)
    return in_maps


# --------------------------------------------------------------------- builder


def build(cfg, meta, maxphase=99, reps=1):
    from contextlib import ExitStack
    import concourse.bacc as bacc
    import concourse.mybir as mybir
    import concourse.tile as tile

    f32 = mybir.dt.float32
    bf16 = mybir.dt.bfloat16
    i16 = mybir.dt.int16
    AF = mybir.ActivationFunctionType
    OP = mybir.AluOpType
    AX = mybir.AxisListType

    H, DG, DL, HEADS = cfg.H, cfg.DG, cfg.DL, cfg.HEADS
    NGRAPH, VTILES, ETILES, CPP = cfg.NGRAPH, cfg.VTILES, cfg.ETILES, cfg.CPP
    NGT, QT, GR, HID = cfg.NGT, cfg.QT, cfg.GR, cfg.HID

    nc = bacc.Bacc("TRN2", target_bir_lowering=False, debug=False,
                   num_devices=cfg.NCORES, dynamic_dma_scratch_size=32768)
    RG = [list(range(cfg.NCORES))]

    def din(name, shape, dtype=f32):
        return nc.dram_tensor(name, list(shape), dtype, kind="ExternalInput")

    D_IN = {}
    for name, shape, dt in [
        ("XgT", (P, 2, cfg.NG), f32), ("XgT_own", (P, 2, GR), f32),
        ("XlT", (DL, cfg.VPAD), f32), ("XeT", (P, 2, cfg.VPAD), f32),
        ("iota", (P, P), f32), ("ident", (P, P), f32),
        ("lingWT", (P, 2, H), f32), ("lingb_col", (P, 2), f32),
        ("WinT", (P, 2, 3 * H), f32), ("Winb_col", (P, 6), f32),
        ("WoutT", (P, 2, H), f32), ("Woutb_row", (P, H), f32),
        ("normgw_row", (P, H), f32), ("normgb_row", (P, H), f32),
        ("linlWT", (DL, H), f32), ("linlb_row", (P, H), f32),
        ("ms_row", (P, H), f32), ("ms2_row", (P, H), f32),
        ("gnw_row", (P, H), f32), ("gnb_row", (P, H), f32),
        ("recipG_col", (P, 1), f32),
        ("gatWTx", (P, 2, 264), f32), ("gatb_row", (P, H), f32),
        ("scoreW_g", (P, 5, H), f32), ("scoreW_l", (P, 5, H), f32),
        ("sbg_col", (P, 1), f32), ("sbl_col", (P, 1), f32),
        ("DWTg", (P, 2, H), f32), ("DWTl", (P, 2, H), f32),
        ("W1T", (P, 2, HID), f32), ("b1_row", (P, HID), f32),
        ("wl_row", (P, HID), f32), ("woh_col", (P, 1), f32),
        ("c1b2_col", (P, 1), f32),
        ("rcols", (P, 5, ETILES), f32),
        ("cedge", (P, VTILES), f32),
    ]:
        D_IN[name] = din(name, shape, dt)
    npieces = meta["npieces"]
    for kind in LIST_KINDS:
        npc = npieces[kind]
        D_IN[f"idx_{kind}"] = din(f"idx_{kind}", (P, npc * CPP * P // 16), i16)
        D_IN[f"seg_{kind}"] = din(f"seg_{kind}", (P, npc * CPP), f32)
    for sfx in ("lo", "hi"):
        npc = npieces[f"GAT_X{sfx}"]
        D_IN[f"idx_GAT_AD{sfx}"] = din(f"idx_GAT_AD{sfx}",
                                       (P, npc * CPP * P // 16), i16)

    Pe_out = nc.dram_tensor("Pe_out", [P, ETILES], f32, kind="ExternalOutput")
    Se_out = nc.dram_tensor("Se_out", [P, ETILES], f32, kind="ExternalOutput")
    De_out = nc.dram_tensor("De_out", [P, ETILES], f32, kind="ExternalOutput")

    gn_in = nc.dram_tensor("gn_in", [NGRAPH, 2 * H], f32)
    gn_out = nc.dram_tensor("gn_out", [NGRAPH, 2 * H], f32, addr_space="Shared")
    xw_in = nc.dram_tensor("xw_in", [P, VTILES, 384], bf16)
    XWtab = nc.dram_tensor("XWtab", [cfg.NCORES * cfg.VPAD, 384], bf16,
                           addr_space="Shared")
    ad_loc = nc.dram_tensor("ad_loc", [cfg.VPAD, 128], bf16)
    g_in = nc.dram_tensor("g_in", [P, cfg.GTILES, 384], bf16)
    Gtab = nc.dram_tensor("Gtab", [cfg.NG, 384], bf16, addr_space="Shared")
    l_in = nc.dram_tensor("l_in", [P, VTILES, 384], bf16)
    Ltab = nc.dram_tensor("Ltab", [cfg.NCORES * cfg.VPAD, 384], bf16,
                          addr_space="Shared")

    GHALF = cfg.GHALF

    with tile.TileContext(nc) as tc, ExitStack() as top:
        const = top.enter_context(tc.tile_pool(name="const", bufs=1))
        persist = top.enter_context(tc.tile_pool(name="persist", bufs=1))

        mid = ExitStack()
        loc = mid.enter_context(tc.tile_pool(name="loc", bufs=1))

        CT = {}

        def cget(name, pool=None):
            if name not in CT:
                d = D_IN[name]
                t = (pool or const).tile(list(d.shape), d.dtype, tag=name)
                nc.sync.dma_start(t[:], d[:])
                CT[name] = t
            return CT[name]

        iota_t = cget("iota")
        ident_t = cget("ident")
        iota_bf = const.tile([P, P], bf16, tag="iota_bf")
        nc.vector.tensor_copy(iota_bf[:], iota_t[:])
        ident_bf = const.tile([P, P], bf16, tag="ident_bf")
        nc.vector.tensor_copy(ident_bf[:], ident_t[:])

        pe_stage = persist.tile([P, ETILES], f32, tag="pe_stage")
        se_stage = persist.tile([P, ETILES], f32, tag="se_stage")
        de_stage = persist.tile([P, ETILES], f32, tag="de_stage")
        nc.vector.memset(pe_stage[:], 0.0)
        nc.vector.memset(se_stage[:], 0.0)
        nc.vector.memset(de_stage[:], 0.0)
        zstG = persist.tile([P, ETILES], f32, tag="zstG")
        zstL = persist.tile([P, ETILES], f32, tag="zstL")
        numG = persist.tile([P, ETILES], f32, tag="numG")
        numL = persist.tile([P, ETILES], f32, tag="numL")
        nnG = persist.tile([P, ETILES], f32, tag="nnG")
        nnL = persist.tile([P, ETILES], f32, tag="nnL")
        Ast = persist.tile([P, ETILES], f32, tag="Ast")
        varst = persist.tile([P, ETILES], f32, tag="varst")
        eps_col = const.tile([P, 1], f32, tag="eps_col")
        nc.vector.memset(eps_col[:], 1e-5)

        def onehot(pool, segc, ncols=P, tag="oh"):
            oh = pool.tile([P, ncols], bf16, tag=tag)
            nc.vector.tensor_scalar(out=oh[:], in0=iota_bf[:, :ncols],
                                    scalar1=segc, scalar2=None,
                                    op0=OP.is_equal)
            return oh

        class GatherList:
            def __init__(self, kind, table_ap, elem, pool, cpool,
                         idx_kind=None):
                self.kind = kind
                self.table_ap = table_ap
                self.elem = elem
                self.pool = pool
                ik = idx_kind or kind
                self.npc = npieces[ik if ik in npieces else kind]
                d_idx = D_IN[f"idx_{kind}"]
                self.idx_t = cpool.tile(list(d_idx.shape), i16,
                                        tag=f"idx_{kind}")
                nc.sync.dma_start(self.idx_t[:], d_idx[:])
                if f"seg_{kind}" in D_IN:
                    d_seg = D_IN[f"seg_{kind}"]
                    self.seg_t = cpool.tile(list(d_seg.shape), f32,
                                            tag=f"seg_{kind}")
                    nc.sync.dma_start(self.seg_t[:], d_seg[:])
                else:
                    self.seg_t = None
                self.k = 0
                self.cur = None

            def chunk(self):
                p, s = divmod(self.k, CPP)
                if s == 0:
                    self.cur = self.pool.tile([P, CPP, self.elem], bf16,
                                              tag=f"gb_{self.kind}")
                    nidx = CPP * P
                    nc.gpsimd.dma_gather(
                        self.cur[:], self.table_ap,
                        self.idx_t[:, p * (nidx // 16):(p + 1) * (nidx // 16)],
                        nidx, nidx, self.elem)
                rows = self.cur[:, s, :]
                segc = (self.seg_t[:, self.k:self.k + 1]
                        if self.seg_t is not None else None)
                self.k += 1
                return rows, segc

        # ==================================================== phase 1: linl+gn
        if maxphase >= 1:
            xl1ns = loc.tile([P, VTILES, H], bf16, tag="xl1ns")
            with tc.tile_pool(name="p1", bufs=1) as p1, \
                 tc.tile_pool(name="w1", bufs=2) as w1, \
                 tc.tile_pool(name="ps1", bufs=1, space="PSUM") as ps1, \
                 tc.tile_pool(name="ps1g", bufs=1, space="PSUM") as ps1g:
                linlWT_t = cget("linlWT")
                linlb_t = cget("linlb_row")
                cedge_t = cget("cedge")
                xl1th = p1.tile([P, VTILES, H], bf16, tag="xl1th")
                gn_ps = ps1g.tile([NGRAPH, 2 * H], f32, tag="gn")
                for t in range(VTILES):
                    xlc = w1.tile([DL, P], f32, tag="xlc")
                    nc.sync.dma_start(xlc[:], D_IN["XlT"][:, t * P:(t + 1) * P])
                    mm = ps1.tile([P, H], f32, tag="mm1")
                    nc.tensor.matmul(mm[:], lhsT=xlc[:],
                                     rhs=linlWT_t[:], start=True, stop=True)
                    xl1r = w1.tile([P, H], f32, tag="xl1r")
                    nc.vector.tensor_tensor(out=xl1r[:], in0=mm[:],
                                            in1=linlb_t[:], op=OP.add)
                    sc2 = w1.tile([P, 2 * H], bf16, tag="sc2")
                    nc.scalar.activation(sc2[:, :H], xl1r[:], AF.Tanh)
                    nc.scalar.activation(sc2[:, H:], sc2[:, :H], AF.Square)
                    nc.vector.tensor_copy(xl1th[:, t, :], sc2[:, :H])
                    oh = onehot(w1, cedge_t[:, t:t + 1], NGRAPH, tag="ohg")
                    nc.tensor.matmul(gn_ps[:], lhsT=oh[:], rhs=sc2[:],
                                     start=(t == 0), stop=(t == VTILES - 1))
                gn_sb = w1.tile([NGRAPH, 2 * H], f32, tag="gnsb")
                nc.vector.tensor_copy(gn_sb[:], gn_ps[:])
                nc.sync.dma_start(gn_in[:, :], gn_sb[:])
                nc.gpsimd.collective_compute(
                    "AllReduce", OP.add, replica_groups=RG,
                    ins=[gn_in[:, :]], outs=[gn_out[:, :]])
                gn_glob = w1.tile([NGRAPH, 2 * H], f32, tag="gnglob")
                nc.sync.dma_start(gn_glob[:], gn_out[:, :])

                # per-graph affine: x*A + B
                rgc = cget("recipG_col")
                AB = p1.tile([NGRAPH, 2 * H], f32, tag="AB")
                mean_t = w1.tile([NGRAPH, H], f32, tag="gmean")
                nc.vector.tensor_scalar(out=mean_t[:], in0=gn_glob[:, :H],
                                        scalar1=rgc[:NGRAPH, :], scalar2=None,
                                        op0=OP.mult)
                ex2_t = w1.tile([NGRAPH, H], f32, tag="gex2")
                nc.vector.tensor_scalar(out=ex2_t[:], in0=gn_glob[:, H:],
                                        scalar1=rgc[:NGRAPH, :], scalar2=None,
                                        op0=OP.mult)
                var_t = w1.tile([NGRAPH, H], f32, tag="gvar")
                nc.vector.tensor_tensor(out=var_t[:], in0=mean_t[:],
                                        in1=mean_t[:], op=OP.mult)
                nc.vector.tensor_tensor(out=var_t[:], in0=var_t[:],
                                        in1=cget("ms2_row")[:NGRAPH, :],
                                        op=OP.mult)
                nc.vector.tensor_tensor(out=var_t[:], in0=ex2_t[:], in1=var_t[:],
                                        op=OP.subtract)
                sq_t = w1.tile([NGRAPH, H], f32, tag="gsq")
                nc.scalar.activation(sq_t[:], var_t[:], AF.Sqrt,
                                     bias=eps_col[:NGRAPH, :])
                rstd_t = w1.tile([NGRAPH, H], f32, tag="grstd")
                nc.vector.reciprocal(rstd_t[:], sq_t[:])
                nc.vector.tensor_tensor(out=AB[:, :H], in0=rstd_t[:],
                                        in1=cget("gnw_row")[:NGRAPH, :],
                                        op=OP.mult)
                t2 = w1.tile([NGRAPH, H], f32, tag="gt2")
                nc.vector.tensor_tensor(out=t2[:], in0=mean_t[:], in1=AB[:, :H],
                                        op=OP.mult)
                nc.vector.tensor_tensor(out=t2[:], in0=t2[:],
                                        in1=cget("ms_row")[:NGRAPH, :],
                                        op=OP.mult)
                nc.vector.tensor_tensor(out=AB[:, H:],
                                        in0=cget("gnb_row")[:NGRAPH, :],
                                        in1=t2[:], op=OP.subtract)
                AB_bf = p1.tile([NGRAPH, 2 * H], bf16, tag="AB_bf")
                nc.vector.tensor_copy(AB_bf[:], AB[:])

                # ------------------------------------------ phase 1b: gn apply, xw
                gatWTx_t = cget("gatWTx")
                gatWTx_bf = p1.tile([P, 2, 264], bf16, tag="gatWTx_bf")
                nc.vector.tensor_copy(gatWTx_bf[:], gatWTx_t[:])
                for t in range(VTILES):
                    oh = onehot(w1, cedge_t[:, t:t + 1], NGRAPH, tag="ohg")
                    ohT_ps = ps1.tile([NGRAPH, P], bf16, tag="ohTps")
                    nc.tensor.transpose(ohT_ps[:], oh[:], ident_bf[:])
                    ohT = w1.tile([NGRAPH, P], bf16, tag="ohT")
                    nc.vector.tensor_copy(ohT[:], ohT_ps[:])
                    abrows = ps1.tile([P, 2 * H], f32, tag="abrows")
                    nc.tensor.matmul(abrows[:], lhsT=ohT[:], rhs=AB_bf[:],
                                     start=True, stop=True)
                    tmp = w1.tile([P, H], f32, tag="xl1tmp")
                    nc.vector.tensor_tensor(out=tmp[:], in0=xl1th[:, t, :],
                                            in1=abrows[:, :H], op=OP.mult)
                    nc.vector.tensor_tensor(out=xl1ns[:, t, :], in0=tmp[:],
                                            in1=abrows[:, H:], op=OP.add)
                    xnT_ps = ps1.tile([P, 2, P], bf16, tag="xnTps")
                    nc.tensor.transpose(xnT_ps[:, 0, :], xl1ns[:, t, :P],
                                        ident_bf[:])
                    nc.tensor.transpose(xnT_ps[:, 1, :], xl1ns[:, t, P:],
                                        ident_bf[:])
                    xnT = w1.tile([P, 2, P], bf16, tag="xnT")
                    nc.vector.tensor_copy(xnT[:], xnT_ps[:])
                    xw_ps = ps1.tile([P, 264], f32, tag="xwps")
                    for k in range(2):
                        nc.tensor.matmul(xw_ps[:], lhsT=xnT[:, k, :],
                                         rhs=gatWTx_bf[:, k, :],
                                         start=(k == 0), stop=(k == 1))
                    xwt = w1.tile([P, 384], bf16, tag="xwt")
                    nc.gpsimd.memset(xwt[:, 260:384], 0.0)
                    nc.scalar.copy(xwt[:, :260], xw_ps[:, :260])
                    adt = w1.tile([P, 128], bf16, tag="adt")
                    nc.gpsimd.memset(adt[:, 4:128], 0.0)
                    nc.vector.tensor_copy(adt[:, 0:4], xw_ps[:, 260:264])
                    nc.sync.dma_start(xw_in[:, t, :], xwt[:])
                    nc.sync.dma_start(
                        ad_loc[:].rearrange("(p t) d -> p t d", p=P)[:, t, :],
                        adt[:])
                nc.gpsimd.collective_compute(
                    "AllGather", OP.bypass, replica_groups=RG,
                    ins=[xw_in[:, :, :]], outs=[XWtab[:, :]])

        # ======================================================== phase 2: MHA
        if maxphase >= 2:
            with tc.tile_pool(name="mha", bufs=1) as mp, \
                 tc.tile_pool(name="wm", bufs=3) as wm:
                lingWT_t = cget("lingWT")
                lingb_t = cget("lingb_col")
                WinT_t = cget("WinT")
                WinT_bf = mp.tile([P, 2, 3 * H], bf16, tag="WinT_bf")
                nc.vector.tensor_copy(WinT_bf[:], WinT_t[:])
                Winb_t = cget("Winb_col")
                NCH = cfg.NG // 512

                xg1T = mp.tile([P, 2, cfg.NG], bf16, tag="xg1T")
                with tc.tile_pool(name="xgp", bufs=2) as xgp, \
                     tc.tile_pool(name="psx", bufs=2, space="PSUM") as psx:
                    for ch in range(NCH):
                        xgc = xgp.tile([P, 2, 512], f32, tag="xgc")
                        nc.sync.dma_start(
                            xgc[:], D_IN["XgT"][:, :, ch * 512:(ch + 1) * 512])
                        for fb in range(2):
                            mm = psx.tile([P, 512], f32, tag="mmg")
                            for k in range(2):
                                nc.tensor.matmul(
                                    mm[:],
                                    lhsT=lingWT_t[:, k, fb * P:(fb + 1) * P],
                                    rhs=xgc[:, k, :],
                                    start=(k == 0), stop=(k == 1))
                            nc.scalar.activation(
                                xg1T[:, fb, ch * 512:(ch + 1) * 512], mm[:],
                                AF.Tanh, bias=lingb_t[:, fb:fb + 1])
                    xg1T_own = mp.tile([P, 2, GR], f32, tag="xg1T_own")
                    xgTo_t = xgp.tile([P, 2, GR], f32, tag="xgTo")
                    nc.sync.dma_start(xgTo_t[:], D_IN["XgT_own"][:, :, :])
                    for fb in range(2):
                        mm = psx.tile([P, GR], f32, tag="mmgo")
                        for k in range(2):
                            nc.tensor.matmul(
                                mm[:], lhsT=lingWT_t[:, k, fb * P:(fb + 1) * P],
                                rhs=xgTo_t[:, k, :], start=(k == 0), stop=(k == 1))
                        nc.scalar.activation(xg1T_own[:, fb, :], mm[:], AF.Tanh,
                                             bias=lingb_t[:, fb:fb + 1])

                kT = mp.tile([P, 2, cfg.NG], bf16, tag="kT")
                vext = mp.tile([P, NGT, HEADS, 65], bf16, tag="vext")
                nc.vector.memset(vext[:], 1.0)
                qT_own = mp.tile([P, 2, GR], bf16, tag="qT_own")
                xg1own = mp.tile([P, QT, DG], f32, tag="xg1own")
                with tc.tile_pool(name="psk", bufs=1, space="PSUM") as psk:
                    for fb in range(2):
                        mm = psk.tile([P, GR], f32, tag="qmm")
                        for k in range(2):
                            nc.tensor.matmul(
                                mm[:], lhsT=WinT_t[:, k, fb * P:(fb + 1) * P],
                                rhs=xg1T_own[:, k, :], start=(k == 0),
                                stop=(k == 1))
                        nc.scalar.activation(qT_own[:, fb, :], mm[:], AF.Identity,
                                             bias=Winb_t[:, fb:fb + 1])
                    for fb in range(2):
                        for ch in range(NCH):
                            mm = psk.tile([P, 512], f32, tag="kmm")
                            for k in range(2):
                                nc.tensor.matmul(
                                    mm[:],
                                    lhsT=WinT_bf[:, k, (2 + fb) * P:(3 + fb) * P],
                                    rhs=xg1T[:, k, ch * 512:(ch + 1) * 512],
                                    start=(k == 0), stop=(k == 1))
                            nc.scalar.activation(
                                kT[:, fb, ch * 512:(ch + 1) * 512], mm[:],
                                AF.Identity, bias=Winb_t[:, 2 + fb:3 + fb])
                    for fb in range(2):
                        for ch in range(NCH):
                            mm = psk.tile([P, 512], f32, tag="vmm")
                            for k in range(2):
                                nc.tensor.matmul(
                                    mm[:],
                                    lhsT=WinT_bf[:, k, (4 + fb) * P:(5 + fb) * P],
                                    rhs=xg1T[:, k, ch * 512:(ch + 1) * 512],
                                    start=(k == 0), stop=(k == 1))
                            vsb = wm.tile([P, 512], f32, tag="vsb")
                            nc.scalar.activation(vsb[:], mm[:], AF.Identity,
                                                 bias=Winb_t[:, 4 + fb:5 + fb])
                            for hh in range(2):
                                h = fb * 2 + hh
                                for j in range(4):
                                    kt = ch * 4 + j
                                    tp = psk.tile([P, 64], f32, tag="vtp")
                                    nc.tensor.transpose(
                                        tp[:],
                                        vsb[hh * 64:(hh + 1) * 64,
                                            j * P:(j + 1) * P],
                                        ident_t[hh * 64:(hh + 1) * 64,
                                                hh * 64:(hh + 1) * 64])
                                    nc.vector.tensor_copy(
                                        vext[:, kt, h, :64], tp[:])
                    for qt in range(QT):
                        for k in range(2):
                            tp = psk.tile([P, P], f32, tag="xg1ownT")
                            nc.tensor.transpose(
                                tp[:], xg1T_own[:, k, qt * P:(qt + 1) * P],
                                ident_t[:])
                            nc.vector.tensor_copy(
                                xg1own[:, qt, k * P:(k + 1) * P], tp[:])

                attnout = mp.tile([P, QT, H], f32, tag="attnout")
                with tc.tile_pool(name="expp", bufs=1) as expp, \
                     tc.tile_pool(name="psS", bufs=2, space="PSUM") as psS, \
                     tc.tile_pool(name="psAV", bufs=1, space="PSUM") as psAV:
                    HNGT = NGT // 2
                    for h in range(HEADS):
                        kT_h = kT[64 * (h % 2):64 * (h % 2) + 64, h // 2, :]
                        qT_h = qT_own[64 * (h % 2):64 * (h % 2) + 64, h // 2, :]
                        avs = []
                        for qt in range(QT):
                            av_t = psAV.tile([P, 65], f32, tag=f"av{qt}")
                            avs.append(av_t)
                        for half in range(2):
                            expS = expp.tile([P, HNGT, GR], bf16, tag="expS")
                            for kt in range(HNGT):
                                ktg = half * HNGT + kt
                                s_ps = psS.tile([P, GR], f32, tag="sps")
                                nc.tensor.matmul(
                                    s_ps[:], lhsT=kT_h[:, ktg * P:(ktg + 1) * P],
                                    rhs=qT_h[:, :], start=True, stop=True)
                                nc.scalar.activation(expS[:, kt, :], s_ps[:],
                                                     AF.Exp, scale=0.125)
                            for qt in range(QT):
                                for kt in range(HNGT):
                                    ktg = half * HNGT + kt
                                    nc.tensor.matmul(
                                        avs[qt][:],
                                        lhsT=expS[:, kt, qt * P:(qt + 1) * P],
                                        rhs=vext[:, ktg, h, :],
                                        start=(ktg == 0), stop=(ktg == NGT - 1))
                        for qt in range(QT):
                            rc = wm.tile([P, 1], f32, tag="avrc")
                            nc.vector.reciprocal(rc[:], avs[qt][:, 64:65])
                            nc.vector.tensor_scalar(
                                out=attnout[:, qt, h * 64:(h + 1) * 64],
                                in0=avs[qt][:, :64], scalar1=rc[:], scalar2=None,
                                op0=OP.mult)

                # out proj + residual + LN (batched sqrt) + tanh -> staging
                WoutT_t = cget("WoutT")
                scoreWg_t = cget("scoreW_g")
                swg_bf = mp.tile([P, 5, H], bf16, tag="swg_bf")
                nc.vector.tensor_copy(swg_bf[:], scoreWg_t[:])
                gst = mp.tile([P, cfg.GTILES, 384], bf16, tag="gst")
                nc.gpsimd.memset(gst[:, :, 258:384], 0.0)
                xcst = mp.tile([P, QT, H], f32, tag="xcst")
                ss2st = mp.tile([P, QT], f32, tag="ss2st")
                with tc.tile_pool(name="pso", bufs=2, space="PSUM") as pso:
                    for qt in range(QT):
                        aT = wm.tile([P, 2, P], f32, tag="aT")
                        for k in range(2):
                            tp = pso.tile([P, P], f32, tag="aTps")
                            nc.tensor.transpose(
                                tp[:], attnout[:, qt, k * P:(k + 1) * P],
                                ident_t[:])
                            nc.vector.tensor_copy(aT[:, k, :], tp[:])
                        o_ps = pso.tile([P, H], f32, tag="ops")
                        for k in range(2):
                            nc.tensor.matmul(o_ps[:], lhsT=aT[:, k, :],
                                             rhs=WoutT_t[:, k, :],
                                             start=(k == 0), stop=(k == 1))
                        xs = wm.tile([P, H], f32, tag="xs")
                        nc.vector.tensor_tensor(out=xs[:], in0=o_ps[:],
                                                in1=cget("Woutb_row")[:],
                                                op=OP.add)
                        nc.vector.tensor_tensor(out=xs[:], in0=xs[:],
                                                in1=xg1own[:, qt, :], op=OP.add)
                        ssum = wm.tile([P, 1], f32, tag="ssum")
                        nc.vector.tensor_reduce(out=ssum[:], in_=xs[:], axis=AX.X,
                                                op=OP.add)
                        mu = wm.tile([P, 1], f32, tag="mu")
                        nc.vector.tensor_scalar(out=mu[:], in0=ssum[:],
                                                scalar1=1.0 / H, scalar2=None,
                                                op0=OP.mult)
                        nc.vector.tensor_scalar(out=xcst[:, qt, :], in0=xs[:],
                                                scalar1=mu[:], scalar2=None,
                                                op0=OP.subtract)
                        scr = wm.tile([P, H], bf16, tag="lnscr")
                        nc.scalar.activation(scr[:], xcst[:, qt, :], AF.Square,
                                             accum_out=ss2st[:, qt:qt + 1])
                    sqv = wm.tile([P, QT], f32, tag="sqv")
                    nc.scalar.activation(sqv[:], ss2st[:], AF.Sqrt,
                                         bias=eps_col[:], scale=1.0 / H)
                    rstd = wm.tile([P, QT], f32, tag="rstdq")
                    nc.vector.reciprocal(rstd[:], sqv[:])
                    qtmp = wm.tile([P, 2], f32, tag="qtmp")
                    for qt in range(QT):
                        xn = wm.tile([P, H], f32, tag="xn")
                        nc.vector.tensor_scalar(out=xn[:], in0=xcst[:, qt, :],
                                                scalar1=rstd[:, qt:qt + 1],
                                                scalar2=None, op0=OP.mult)
                        nc.vector.tensor_tensor(out=xn[:], in0=xn[:],
                                                in1=cget("normgw_row")[:],
                                                op=OP.mult)
                        nc.vector.tensor_tensor(out=xn[:], in0=xn[:],
                                                in1=cget("normgb_row")[:],
                                                op=OP.add)
                        nc.scalar.activation(gst[:, qt, :H], xn[:], AF.Tanh)
                        pq = wm.tile([P, H], bf16, tag="pq")
                        scr2 = wm.tile([P, H], bf16, tag="scr2")
                        nc.vector.tensor_tensor(out=pq[:], in0=gst[:, qt, :H],
                                                in1=swg_bf[:, 0, :], op=OP.mult)
                        ttracc(scr2[:], pq[:], gst[:, qt, :H], qtmp[:, 0:1])
                        nc.vector.tensor_tensor(out=pq[:], in0=gst[:, qt, :H],
                                                in1=swg_bf[:, 1, :], op=OP.mult)
                        ttracc(scr2[:], pq[:], gst[:, qt, :H], qtmp[:, 1:2])
                        nc.vector.tensor_copy(gst[:, qt, H:258], qtmp[:])
                nc.sync.dma_start(g_in[:, :, :], gst[:])
                nc.gpsimd.collective_compute(
                    "AllGather", OP.bypass, replica_groups=RG,
                    ins=[g_in[:, :, :]], outs=[Gtab[:, :]])

        # ======================================================== phase 4: MLP
        def mlp_pass(p4_range):
            with tc.tile_pool(name="mlpp", bufs=1) as mlpp, \
                 tc.tile_pool(name="wp", bufs=3) as wp, \
                 tc.tile_pool(name="psm", bufs=2, space="PSUM") as psm:
                W1T_t = cget("W1T")
                b1_t = cget("b1_row")
                wl_t = cget("wl_row")
                woh_t = cget("woh_col")
                xeT_t = mlpp.tile([P, 2, cfg.VPAD], f32, tag="xeT")
                nc.sync.dma_start(xeT_t[:], D_IN["XeT"][:, :, :])
                for t in range(*p4_range):
                    mm = psm.tile([P, HID], f32, tag="mmp")
                    for k in range(2):
                        nc.tensor.matmul(mm[:],
                                         lhsT=xeT_t[:, k, t * P:(t + 1) * P],
                                         rhs=W1T_t[:, k, :],
                                         start=(k == 0), stop=(k == 1))
                    h1 = wp.tile([P, HID], f32, tag="h1")
                    nc.vector.tensor_tensor(out=h1[:], in0=mm[:], in1=b1_t[:],
                                            op=OP.add)
                    s1 = wp.tile([P, 1], f32, tag="ps1c")
                    nc.scalar.activation(h1[:], h1[:], AF.Relu, accum_out=s1[:])
                    scr = wp.tile([P, HID], bf16, tag="pscr")
                    ss2 = wp.tile([P, 1], f32, tag="pss2")
                    nc.scalar.activation(scr[:], h1[:], AF.Square,
                                         accum_out=ss2[:])
                    sw1 = wp.tile([P, 1], f32, tag="psw1")
                    nc.vector.tensor_tensor_reduce(
                        out=scr[:], in0=h1[:], in1=wl_t[:], scale=1.0,
                        scalar=0.0, op0=OP.mult, op1=OP.add, accum_out=sw1[:])
                    musq = wp.tile([P, 1], f32, tag="pmusq")
                    nc.vector.tensor_scalar(
                        out=musq[:], in0=s1[:], scalar1=s1[:],
                        scalar2=1.0 / (HID * HID), op0=OP.mult, op1=OP.mult)
                    v1 = wp.tile([P, 1], f32, tag="pv1")
                    nc.vector.tensor_scalar(out=v1[:], in0=ss2[:],
                                            scalar1=1.0 / HID, scalar2=1e-5,
                                            op0=OP.mult, op1=OP.add)
                    nc.vector.tensor_tensor(out=varst[:, t:t + 1], in0=v1[:],
                                            in1=musq[:], op=OP.subtract)
                    amu = wp.tile([P, 1], f32, tag="pamu")
                    nc.vector.tensor_scalar(out=amu[:], in0=s1[:],
                                            scalar1=woh_t[:], scalar2=None,
                                            op0=OP.mult)
                    nc.vector.tensor_tensor(out=Ast[:, t:t + 1], in0=sw1[:],
                                            in1=amu[:], op=OP.subtract)

        if maxphase >= 4:
            mlp_pass((0, ETILES // 2))

        # ======================================================== phase 3: GAT
        if maxphase >= 3:
            with tc.tile_pool(name="gatp", bufs=1) as gp, \
                 tc.tile_pool(name="wg", bufs=3) as wg, \
                 tc.tile_pool(name="gbuf", bufs=2) as gbuf, \
                 tc.tile_pool(name="psg", bufs=2, space="PSUM") as psg:
                gatb_t = cget("gatb_row")
                scoreWl_t = cget("scoreW_l")
                swl_bf = gp.tile([P, 5, H], bf16, tag="swl_bf")
                nc.vector.tensor_copy(swl_bf[:], scoreWl_t[:])
                xlo = GatherList("GAT_Xlo", XWtab[:GHALF, :], 384, gbuf, gp)
                xhi = GatherList("GAT_Xhi", XWtab[GHALF:, :], 384, gbuf, gp)
                adlo = GatherList("GAT_ADlo", ad_loc[:, :], 128, gbuf, gp,
                                  idx_kind="GAT_Xlo")
                adhi = GatherList("GAT_ADhi", ad_loc[:, :], 128, gbuf, gp,
                                  idx_kind="GAT_Xhi")

                cnt_lo = meta["shared_cnt"]["GAT_Xlo"]
                cnt_hi = meta["shared_cnt"]["GAT_Xhi"]
                ex_pid, ex_tile = {}, {}
                qtmp3 = gp.tile([P, 2], f32, tag="qtmp3")
                for ot in range(VTILES):
                    acc = psg.tile([P, 260], f32, tag="gatacc")
                    total = int(cnt_lo[ot]) + int(cnt_hi[ot])
                    done = 0
                    for gl, adl, cnt in ((xlo, adlo, int(cnt_lo[ot])),
                                         (xhi, adhi, int(cnt_hi[ot]))):
                        for _ in range(cnt):
                            rows, segc = gl.chunk()
                            adrows, _ = adl.chunk()
                            pid = (gl.k - 1) // CPP
                            if ex_pid.get(gl.kind) != pid:
                                ex_pid[gl.kind] = pid
                                ext = wg.tile([P, CPP, HEADS], f32,
                                              tag=f"ex_{gl.kind}")
                                nc.vector.tensor_tensor(
                                    out=ext[:], in0=gl.cur[:, :, 256:260],
                                    in1=adl.cur[:, :, 0:HEADS], op=OP.add)
                                t02 = wg.tile([P, CPP, HEADS], f32, tag="t02")
                                nc.vector.tensor_scalar(out=t02[:], in0=ext[:],
                                                        scalar1=0.2,
                                                        scalar2=None,
                                                        op0=OP.mult)
                                nc.vector.tensor_tensor(out=ext[:], in0=t02[:],
                                                        in1=ext[:], op=OP.max)
                                nc.scalar.activation(ext[:], ext[:], AF.Exp)
                                ex_tile[gl.kind] = ext
                            s = (gl.k - 1) % CPP
                            ex = ex_tile[gl.kind][:, s, :]
                            rhs = wg.tile([P, 260], bf16, tag="gatrhs")
                            for h in range(2):
                                nc.vector.tensor_scalar(
                                    out=rhs[:, h * 64:(h + 1) * 64],
                                    in0=rows[:, h * 64:(h + 1) * 64],
                                    scalar1=ex[:, h:h + 1], scalar2=None,
                                    op0=OP.mult)
                            for h in range(2, HEADS):
                                nc.scalar.activation(
                                    rhs[:, h * 64:(h + 1) * 64],
                                    rows[:, h * 64:(h + 1) * 64],
                                    AF.Copy, scale=ex[:, h:h + 1])
                            nc.vector.tensor_copy(rhs[:, 256:260], ex[:])
                            oh = onehot(wg, segc, tag="ohgat")
                            nc.tensor.matmul(acc[:], lhsT=oh[:], rhs=rhs[:],
                                             start=(done == 0),
                                             stop=(done == total - 1))
                            done += 1
                    dene = wg.tile([P, HEADS], f32, tag="dene")
                    nc.vector.tensor_scalar(out=dene[:], in0=acc[:, 256:260],
                                            scalar1=1e-20, scalar2=None,
                                            op0=OP.add)
                    rc = wg.tile([P, HEADS], f32, tag="gatrc")
                    nc.vector.reciprocal(rc[:], dene[:])
                    xl2 = wg.tile([P, H], f32, tag="xl2")
                    for h in range(HEADS):
                        nc.vector.tensor_scalar(
                            out=xl2[:, h * 64:(h + 1) * 64],
                            in0=acc[:, h * 64:(h + 1) * 64],
                            scalar1=rc[:, h:h + 1], scalar2=None, op0=OP.mult)
                    nc.vector.tensor_tensor(out=xl2[:], in0=xl2[:], in1=gatb_t[:],
                                            op=OP.add)
                    nc.vector.tensor_tensor(out=xl2[:], in0=xl2[:],
                                            in1=xl1ns[:, ot, :], op=OP.add)
                    xlst = wg.tile([P, 384], bf16, tag="xlst")
                    nc.gpsimd.memset(xlst[:, 258:384], 0.0)
                    nc.scalar.activation(xlst[:, :H], xl2[:], AF.Tanh)
                    pq = wg.tile([P, H], bf16, tag="pq3")
                    scr3 = wg.tile([P, H], bf16, tag="scr3")
                    nc.vector.tensor_tensor(out=pq[:], in0=xlst[:, :H],
                                            in1=swl_bf[:, 0, :], op=OP.mult)
                    ttracc(scr3[:], pq[:], xlst[:, :H], qtmp3[:, 0:1])
                    nc.vector.tensor_tensor(out=pq[:], in0=xlst[:, :H],
                                            in1=swl_bf[:, 1, :], op=OP.mult)
                    ttracc(scr3[:], pq[:], xlst[:, :H], qtmp3[:, 1:2])
                    nc.vector.tensor_copy(xlst[:, H:258], qtmp3[:])
                    nc.sync.dma_start(l_in[:, ot, :], xlst[:])
                nc.gpsimd.collective_compute(
                    "AllGather", OP.bypass, replica_groups=RG,
                    ins=[l_in[:, :, :]], outs=[Ltab[:, :]])

        mid.close()

        # ====================================================== phase 5: score
        def score_pass(br, lists, sw_bf, dw_bf, zst, nst, nnst, ws, psacc,
                       psep, rcols_t):
            cnts = meta["shared_cnt"]
            for ot in range(ETILES):
                accs = {}
                for side in "TH":
                    acc = psacc.tile([P, 258], f32, tag=f"acc{side}{br}")
                    kinds = ([f"{side}_G"] if br == "G"
                             else [f"{side}_Llo", f"{side}_Lhi"])
                    total = sum(int(cnts[k][ot]) for k in kinds)
                    done = 0
                    for k in kinds:
                        for _ in range(int(cnts[k][ot])):
                            rows, segc = lists[k].chunk()
                            oh = onehot(ws, segc, tag="ohs")
                            nc.tensor.matmul(
                                acc[:], lhsT=oh[:], rhs=rows[:, :258],
                                start=(done == 0), stop=(done == total - 1))
                            done += 1
                    accs[side] = acc
                accT, accH = accs["T"], accs["H"]
                smT = ws.tile([P, H], bf16, tag="smT")
                nc.scalar.copy(smT[:], accT[:, :H])
                smH = ws.tile([P, H], bf16, tag="smH")
                nc.scalar.copy(smH[:], accH[:, :H])
                zbuf = ws.tile([P, 5], f32, tag="zbuf")
                pq = ws.tile([P, H], bf16, tag="pqs")
                scr = ws.tile([P, H], bf16, tag="scrs")
                nc.vector.tensor_tensor(out=pq[:], in0=smT[:],
                                        in1=sw_bf[:, 2, :], op=OP.mult)
                nc.vector.tensor_tensor_reduce(
                    out=scr[:], in0=pq[:], in1=smT[:], scale=1.0, scalar=0.0,
                    op0=OP.mult, op1=OP.add, accum_out=zbuf[:, 0:1])
                nc.vector.tensor_tensor(out=pq[:], in0=smH[:],
                                        in1=sw_bf[:, 3, :], op=OP.mult)
                nc.vector.tensor_tensor_reduce(
                    out=scr[:], in0=pq[:], in1=smH[:], scale=1.0, scalar=0.0,
                    op0=OP.mult, op1=OP.add, accum_out=zbuf[:, 1:2])
                nc.vector.tensor_tensor(out=pq[:], in0=smT[:],
                                        in1=sw_bf[:, 4, :], op=OP.mult)
                nc.vector.tensor_tensor_reduce(
                    out=scr[:], in0=pq[:], in1=smH[:], scale=1.0, scalar=0.0,
                    op0=OP.mult, op1=OP.add, accum_out=zbuf[:, 2:3])
                nc.vector.tensor_copy(zbuf[:, 3:4], accT[:, 256:257])
                nc.vector.tensor_copy(zbuf[:, 4:5], accH[:, 257:258])
                scr5 = ws.tile([P, 5], f32, tag="scr5")
                ttracc(scr5[:], zbuf[:], rcols_t[:, :, ot], zst[:, ot:ot + 1])
                # De: a = smT @ DW.T (scale-invariant)
                mtT = ws.tile([P, 2, P], bf16, tag="mtT")
                for k in range(2):
                    tp = psep.tile([P, P], bf16, tag="mtTps")
                    nc.tensor.transpose(tp[:], smT[:, k * P:(k + 1) * P],
                                        ident_bf[:])
                    nc.vector.tensor_copy(mtT[:, k, :], tp[:])
                a_ps = psep.tile([P, H], f32, tag="aps")
                for k in range(2):
                    nc.tensor.matmul(a_ps[:], lhsT=mtT[:, k, :],
                                     rhs=dw_bf[:, k, :],
                                     start=(k == 0), stop=(k == 1))
                nc.vector.tensor_tensor_reduce(
                    out=scr[:], in0=a_ps[:], in1=smH[:], scale=1.0, scalar=0.0,
                    op0=OP.mult, op1=OP.add, accum_out=nst[:, ot:ot + 1])
                na2 = ws.tile([P, 1], f32, tag="na2")
                nc.scalar.activation(scr[:], a_ps[:], AF.Square,
                                     accum_out=na2[:])
                nh2 = ws.tile([P, 1], f32, tag="nh2")
                nc.scalar.activation(scr[:], smH[:], AF.Square,
                                     accum_out=nh2[:])
                nn = ws.tile([P, 1], f32, tag="nn")
                nc.vector.tensor_tensor(out=nn[:], in0=na2[:], in1=nh2[:],
                                        op=OP.mult)
                nc.vector.tensor_scalar(out=nnst[:, ot:ot + 1], in0=nn[:],
                                        scalar1=1e-16, scalar2=None, op0=OP.max)

        if maxphase >= 5:
            rcols_t = cget("rcols")
            # ---- G pass (overlaps Ltab AllGather)
            with tc.tile_pool(name="scoG", bufs=1) as scoG, \
                 tc.tile_pool(name="gbufG", bufs=2) as gbufG, \
                 tc.tile_pool(name="wsG", bufs=2) as wsG, \
                 tc.tile_pool(name="psaG", bufs=2, space="PSUM") as psaG, \
                 tc.tile_pool(name="pseG", bufs=2, space="PSUM") as pseG:
                swg_bf2 = scoG.tile([P, 5, H], bf16, tag="swg_bf2")
                nc.vector.tensor_copy(swg_bf2[:], cget("scoreW_g")[:])
                dwg_bf = scoG.tile([P, 2, H], bf16, tag="dwg_bf")
                nc.vector.tensor_copy(dwg_bf[:], cget("DWTg")[:])
                listsG = {}
                for side in "TH":
                    listsG[f"{side}_G"] = GatherList(f"{side}_G", Gtab[:, :],
                                                     384, gbufG, scoG)
                score_pass("G", listsG, swg_bf2, dwg_bf, zstG, numG, nnG,
                           wsG, psaG, pseG, rcols_t)
            mlp_pass((ETILES // 2, ETILES))
            # ---- L pass
            with tc.tile_pool(name="scoL", bufs=1) as scoL, \
                 tc.tile_pool(name="gbufL", bufs=2) as gbufL, \
                 tc.tile_pool(name="wsL", bufs=2) as wsL, \
                 tc.tile_pool(name="psaL", bufs=2, space="PSUM") as psaL, \
                 tc.tile_pool(name="pseL", bufs=2, space="PSUM") as pseL:
                swl_bf2 = scoL.tile([P, 5, H], bf16, tag="swl_bf2")
                nc.vector.tensor_copy(swl_bf2[:], cget("scoreW_l")[:])
                dwl_bf = scoL.tile([P, 2, H], bf16, tag="dwl_bf")
                nc.vector.tensor_copy(dwl_bf[:], cget("DWTl")[:])
                listsL = {}
                for side in "TH":
                    listsL[f"{side}_Llo"] = GatherList(
                        f"{side}_Llo", Ltab[:GHALF, :], 384, gbufL, scoL)
                    listsL[f"{side}_Lhi"] = GatherList(
                        f"{side}_Lhi", Ltab[GHALF:, :], 384, gbufL, scoL)
                score_pass("L", listsL, swl_bf2, dwl_bf, zstL, numL, nnL,
                           wsL, psaL, pseL, rcols_t)

            # ---- batched finish
            with tc.tile_pool(name="fin", bufs=1) as fin:
                Qc = cfg.Q
                sqG = fin.tile([P, ETILES], f32, tag="sqG")
                nc.scalar.activation(sqG[:], nnG[:], AF.Sqrt)
                sqL = fin.tile([P, ETILES], f32, tag="sqL")
                nc.scalar.activation(sqL[:], nnL[:], AF.Sqrt)
                sqV = fin.tile([P, ETILES], f32, tag="sqV")
                nc.scalar.activation(sqV[:], varst[:], AF.Sqrt)
                rG = fin.tile([P, ETILES], f32, tag="rG")
                nc.vector.reciprocal(rG[:], sqG[:])
                rL = fin.tile([P, ETILES], f32, tag="rL")
                nc.vector.reciprocal(rL[:], sqL[:])
                r4 = fin.tile([P, ETILES], f32, tag="r4")
                nc.vector.reciprocal(r4[:], sqV[:])
                cosG = fin.tile([P, ETILES], f32, tag="cosG")
                nc.vector.tensor_tensor(out=cosG[:], in0=numG[:], in1=rG[:],
                                        op=OP.mult)
                cosL = fin.tile([P, ETILES], f32, tag="cosL")
                nc.vector.tensor_tensor(out=cosL[:], in0=numL[:], in1=rL[:],
                                        op=OP.mult)
                nc.vector.tensor_scalar(out=cosG[:], in0=cosG[:],
                                        scalar1=0.5 * Qc, scalar2=None,
                                        op0=OP.mult)
                nc.vector.tensor_scalar(out=cosL[:], in0=cosL[:],
                                        scalar1=0.5 * (1.0 - Qc), scalar2=None,
                                        op0=OP.mult)
                nc.vector.tensor_tensor(out=cosG[:], in0=cosG[:], in1=cosL[:],
                                        op=OP.add)
                nc.vector.tensor_scalar(out=de_stage[:], in0=cosG[:],
                                        scalar1=0.5, scalar2=None, op0=OP.add)
                zp = fin.tile([P, ETILES], f32, tag="zp")
                nc.vector.tensor_tensor(out=zp[:], in0=Ast[:], in1=r4[:],
                                        op=OP.mult)
                nc.scalar.activation(pe_stage[:], zp[:], AF.Sigmoid,
                                     bias=cget("c1b2_col")[:])
                seG = fin.tile([P, ETILES], f32, tag="seG")
                nc.scalar.activation(seG[:], zstG[:], AF.Sigmoid,
                                     bias=cget("sbg_col")[:])
                seL = fin.tile([P, ETILES], f32, tag="seL")
                nc.scalar.activation(seL[:], zstL[:], AF.Sigmoid,
                                     bias=cget("sbl_col")[:])
                nc.vector.tensor_scalar(out=seG[:], in0=seG[:], scalar1=Qc,
                                        scalar2=None, op0=OP.mult)
                nc.vector.tensor_scalar(out=seL[:], in0=seL[:],
                                        scalar1=1.0 - Qc, scalar2=None,
                                        op0=OP.mult)
                nc.vector.tensor_tensor(out=se_stage[:], in0=seG[:],
                                        in1=seL[:], op=OP.add)

        nc.sync.dma_start(Pe_out[:, :], pe_stage[:])
        nc.sync.dma_start(Se_out[:, :], se_stage[:])
        nc.sync.dma_start(De_out[:, :], de_stage[:])

    nc.compile()
    return nc


# ------------------------------------------------------------------- driver

_CACHE = {}

_JIT_CACHE = {}


def _pjrt_run(nc, in_maps, n_cores, repeats=1, chain=0):
    """Execute the compiled Bass module on n_cores via PJRT (axon), caching
    the jitted executable, optionally timing pipelined executions."""
    import time as _time
    import jax
    import concourse.mybir as mybir
    from concourse import bass2jax
    from jax.experimental.shard_map import shard_map
    from jax.sharding import Mesh, PartitionSpec

    bass2jax.install_neuronx_cc_hook()

    partition_name = (nc.partition_id_tensor.name
                      if nc.partition_id_tensor else None)
    in_names, out_names, out_avals = [], [], []
    for alloc in nc.m.functions[0].allocations:
        if not isinstance(alloc, bass2jax.mybir.MemoryLocationSet):
            continue
        name = alloc.memorylocations[0].name
        if alloc.kind == "ExternalInput":
            if name != partition_name:
                in_names.append(name)
        elif alloc.kind == "ExternalOutput":
            out_names.append(name)
            out_avals.append(jax.core.ShapedArray(
                tuple(alloc.tensor_shape), mybir.dt.np(alloc.dtype)))
    n_params = len(in_names)
    all_in = list(in_names) + list(out_names)
    if partition_name is not None:
        all_in.append(partition_name)

    key = (id(nc), chain)
    if key not in _JIT_CACHE:
        def _body(*args):
            operands = list(args)
            if partition_name is not None:
                operands.append(bass2jax.partition_id_tensor())
            outs = bass2jax._bass_exec_p.bind(
                *operands, out_avals=tuple(out_avals),
                in_names=tuple(all_in), out_names=tuple(out_names),
                lowering_input_output_aliases=(),
                sim_require_finite=True, sim_require_nnan=True, nc=nc)
            return tuple(outs)

        n_outs = len(out_names)
        nrep = max(1, chain)

        def _chained(*args):
            outs = None
            for i in range(nrep):
                zs = args[n_params + i * n_outs:n_params + (i + 1) * n_outs]
                outs = _body(*args[:n_params], *zs)
            return outs

        devices = jax.devices()[:n_cores]
        mesh = Mesh(np.asarray(devices), ("core",))
        donate = tuple(range(n_params, n_params + nrep * n_outs))
        _JIT_CACHE[key] = jax.jit(
            shard_map(_chained, mesh=mesh,
                      in_specs=(PartitionSpec("core"),) * (n_params +
                                                           nrep * n_outs),
                      out_specs=(PartitionSpec("core"),) * n_outs,
                      check_rep=False),
            donate_argnums=donate, keep_unused=True)
    fn = _JIT_CACHE[key]
    nrep = max(1, chain)

    from jax.sharding import NamedSharding
    devices = jax.devices()[:n_cores]
    mesh = Mesh(np.asarray(devices), ("core",))
    shd = NamedSharding(mesh, PartitionSpec("core"))
    concat_in = [jax.device_put(
        np.concatenate([np.asarray(in_maps[c][nm]) for c in range(n_cores)],
                       axis=0), shd) for nm in in_names]

    def zeros():
        return [jax.device_put(
            np.zeros((n_cores * a.shape[0], *a.shape[1:]), a.dtype), shd)
            for a in out_avals for _ in range(1)]

    def zchain():
        out = []
        for _ in range(nrep):
            out.extend(zeros())
        return out

    staged = [zchain() for _ in range(repeats)]
    out_arrs = jax.block_until_ready(fn(*concat_in, *staged[0]))
    best_ns = None
    if repeats > 1:
        t0 = _time.perf_counter()
        pend = [fn(*concat_in, *staged[r]) for r in range(1, repeats)]
        jax.block_until_ready(pend)
        best_ns = (_time.perf_counter() - t0) * 1e9 / (repeats - 1)
        out_arrs = pend[-1]
    results = [{nm: np.asarray(out_arrs[i]).reshape(
        n_cores, *out_avals[i].shape)[c] for i, nm in enumerate(out_names)}
        for c in range(n_cores)]
    return results, best_ns


def _run(cfg, inputs, sim=False, trace=False, maxphase=99, repeats=1,
         chain=0, reps=1):
    meta = preprocess(cfg, inputs["C_vertex"], inputs["C_edge"],
                      inputs["T_vertex"], inputs["H_vertex"],
                      inputs["T_edge"], inputs["H_edge"], inputs["e_index"])
    in_maps = prepare_inputs(cfg, inputs, meta)

    key = (cfg.NV, cfg.NE, cfg.NG,
           hash(np.asarray(inputs["T_edge"]).tobytes()) ^
           hash(np.asarray(inputs["e_index"]).tobytes()) ^
           hash(np.asarray(inputs["C_edge"]).tobytes()) ^
           hash(np.asarray(inputs["C_vertex"]).tobytes()) ^
           hash(np.asarray(inputs["T_vertex"]).tobytes()) ^
           hash(np.asarray(inputs["H_vertex"]).tobytes()) ^
           hash(np.asarray(inputs["H_edge"]).tobytes()))
    key = key + (maxphase, reps)
    if key not in _CACHE:
        _CACHE[key] = build(cfg, meta, maxphase, reps)
    nc = _CACHE[key]

    if sim:
        from concourse.bass_interp import MultiCoreSim
        s = MultiCoreSim(nc, cfg.NCORES, num_workers=cfg.NCORES,
                         ignore_data_errors=True)
        for c in range(cfg.NCORES):
            for k, v in in_maps[c].items():
                s.cores[c].tensor(k)[:] = v
        s.simulate()
        results = [{n: np.asarray(s.cores[c].tensor(n))
                    for n in ("Pe_out", "Se_out", "De_out")}
                   for c in range(cfg.NCORES)]
        exec_ns = None
    else:
        results, exec_ns = _pjrt_run(nc, in_maps, cfg.NCORES,
                                     repeats=repeats, chain=chain)

    def unpack(name):
        full = np.zeros((cfg.NE, 1), np.float32)
        for c in range(cfg.NCORES):
            vals = np.asarray(results[c][name])      # (128, ETILES)
            flat = vals.T.reshape(-1)[:cfg.ER]
            full[c * cfg.ER:(c + 1) * cfg.ER, 0] = flat
        return full

    return (unpack("Pe_out"), unpack("Se_out"), unpack("De_out")), exec_ns


def kernel(**inputs):
    (Pe, Se, De), _ = _run(FULL, inputs, sim=False)
    return Pe, Se, De


if __name__ == "__main__":
    pass


# revision 27
# speedup vs baseline: 2.0278x; 1.0033x over previous
"""Trainium2 Bass kernel for the DHMNN gnn_message_passing problem.

kernel(**inputs) takes FULL unsharded inputs, shards across 8 NeuronCores,
runs one SPMD Bass/Tile program, returns full (Pe, Se, De).

v2 design (vs baseline):
- Tables carry [x | q_u | q_v] (258 cols bf16) instead of [x | x^2] (512):
  the Se terms sum(u*x^2) are precomputed per vertex before the AllGather,
  halving AllGather bytes, gather DMA and score matmul width.
- Se computed via z = rT*A + rH*B + rT^2*C + rH^2*D + rT*rH*E with
  host-combined weight rows (u=sw0+sw2, v=sw1+sw3, w=sw2+sw3).
- De cosine is scale-invariant: segment sums used directly (no recip).
- Sqrt/Sigmoid batched at the end -> ~5 activation-table loads total.
- GAT att_src/att_dst folded into the gatW matmul (264-col weights).
- MLP LayerNorm folded algebraically into two staged columns.
- Score G-branch runs before/during the Ltab AllGather; CPP=12.
"""

import math
import numpy as np

P = 128


class Cfg:
    def __init__(self, NG, NV, NE, E, NT, NH, NGRAPH):
        self.NG, self.NV, self.NE, self.E, self.NT, self.NH = NG, NV, NE, E, NT, NH
        self.NGRAPH = NGRAPH
        self.DG, self.DL, self.H, self.HEADS, self.HID = 256, 128, 256, 4, 256
        self.D = self.H // self.HEADS
        self.Q = 0.5
        self.NCORES = 8
        self.VR = NV // self.NCORES
        self.ER = NE // self.NCORES
        self.GR = NG // self.NCORES
        self.VTILES = math.ceil(self.VR / P)
        self.ETILES = math.ceil(self.ER / P)
        self.GTILES = math.ceil(self.GR / P)
        self.VPAD = self.VTILES * P
        self.GHALF = (self.NCORES // 2) * self.VPAD
        self.NGT = NG // P            # global seq tiles (MHA keys)
        self.QT = self.GR // P        # own query tiles
        self.CPP = 8                  # gather chunks per dma_gather piece


FULL = Cfg(NG=4096, NV=50000, NE=50000, E=300000, NT=300000, NH=300000,
           NGRAPH=64)
SMALL = Cfg(NG=1024, NV=10000, NE=10000, E=60000, NT=60000, NH=60000,
            NGRAPH=64)

# ------------------------------------------------------------- layout helpers


def vhat(cfg, v):
    v = np.asarray(v)
    c, r = v // cfg.VR, v % cfg.VR
    p, t = r % P, r // P
    return c * cfg.VPAD + p * cfg.VTILES + t


def ghat(cfg, g):
    g = np.asarray(g)
    c, r = g // cfg.GR, g % cfg.GR
    p, t = r % P, r // P
    return c * cfg.GR + p * cfg.GTILES + t


def col_layout_f32(vals, lo, n_rows, ntiles):
    """(128, ntiles) f32: [p, t] = vals[lo + t*128 + p] (pad 0)."""
    arr = np.zeros(ntiles * P, np.float32)
    v = np.asarray(vals[lo:lo + n_rows], np.float32)
    arr[:len(v)] = v
    return np.ascontiguousarray(arr.reshape(ntiles, P).T)


def row_bcast(vec, n=P):
    return np.ascontiguousarray(
        np.tile(np.asarray(vec, np.float32).reshape(1, -1), (n, 1)))


def col_of(vec, nblk):
    v = np.asarray(vec, np.float32).reshape(nblk, P)
    return np.ascontiguousarray(v.T)


# --------------------------------------------------------------- preprocessing


def _tile_groups(seg, core_lo, core_hi, ntiles):
    order = np.argsort(seg, kind="stable")
    seg_s = seg[order]
    groups, rebased = [], []
    for o in range(ntiles):
        s0 = core_lo + P * o
        s1 = min(core_lo + P * (o + 1), core_hi)
        j0 = np.searchsorted(seg_s, s0)
        j1 = np.searchsorted(seg_s, s1)
        groups.append(order[j0:j1])
        rebased.append((seg_s[j0:j1] - s0).astype(np.float32))
    return groups, rebased


def _assemble(groups_per_core, shared_cnt, idx_fn, cpp):
    ntiles = len(shared_cnt)
    total_chunks = int(shared_cnt.sum())
    npieces = max(1, math.ceil(total_chunks / cpp))
    tot_rows = npieces * cpp * P
    out = []
    for rows_tiles, seg_tiles in groups_per_core:
        idx = np.zeros(tot_rows, np.int64)
        seg = np.full(tot_rows, -1.0, np.float32)
        pos = 0
        for o in range(ntiles):
            rows, sg = rows_tiles[o], seg_tiles[o]
            n = len(rows)
            idx[pos:pos + n] = idx_fn(rows)
            seg[pos:pos + n] = sg
            pos += int(shared_cnt[o]) * P
        out.append(dict(idx=idx, seg=seg))
    return out, npieces


def idx_i16(idx):
    """(128, n/16) int16 wrapped layout, replicated across partition groups."""
    idx = np.asarray(idx, np.int64)
    n = len(idx)
    assert n % 16 == 0
    arr = np.zeros((16, n // 16), np.int64)
    arr[np.arange(n) % 16, np.arange(n) // 16] = idx
    assert arr.max() < 32768 and arr.min() >= 0
    return np.ascontiguousarray(np.tile(arr.astype(np.int16), (8, 1)))


def seg_cols(seg):
    nch = len(seg) // P
    return np.ascontiguousarray(seg.reshape(nch, P).T)


def preprocess(cfg, C_vertex, C_edge, T_vertex, H_vertex, T_edge, H_edge,
               e_index):
    c64 = lambda x: np.asarray(x, np.int64)
    C_vertex, C_edge = c64(C_vertex), c64(C_edge)
    T_vertex, H_vertex = c64(T_vertex), c64(H_vertex)
    T_edge, H_edge, e_index = c64(T_edge), c64(H_edge), c64(e_index)

    cnt_T = np.bincount(T_edge, minlength=cfg.NE)
    cnt_H = np.bincount(H_edge, minlength=cfg.NE)
    recip_T = (1.0 / np.maximum(cnt_T, 1)).astype(np.float32)
    recip_H = (1.0 / np.maximum(cnt_H, 1)).astype(np.float32)
    gcnt = np.bincount(C_edge, minlength=cfg.NGRAPH)
    recip_G = (1.0 / np.maximum(gcnt, 1)).astype(np.float32)

    src2 = np.concatenate([e_index[0], np.arange(cfg.NV)])
    dst2 = np.concatenate([e_index[1], np.arange(cfg.NV)])

    meta = dict(lists={}, npieces={}, shared_cnt={})

    def groups_for(seg_arr, core_n, ntiles):
        out = []
        for c in range(cfg.NCORES):
            out.append(_tile_groups(seg_arr, c * core_n, (c + 1) * core_n,
                                    ntiles))
        return out

    def split_groups(groups, tab_fn, half, ntiles):
        res = []
        for c in range(cfg.NCORES):
            g_rows, g_segs = [], []
            for o in range(ntiles):
                rows, sg = groups[c][0][o], groups[c][1][o]
                tab = tab_fn(rows)
                sel = (tab < cfg.GHALF) if half == 0 else (tab >= cfg.GHALF)
                g_rows.append(rows[sel])
                g_segs.append(sg[sel])
            res.append((g_rows, g_segs))
        return res

    def shared_counts(groups, ntiles):
        cnts = np.zeros((cfg.NCORES, ntiles), np.int64)
        for c in range(cfg.NCORES):
            for o in range(ntiles):
                cnts[c, o] = math.ceil(len(groups[c][0][o]) / P)
        return cnts.max(0)

    def emit(name, groups, idx_fn, ntiles):
        shared = shared_counts(groups, ntiles)
        lists, npieces = _assemble(groups, shared, idx_fn, cfg.CPP)
        meta["lists"][name] = lists
        meta["npieces"][name] = npieces
        meta["shared_cnt"][name] = shared

    for side, (seg, gl) in dict(T=(T_edge, T_vertex),
                                H=(H_edge, H_vertex)).items():
        gg = C_vertex[gl]
        groups = groups_for(seg, cfg.ER, cfg.ETILES)
        emit(f"{side}_G", groups, lambda rows, gg=gg: ghat(cfg, gg[rows]),
             cfg.ETILES)
        tab_fn = lambda rows, gl=gl: vhat(cfg, gl[rows])
        for half, sfx in ((0, "lo"), (1, "hi")):
            sub = split_groups(groups, tab_fn, half, cfg.ETILES)
            off = 0 if half == 0 else cfg.GHALF
            emit(f"{side}_L{sfx}", sub,
                 lambda rows, off=off, tab_fn=tab_fn: tab_fn(rows) - off,
                 cfg.ETILES)

    groups = groups_for(dst2, cfg.VR, cfg.VTILES)
    tab_fn = lambda rows: vhat(cfg, src2[rows])
    for half, sfx in ((0, "lo"), (1, "hi")):
        sub = split_groups(groups, tab_fn, half, cfg.VTILES)
        off = 0 if half == 0 else cfg.GHALF
        emit(f"GAT_X{sfx}", sub,
             lambda rows, off=off: vhat(cfg, src2[rows]) - off, cfg.VTILES)
        ad_fn = lambda rows: vhat(cfg, dst2[rows])
        shared = meta["shared_cnt"][f"GAT_X{sfx}"]
        ad_lists, _ = _assemble(sub, shared, ad_fn, cfg.CPP)
        for c in range(cfg.NCORES):
            ad_lists[c]["idx"] = np.maximum(
                ad_lists[c]["idx"] - c * cfg.VPAD, 0)
        meta["lists"][f"GAT_AD{sfx}"] = ad_lists
        meta["npieces"][f"GAT_AD{sfx}"] = meta["npieces"][f"GAT_X{sfx}"]

    percore = []
    for c in range(cfg.NCORES):
        md = {}
        rT = col_layout_f32(recip_T, c * cfg.ER, cfg.ER, cfg.ETILES)
        rH = col_layout_f32(recip_H, c * cfg.ER, cfg.ER, cfg.ETILES)
        # rcols order matches zbuf [C, D, E, A, B]
        md["rcols"] = np.ascontiguousarray(
            np.stack([rT * rT, rH * rH, rT * rH, rT, rH], axis=1))
        ce = np.full(cfg.VPAD, -1.0, np.float32)
        ce[:cfg.VR] = C_edge[c * cfg.VR:(c + 1) * cfg.VR]
        md["cedge"] = np.ascontiguousarray(ce.reshape(cfg.VTILES, P).T)
        percore.append(md)
    meta["percore"] = percore
    meta["recip_G"] = recip_G
    return meta


# ----------------------------------------------------------- input preparation

LIST_KINDS = ["T_G", "T_Llo", "T_Lhi", "H_G", "H_Llo", "H_Lhi",
              "GAT_Xlo", "GAT_Xhi"]


def prepare_inputs(cfg, inputs, meta):
    f32 = np.float32
    H, DG, DL, HEADS, D = cfg.H, cfg.DG, cfg.DL, cfg.HEADS, cfg.D
    Xg = np.asarray(inputs["Xg"], f32)
    Xl = np.asarray(inputs["Xl"], f32)
    Xe = np.asarray(inputs["Xe"], f32)

    s = {}
    s["XgT"] = np.ascontiguousarray(Xg.T.reshape(2, P, -1).transpose(1, 0, 2))
    s["iota"] = row_bcast(np.arange(P, dtype=f32))
    s["ident"] = np.eye(P, dtype=f32)
    s["lingWT"] = np.ascontiguousarray(
        np.asarray(inputs["ling_W"], f32).T.reshape(2, P, H).transpose(1, 0, 2))
    s["lingb_col"] = col_of(inputs["ling_b"], 2)
    s["WinT"] = np.ascontiguousarray(
        np.asarray(inputs["mha_in_W"], f32).T.reshape(2, P, 3 * H)
        .transpose(1, 0, 2))
    s["Winb_col"] = col_of(inputs["mha_in_b"], 6)
    s["WoutT"] = np.ascontiguousarray(
        np.asarray(inputs["mha_out_W"], f32).T.reshape(2, P, H).transpose(1, 0, 2))
    s["Woutb_row"] = row_bcast(inputs["mha_out_b"])
    s["normgw_row"] = row_bcast(inputs["normg_w"])
    s["normgb_row"] = row_bcast(inputs["normg_b"])
    s["linlWT"] = np.ascontiguousarray(np.asarray(inputs["linl_W"], f32).T)
    s["linlb_row"] = row_bcast(inputs["linl_b"])
    ms = np.asarray(inputs["gn_ms"], f32)
    s["ms_row"] = row_bcast(ms)
    s["ms2_row"] = row_bcast(2.0 * ms - ms * ms)
    s["gnw_row"] = row_bcast(inputs["gn_w"])
    s["gnb_row"] = row_bcast(inputs["gn_b"])
    rg = np.zeros((P, 1), f32)
    rg[:cfg.NGRAPH, 0] = meta["recip_G"]
    s["recipG_col"] = rg

    # GAT weights extended with folded attention vectors:
    # xw = x @ W.T ; a_s = sum_d xw[h*64+d]*att_src[h,d] = x @ WAs[:,h]
    gw = np.asarray(inputs["gat_W"], f32)            # [H, H] (out, in)
    asrc = np.asarray(inputs["gat_att_src"], f32)    # [HEADS, D]
    adst = np.asarray(inputs["gat_att_dst"], f32)
    WAs = np.einsum("hdk,hd->kh", gw.reshape(HEADS, D, H), asrc)  # [H, HEADS]
    WAd = np.einsum("hdk,hd->kh", gw.reshape(HEADS, D, H), adst)
    gatx = np.concatenate([gw.T, WAs, WAd], axis=1)  # [H(in), 264]
    s["gatWTx"] = np.ascontiguousarray(
        gatx.reshape(2, P, 264).transpose(1, 0, 2))
    s["gatb_row"] = row_bcast(inputs["gat_b"])

    # score weight rows, order [u, v, wu, wv, w2m] per branch
    def score_rows(SW):
        sw = np.asarray(SW, f32).reshape(4, H)
        u, v, w = sw[0] + sw[2], sw[1] + sw[3], sw[2] + sw[3]
        return np.ascontiguousarray(np.stack(
            [row_bcast(u), row_bcast(v), row_bcast(w - u), row_bcast(w - v),
             row_bcast(-2.0 * w)], axis=1))
    s["scoreW_g"] = score_rows(inputs["lingS_W"])
    s["scoreW_l"] = score_rows(inputs["linlS_W"])
    s["sbg_col"] = np.full((P, 1), np.asarray(inputs["lingS_b"], f32)
                           .reshape(-1)[0], f32)
    s["sbl_col"] = np.full((P, 1), np.asarray(inputs["linlS_b"], f32)
                           .reshape(-1)[0], f32)
    s["DWTg"] = np.ascontiguousarray(
        np.asarray(inputs["lingD_W"], f32).T.reshape(2, P, H).transpose(1, 0, 2))
    s["DWTl"] = np.ascontiguousarray(
        np.asarray(inputs["linlD_W"], f32).T.reshape(2, P, H).transpose(1, 0, 2))

    # MLP folds
    s["W1T"] = np.ascontiguousarray(
        np.asarray(inputs["mlp_W1"], f32).T.reshape(2, P, cfg.HID).transpose(1, 0, 2))
    s["b1_row"] = row_bcast(inputs["mlp_b1"])
    lnw = np.asarray(inputs["mlp_ln_w"], f32)
    lnb = np.asarray(inputs["mlp_ln_b"], f32)
    w2 = np.asarray(inputs["mlp_W2"], f32).reshape(-1)
    wl = lnw * w2
    s["wl_row"] = row_bcast(wl)
    s["woh_col"] = np.full((P, 1), float(wl.sum()) / cfg.HID, f32)
    c1 = float((w2 * lnb).sum()) + float(np.asarray(inputs["mlp_b2"], f32)
                                         .reshape(-1)[0])
    s["c1b2_col"] = np.full((P, 1), c1, f32)

    in_maps = []
    for c in range(cfg.NCORES):
        md = meta["percore"][c]
        m = dict(s)
        m["XgT_own"] = np.ascontiguousarray(
            Xg[c * cfg.GR:(c + 1) * cfg.GR].T.reshape(2, P, -1)
            .transpose(1, 0, 2))
        xlp = np.zeros((cfg.VPAD, DL), f32)
        xlp[:cfg.VR] = Xl[c * cfg.VR:(c + 1) * cfg.VR]
        m["XlT"] = np.ascontiguousarray(xlp.T)
        xep = np.zeros((cfg.VPAD, DG), f32)
        xep[:cfg.ER] = Xe[c * cfg.ER:(c + 1) * cfg.ER]
        m["XeT"] = np.ascontiguousarray(xep.T.reshape(2, P, -1).transpose(1, 0, 2))
        m["rcols"] = md["rcols"]
        m["cedge"] = md["cedge"]
        for kind in LIST_KINDS:
            lst = meta["lists"][kind][c]
            m[f"idx_{kind}"] = idx_i16(lst["idx"])
            m[f"seg_{kind}"] = seg_cols(lst["seg"])
        for sfx in ("lo", "hi"):
            m[f"idx_GAT_AD{sfx}"] = idx_i16(
                meta["lists"][f"GAT_AD{sfx}"][c]["idx"])
        in_maps.append(m)
    return in_maps


# --------------------------------------------------------------------- builder


def build(cfg, meta, maxphase=99, reps=1):
    from contextlib import ExitStack
    import concourse.bacc as bacc
    import concourse.mybir as mybir
    import concourse.tile as tile

    f32 = mybir.dt.float32
    bf16 = mybir.dt.bfloat16
    i16 = mybir.dt.int16
    AF = mybir.ActivationFunctionType
    OP = mybir.AluOpType
    AX = mybir.AxisListType

    H, DG, DL, HEADS = cfg.H, cfg.DG, cfg.DL, cfg.HEADS
    NGRAPH, VTILES, ETILES, CPP = cfg.NGRAPH, cfg.VTILES, cfg.ETILES, cfg.CPP
    NGT, QT, GR, HID = cfg.NGT, cfg.QT, cfg.GR, cfg.HID

    nc = bacc.Bacc("TRN2", target_bir_lowering=False, debug=False,
                   num_devices=cfg.NCORES, dynamic_dma_scratch_size=32768)
    RG = [list(range(cfg.NCORES))]

    def din(name, shape, dtype=f32):
        return nc.dram_tensor(name, list(shape), dtype, kind="ExternalInput")

    D_IN = {}
    for name, shape, dt in [
        ("XgT", (P, 2, cfg.NG), f32), ("XgT_own", (P, 2, GR), f32),
        ("XlT", (DL, cfg.VPAD), f32), ("XeT", (P, 2, cfg.VPAD), f32),
        ("iota", (P, P), f32), ("ident", (P, P), f32),
        ("lingWT", (P, 2, H), f32), ("lingb_col", (P, 2), f32),
        ("WinT", (P, 2, 3 * H), f32), ("Winb_col", (P, 6), f32),
        ("WoutT", (P, 2, H), f32), ("Woutb_row", (P, H), f32),
        ("normgw_row", (P, H), f32), ("normgb_row", (P, H), f32),
        ("linlWT", (DL, H), f32), ("linlb_row", (P, H), f32),
        ("ms_row", (P, H), f32), ("ms2_row", (P, H), f32),
        ("gnw_row", (P, H), f32), ("gnb_row", (P, H), f32),
        ("recipG_col", (P, 1), f32),
        ("gatWTx", (P, 2, 264), f32), ("gatb_row", (P, H), f32),
        ("scoreW_g", (P, 5, H), f32), ("scoreW_l", (P, 5, H), f32),
        ("sbg_col", (P, 1), f32), ("sbl_col", (P, 1), f32),
        ("DWTg", (P, 2, H), f32), ("DWTl", (P, 2, H), f32),
        ("W1T", (P, 2, HID), f32), ("b1_row", (P, HID), f32),
        ("wl_row", (P, HID), f32), ("woh_col", (P, 1), f32),
        ("c1b2_col", (P, 1), f32),
        ("rcols", (P, 5, ETILES), f32),
        ("cedge", (P, VTILES), f32),
    ]:
        D_IN[name] = din(name, shape, dt)
    npieces = meta["npieces"]
    for kind in LIST_KINDS:
        npc = npieces[kind]
        D_IN[f"idx_{kind}"] = din(f"idx_{kind}", (P, npc * CPP * P // 16), i16)
        D_IN[f"seg_{kind}"] = din(f"seg_{kind}", (P, npc * CPP), f32)
    for sfx in ("lo", "hi"):
        npc = npieces[f"GAT_X{sfx}"]
        D_IN[f"idx_GAT_AD{sfx}"] = din(f"idx_GAT_AD{sfx}",
                                       (P, npc * CPP * P // 16), i16)

    Pe_out = nc.dram_tensor("Pe_out", [P, ETILES], f32, kind="ExternalOutput")
    Se_out = nc.dram_tensor("Se_out", [P, ETILES], f32, kind="ExternalOutput")
    De_out = nc.dram_tensor("De_out", [P, ETILES], f32, kind="ExternalOutput")

    gn_in = nc.dram_tensor("gn_in", [NGRAPH, 2 * H], f32)
    gn_out = nc.dram_tensor("gn_out", [NGRAPH, 2 * H], f32, addr_space="Shared")
    xw_in = nc.dram_tensor("xw_in", [P, VTILES, 384], bf16)
    XWtab = nc.dram_tensor("XWtab", [cfg.NCORES * cfg.VPAD, 384], bf16,
                           addr_space="Shared")
    ad_loc = nc.dram_tensor("ad_loc", [cfg.VPAD, 128], bf16)
    g_in = nc.dram_tensor("g_in", [P, cfg.GTILES, 384], bf16)
    Gtab = nc.dram_tensor("Gtab", [cfg.NG, 384], bf16, addr_space="Shared")
    l_in = nc.dram_tensor("l_in", [P, VTILES, 384], bf16)
    Ltab = nc.dram_tensor("Ltab", [cfg.NCORES * cfg.VPAD, 384], bf16,
                          addr_space="Shared")

    GHALF = cfg.GHALF

    with tile.TileContext(nc) as tc, ExitStack() as top:
        const = top.enter_context(tc.tile_pool(name="const", bufs=1))
        persist = top.enter_context(tc.tile_pool(name="persist", bufs=1))

        mid = ExitStack()
        loc = mid.enter_context(tc.tile_pool(name="loc", bufs=1))

        CT = {}

        def cget(name, pool=None):
            if name not in CT:
                d = D_IN[name]
                t = (pool or const).tile(list(d.shape), d.dtype, tag=name)
                nc.sync.dma_start(t[:], d[:])
                CT[name] = t
            return CT[name]

        iota_t = cget("iota")
        ident_t = cget("ident")
        iota_bf = const.tile([P, P], bf16, tag="iota_bf")
        nc.vector.tensor_copy(iota_bf[:], iota_t[:])
        ident_bf = const.tile([P, P], bf16, tag="ident_bf")
        nc.vector.tensor_copy(ident_bf[:], ident_t[:])

        pe_stage = persist.tile([P, ETILES], f32, tag="pe_stage")
        se_stage = persist.tile([P, ETILES], f32, tag="se_stage")
        de_stage = persist.tile([P, ETILES], f32, tag="de_stage")
        nc.vector.memset(pe_stage[:], 0.0)
        nc.vector.memset(se_stage[:], 0.0)
        nc.vector.memset(de_stage[:], 0.0)
        zstG = persist.tile([P, ETILES], f32, tag="zstG")
        zstL = persist.tile([P, ETILES], f32, tag="zstL")
        numG = persist.tile([P, ETILES], f32, tag="numG")
        numL = persist.tile([P, ETILES], f32, tag="numL")
        nnG = persist.tile([P, ETILES], f32, tag="nnG")
        nnL = persist.tile([P, ETILES], f32, tag="nnL")
        Ast = persist.tile([P, ETILES], f32, tag="Ast")
        varst = persist.tile([P, ETILES], f32, tag="varst")
        eps_col = const.tile([P, 1], f32, tag="eps_col")
        nc.vector.memset(eps_col[:], 1e-5)

        def onehot(pool, segc, ncols=P, tag="oh"):
            oh = pool.tile([P, ncols], bf16, tag=tag)
            nc.vector.tensor_scalar(out=oh[:], in0=iota_bf[:, :ncols],
                                    scalar1=segc, scalar2=None,
                                    op0=OP.is_equal)
            return oh

        class GatherList:
            def __init__(self, kind, table_ap, elem, pool, cpool,
                         idx_kind=None):
                self.kind = kind
                self.table_ap = table_ap
                self.elem = elem
                self.pool = pool
                ik = idx_kind or kind
                self.npc = npieces[ik if ik in npieces else kind]
                d_idx = D_IN[f"idx_{kind}"]
                self.idx_t = cpool.tile(list(d_idx.shape), i16,
                                        tag=f"idx_{kind}")
                nc.sync.dma_start(self.idx_t[:], d_idx[:])
                if f"seg_{kind}" in D_IN:
                    d_seg = D_IN[f"seg_{kind}"]
                    self.seg_t = cpool.tile(list(d_seg.shape), f32,
                                            tag=f"seg_{kind}")
                    nc.sync.dma_start(self.seg_t[:], d_seg[:])
                else:
                    self.seg_t = None
                self.k = 0
                self.cur = None

            def chunk(self):
                p, s = divmod(self.k, CPP)
                if s == 0:
                    self.cur = self.pool.tile([P, CPP, self.elem], bf16,
                                              tag=f"gb_{self.kind}")
                    nidx = CPP * P
                    nc.gpsimd.dma_gather(
                        self.cur[:], self.table_ap,
                        self.idx_t[:, p * (nidx // 16):(p + 1) * (nidx // 16)],
                        nidx, nidx, self.elem)
                rows = self.cur[:, s, :]
                segc = (self.seg_t[:, self.k:self.k + 1]
                        if self.seg_t is not None else None)
                self.k += 1
                return rows, segc

        # ==================================================== phase 1: linl+gn
        if maxphase >= 1:
            xl1ns = loc.tile([P, VTILES, H], bf16, tag="xl1ns")
            with tc.tile_pool(name="p1", bufs=1) as p1, \
                 tc.tile_pool(name="w1", bufs=3) as w1, \
                 tc.tile_pool(name="ps1", bufs=1, space="PSUM") as ps1, \
                 tc.tile_pool(name="ps1g", bufs=1, space="PSUM") as ps1g:
                linlWT_t = cget("linlWT")
                linlb_t = cget("linlb_row")
                cedge_t = cget("cedge")
                xl1th = p1.tile([P, VTILES, H], bf16, tag="xl1th")
                gn_ps = ps1g.tile([NGRAPH, 2 * H], f32, tag="gn")
                for t in range(VTILES):
                    xlc = w1.tile([DL, P], f32, tag="xlc")
                    nc.sync.dma_start(xlc[:], D_IN["XlT"][:, t * P:(t + 1) * P])
                    mm = ps1.tile([P, H], f32, tag="mm1")
                    nc.tensor.matmul(mm[:], lhsT=xlc[:],
                                     rhs=linlWT_t[:], start=True, stop=True)
                    xl1r = w1.tile([P, H], f32, tag="xl1r")
                    nc.vector.tensor_tensor(out=xl1r[:], in0=mm[:],
                                            in1=linlb_t[:], op=OP.add)
                    sc2 = w1.tile([P, 2 * H], bf16, tag="sc2")
                    nc.scalar.activation(sc2[:, :H], xl1r[:], AF.Tanh)
                    nc.scalar.activation(sc2[:, H:], sc2[:, :H], AF.Square)
                    nc.vector.tensor_copy(xl1th[:, t, :], sc2[:, :H])
                    oh = onehot(w1, cedge_t[:, t:t + 1], NGRAPH, tag="ohg")
                    nc.tensor.matmul(gn_ps[:], lhsT=oh[:], rhs=sc2[:],
                                     start=(t == 0), stop=(t == VTILES - 1))
                gn_sb = w1.tile([NGRAPH, 2 * H], f32, tag="gnsb")
                nc.vector.tensor_copy(gn_sb[:], gn_ps[:])
                nc.sync.dma_start(gn_in[:, :], gn_sb[:])
                nc.gpsimd.collective_compute(
                    "AllReduce", OP.add, replica_groups=RG,
                    ins=[gn_in[:, :]], outs=[gn_out[:, :]])
                gn_glob = w1.tile([NGRAPH, 2 * H], f32, tag="gnglob")
                nc.sync.dma_start(gn_glob[:], gn_out[:, :])

                # per-graph affine: x*A + B
                rgc = cget("recipG_col")
                AB = p1.tile([NGRAPH, 2 * H], f32, tag="AB")
                mean_t = w1.tile([NGRAPH, H], f32, tag="gmean")
                nc.vector.tensor_scalar(out=mean_t[:], in0=gn_glob[:, :H],
                                        scalar1=rgc[:NGRAPH, :], scalar2=None,
                                        op0=OP.mult)
                ex2_t = w1.tile([NGRAPH, H], f32, tag="gex2")
                nc.vector.tensor_scalar(out=ex2_t[:], in0=gn_glob[:, H:],
                                        scalar1=rgc[:NGRAPH, :], scalar2=None,
                                        op0=OP.mult)
                var_t = w1.tile([NGRAPH, H], f32, tag="gvar")
                nc.vector.tensor_tensor(out=var_t[:], in0=mean_t[:],
                                        in1=mean_t[:], op=OP.mult)
                nc.vector.tensor_tensor(out=var_t[:], in0=var_t[:],
                                        in1=cget("ms2_row")[:NGRAPH, :],
                                        op=OP.mult)
                nc.vector.tensor_tensor(out=var_t[:], in0=ex2_t[:], in1=var_t[:],
                                        op=OP.subtract)
                sq_t = w1.tile([NGRAPH, H], f32, tag="gsq")
                nc.scalar.activation(sq_t[:], var_t[:], AF.Sqrt,
                                     bias=eps_col[:NGRAPH, :])
                rstd_t = w1.tile([NGRAPH, H], f32, tag="grstd")
                nc.vector.reciprocal(rstd_t[:], sq_t[:])
                nc.vector.tensor_tensor(out=AB[:, :H], in0=rstd_t[:],
                                        in1=cget("gnw_row")[:NGRAPH, :],
                                        op=OP.mult)
                t2 = w1.tile([NGRAPH, H], f32, tag="gt2")
                nc.vector.tensor_tensor(out=t2[:], in0=mean_t[:], in1=AB[:, :H],
                                        op=OP.mult)
                nc.vector.tensor_tensor(out=t2[:], in0=t2[:],
                                        in1=cget("ms_row")[:NGRAPH, :],
                                        op=OP.mult)
                nc.vector.tensor_tensor(out=AB[:, H:],
                                        in0=cget("gnb_row")[:NGRAPH, :],
                                        in1=t2[:], op=OP.subtract)
                AB_bf = p1.tile([NGRAPH, 2 * H], bf16, tag="AB_bf")
                nc.vector.tensor_copy(AB_bf[:], AB[:])

                # ------------------------------------------ phase 1b: gn apply, xw
                gatWTx_t = cget("gatWTx")
                gatWTx_bf = p1.tile([P, 2, 264], bf16, tag="gatWTx_bf")
                nc.vector.tensor_copy(gatWTx_bf[:], gatWTx_t[:])
                for t in range(VTILES):
                    oh = onehot(w1, cedge_t[:, t:t + 1], NGRAPH, tag="ohg")
                    ohT_ps = ps1.tile([NGRAPH, P], bf16, tag="ohTps")
                    nc.tensor.transpose(ohT_ps[:], oh[:], ident_bf[:])
                    ohT = w1.tile([NGRAPH, P], bf16, tag="ohT")
                    nc.vector.tensor_copy(ohT[:], ohT_ps[:])
                    abrows = ps1.tile([P, 2 * H], f32, tag="abrows")
                    nc.tensor.matmul(abrows[:], lhsT=ohT[:], rhs=AB_bf[:],
                                     start=True, stop=True)
                    tmp = w1.tile([P, H], f32, tag="xl1tmp")
                    nc.vector.tensor_tensor(out=tmp[:], in0=xl1th[:, t, :],
                                            in1=abrows[:, :H], op=OP.mult)
                    nc.vector.tensor_tensor(out=xl1ns[:, t, :], in0=tmp[:],
                                            in1=abrows[:, H:], op=OP.add)
                    xnT_ps = ps1.tile([P, 2, P], bf16, tag="xnTps")
                    nc.tensor.transpose(xnT_ps[:, 0, :], xl1ns[:, t, :P],
                                        ident_bf[:])
                    nc.tensor.transpose(xnT_ps[:, 1, :], xl1ns[:, t, P:],
                                        ident_bf[:])
                    xnT = w1.tile([P, 2, P], bf16, tag="xnT")
                    nc.vector.tensor_copy(xnT[:], xnT_ps[:])
                    xw_ps = ps1.tile([P, 264], f32, tag="xwps")
                    for k in range(2):
                        nc.tensor.matmul(xw_ps[:], lhsT=xnT[:, k, :],
                                         rhs=gatWTx_bf[:, k, :],
                                         start=(k == 0), stop=(k == 1))
                    xwt = w1.tile([P, 384], bf16, tag="xwt")
                    nc.gpsimd.memset(xwt[:, 260:384], 0.0)
                    nc.scalar.copy(xwt[:, :260], xw_ps[:, :260])
                    adt = w1.tile([P, 128], bf16, tag="adt")
                    nc.gpsimd.memset(adt[:, 4:128], 0.0)
                    nc.vector.tensor_copy(adt[:, 0:4], xw_ps[:, 260:264])
                    nc.sync.dma_start(xw_in[:, t, :], xwt[:])
                    nc.sync.dma_start(
                        ad_loc[:].rearrange("(p t) d -> p t d", p=P)[:, t, :],
                        adt[:])
                nc.gpsimd.collective_compute(
                    "AllGather", OP.bypass, replica_groups=RG,
                    ins=[xw_in[:, :, :]], outs=[XWtab[:, :]])

        # ======================================================== phase 2: MHA
        if maxphase >= 2:
            with tc.tile_pool(name="mha", bufs=1) as mp, \
                 tc.tile_pool(name="wm", bufs=3) as wm:
                lingWT_t = cget("lingWT")
                lingb_t = cget("lingb_col")
                WinT_t = cget("WinT")
                WinT_bf = mp.tile([P, 2, 3 * H], bf16, tag="WinT_bf")
                nc.vector.tensor_copy(WinT_bf[:], WinT_t[:])
                Winb_t = cget("Winb_col")
                NCH = cfg.NG // 512

                xg1T = mp.tile([P, 2, cfg.NG], bf16, tag="xg1T")
                with tc.tile_pool(name="xgp", bufs=2) as xgp, \
                     tc.tile_pool(name="psx", bufs=2, space="PSUM") as psx:
                    for ch in range(NCH):
                        xgc = xgp.tile([P, 2, 512], f32, tag="xgc")
                        nc.sync.dma_start(
                            xgc[:], D_IN["XgT"][:, :, ch * 512:(ch + 1) * 512])
                        for fb in range(2):
                            mm = psx.tile([P, 512], f32, tag="mmg")
                            for k in range(2):
                                nc.tensor.matmul(
                                    mm[:],
                                    lhsT=lingWT_t[:, k, fb * P:(fb + 1) * P],
                                    rhs=xgc[:, k, :],
                                    start=(k == 0), stop=(k == 1))
                            nc.scalar.activation(
                                xg1T[:, fb, ch * 512:(ch + 1) * 512], mm[:],
                                AF.Tanh, bias=lingb_t[:, fb:fb + 1])
                    xg1T_own = mp.tile([P, 2, GR], f32, tag="xg1T_own")
                    xgTo_t = xgp.tile([P, 2, GR], f32, tag="xgTo")
                    nc.sync.dma_start(xgTo_t[:], D_IN["XgT_own"][:, :, :])
                    for fb in range(2):
                        mm = psx.tile([P, GR], f32, tag="mmgo")
                        for k in range(2):
                            nc.tensor.matmul(
                                mm[:], lhsT=lingWT_t[:, k, fb * P:(fb + 1) * P],
                                rhs=xgTo_t[:, k, :], start=(k == 0), stop=(k == 1))
                        nc.scalar.activation(xg1T_own[:, fb, :], mm[:], AF.Tanh,
                                             bias=lingb_t[:, fb:fb + 1])

                kT = mp.tile([P, 2, cfg.NG], bf16, tag="kT")
                vext = mp.tile([P, NGT, HEADS, 65], bf16, tag="vext")
                nc.vector.memset(vext[:], 1.0)
                qT_own = mp.tile([P, 2, GR], bf16, tag="qT_own")
                xg1own = mp.tile([P, QT, DG], f32, tag="xg1own")
                with tc.tile_pool(name="psk", bufs=1, space="PSUM") as psk:
                    for fb in range(2):
                        mm = psk.tile([P, GR], f32, tag="qmm")
                        for k in range(2):
                            nc.tensor.matmul(
                                mm[:], lhsT=WinT_t[:, k, fb * P:(fb + 1) * P],
                                rhs=xg1T_own[:, k, :], start=(k == 0),
                                stop=(k == 1))
                        nc.scalar.activation(qT_own[:, fb, :], mm[:], AF.Identity,
                                             bias=Winb_t[:, fb:fb + 1])
                    for fb in range(2):
                        for ch in range(NCH):
                            mm = psk.tile([P, 512], f32, tag="kmm")
                            for k in range(2):
                                nc.tensor.matmul(
                                    mm[:],
                                    lhsT=WinT_bf[:, k, (2 + fb) * P:(3 + fb) * P],
                                    rhs=xg1T[:, k, ch * 512:(ch + 1) * 512],
                                    start=(k == 0), stop=(k == 1))
                            nc.scalar.activation(
                                kT[:, fb, ch * 512:(ch + 1) * 512], mm[:],
                                AF.Identity, bias=Winb_t[:, 2 + fb:3 + fb])
                    for fb in range(2):
                        for ch in range(NCH):
                            mm = psk.tile([P, 512], f32, tag="vmm")
                            for k in range(2):
                                nc.tensor.matmul(
                                    mm[:],
                                    lhsT=WinT_bf[:, k, (4 + fb) * P:(5 + fb) * P],
                                    rhs=xg1T[:, k, ch * 512:(ch + 1) * 512],
                                    start=(k == 0), stop=(k == 1))
                            vsb = wm.tile([P, 512], f32, tag="vsb")
                            nc.scalar.activation(vsb[:], mm[:], AF.Identity,
                                                 bias=Winb_t[:, 4 + fb:5 + fb])
                            for hh in range(2):
                                h = fb * 2 + hh
                                for j in range(4):
                                    kt = ch * 4 + j
                                    tp = psk.tile([P, 64], f32, tag="vtp")
                                    nc.tensor.transpose(
                                        tp[:],
                                        vsb[hh * 64:(hh + 1) * 64,
                                            j * P:(j + 1) * P],
                                        ident_t[hh * 64:(hh + 1) * 64,
                                                hh * 64:(hh + 1) * 64])
                                    nc.vector.tensor_copy(
                                        vext[:, kt, h, :64], tp[:])
                    for qt in range(QT):
                        for k in range(2):
                            tp = psk.tile([P, P], f32, tag="xg1ownT")
                            nc.tensor.transpose(
                                tp[:], xg1T_own[:, k, qt * P:(qt + 1) * P],
                                ident_t[:])
                            nc.vector.tensor_copy(
                                xg1own[:, qt, k * P:(k + 1) * P], tp[:])

                attnout = mp.tile([P, QT, H], f32, tag="attnout")
                with tc.tile_pool(name="expp", bufs=1) as expp, \
                     tc.tile_pool(name="psS", bufs=2, space="PSUM") as psS, \
                     tc.tile_pool(name="psAV", bufs=1, space="PSUM") as psAV:
                    HNGT = NGT // 2
                    for h in range(HEADS):
                        kT_h = kT[64 * (h % 2):64 * (h % 2) + 64, h // 2, :]
                        qT_h = qT_own[64 * (h % 2):64 * (h % 2) + 64, h // 2, :]
                        avs = []
                        for qt in range(QT):
                            av_t = psAV.tile([P, 65], f32, tag=f"av{qt}")
                            avs.append(av_t)
                        for half in range(2):
                            expS = expp.tile([P, HNGT, GR], bf16, tag="expS")
                            for kt in range(HNGT):
                                ktg = half * HNGT + kt
                                s_ps = psS.tile([P, GR], f32, tag="sps")
                                nc.tensor.matmul(
                                    s_ps[:], lhsT=kT_h[:, ktg * P:(ktg + 1) * P],
                                    rhs=qT_h[:, :], start=True, stop=True)
                                nc.scalar.activation(expS[:, kt, :], s_ps[:],
                                                     AF.Exp, scale=0.125)
                            for qt in range(QT):
                                for kt in range(HNGT):
                                    ktg = half * HNGT + kt
                                    nc.tensor.matmul(
                                        avs[qt][:],
                                        lhsT=expS[:, kt, qt * P:(qt + 1) * P],
                                        rhs=vext[:, ktg, h, :],
                                        start=(ktg == 0), stop=(ktg == NGT - 1))
                        for qt in range(QT):
                            rc = wm.tile([P, 1], f32, tag="avrc")
                            nc.vector.reciprocal(rc[:], avs[qt][:, 64:65])
                            nc.vector.tensor_scalar(
                                out=attnout[:, qt, h * 64:(h + 1) * 64],
                                in0=avs[qt][:, :64], scalar1=rc[:], scalar2=None,
                                op0=OP.mult)

                # out proj + residual + LN (batched sqrt) + tanh -> staging
                WoutT_t = cget("WoutT")
                scoreWg_t = cget("scoreW_g")
                swg_bf = mp.tile([P, 5, H], bf16, tag="swg_bf")
                nc.vector.tensor_copy(swg_bf[:], scoreWg_t[:])
                gst = mp.tile([P, cfg.GTILES, 384], bf16, tag="gst")
                nc.gpsimd.memset(gst[:, :, 258:384], 0.0)
                xcst = mp.tile([P, QT, H], f32, tag="xcst")
                ss2st = mp.tile([P, QT], f32, tag="ss2st")
                with tc.tile_pool(name="pso", bufs=2, space="PSUM") as pso:
                    for qt in range(QT):
                        aT = wm.tile([P, 2, P], f32, tag="aT")
                        for k in range(2):
                            tp = pso.tile([P, P], f32, tag="aTps")
                            nc.tensor.transpose(
                                tp[:], attnout[:, qt, k * P:(k + 1) * P],
                                ident_t[:])
                            nc.vector.tensor_copy(aT[:, k, :], tp[:])
                        o_ps = pso.tile([P, H], f32, tag="ops")
                        for k in range(2):
                            nc.tensor.matmul(o_ps[:], lhsT=aT[:, k, :],
                                             rhs=WoutT_t[:, k, :],
                                             start=(k == 0), stop=(k == 1))
                        xs = wm.tile([P, H], f32, tag="xs")
                        nc.vector.tensor_tensor(out=xs[:], in0=o_ps[:],
                                                in1=cget("Woutb_row")[:],
                                                op=OP.add)
                        nc.vector.tensor_tensor(out=xs[:], in0=xs[:],
                                                in1=xg1own[:, qt, :], op=OP.add)
                        ssum = wm.tile([P, 1], f32, tag="ssum")
                        nc.vector.tensor_reduce(out=ssum[:], in_=xs[:], axis=AX.X,
                                                op=OP.add)
                        mu = wm.tile([P, 1], f32, tag="mu")
                        nc.vector.tensor_scalar(out=mu[:], in0=ssum[:],
                                                scalar1=1.0 / H, scalar2=None,
                                                op0=OP.mult)
                        nc.vector.tensor_scalar(out=xcst[:, qt, :], in0=xs[:],
                                                scalar1=mu[:], scalar2=None,
                                                op0=OP.subtract)
                        scr = wm.tile([P, H], bf16, tag="lnscr")
                        nc.scalar.activation(scr[:], xcst[:, qt, :], AF.Square,
                                             accum_out=ss2st[:, qt:qt + 1])
                    sqv = wm.tile([P, QT], f32, tag="sqv")
                    nc.scalar.activation(sqv[:], ss2st[:], AF.Sqrt,
                                         bias=eps_col[:], scale=1.0 / H)
                    rstd = wm.tile([P, QT], f32, tag="rstdq")
                    nc.vector.reciprocal(rstd[:], sqv[:])
                    for qt in range(QT):
                        qtmp = wm.tile([P, 2], f32, tag="qtmp")
                        xn = wm.tile([P, H], f32, tag="xn")
                        nc.vector.tensor_scalar(out=xn[:], in0=xcst[:, qt, :],
                                                scalar1=rstd[:, qt:qt + 1],
                                                scalar2=None, op0=OP.mult)
                        nc.vector.tensor_tensor(out=xn[:], in0=xn[:],
                                                in1=cget("normgw_row")[:],
                                                op=OP.mult)
                        nc.vector.tensor_tensor(out=xn[:], in0=xn[:],
                                                in1=cget("normgb_row")[:],
                                                op=OP.add)
                        nc.scalar.activation(gst[:, qt, :H], xn[:], AF.Tanh)
                        pq = wm.tile([P, H], bf16, tag="pq")
                        scr2 = wm.tile([P, H], bf16, tag="scr2")
                        nc.vector.tensor_tensor(out=pq[:], in0=gst[:, qt, :H],
                                                in1=swg_bf[:, 0, :], op=OP.mult)
                        ttracc(scr2[:], pq[:], gst[:, qt, :H], qtmp[:, 0:1])
                        nc.vector.tensor_tensor(out=pq[:], in0=gst[:, qt, :H],
                                                in1=swg_bf[:, 1, :], op=OP.mult)
                        ttracc(scr2[:], pq[:], gst[:, qt, :H], qtmp[:, 1:2])
                        nc.vector.tensor_copy(gst[:, qt, H:258], qtmp[:])
                nc.sync.dma_start(g_in[:, :, :], gst[:])
                nc.gpsimd.collective_compute(
                    "AllGather", OP.bypass, replica_groups=RG,
                    ins=[g_in[:, :, :]], outs=[Gtab[:, :]])

        # ======================================================== phase 4: MLP
        def mlp_pass(p4_range):
            with tc.tile_pool(name="mlpp", bufs=1) as mlpp, \
                 tc.tile_pool(name="wp", bufs=3) as wp, \
                 tc.tile_pool(name="psm", bufs=2, space="PSUM") as psm:
                W1T_t = cget("W1T")
                b1_t = cget("b1_row")
                wl_t = cget("wl_row")
                woh_t = cget("woh_col")
                xeT_t = mlpp.tile([P, 2, cfg.VPAD], f32, tag="xeT")
                nc.sync.dma_start(xeT_t[:], D_IN["XeT"][:, :, :])
                for t in range(*p4_range):
                    mm = psm.tile([P, HID], f32, tag="mmp")
                    for k in range(2):
                        nc.tensor.matmul(mm[:],
                                         lhsT=xeT_t[:, k, t * P:(t + 1) * P],
                                         rhs=W1T_t[:, k, :],
                                         start=(k == 0), stop=(k == 1))
                    h1 = wp.tile([P, HID], f32, tag="h1")
                    nc.vector.tensor_tensor(out=h1[:], in0=mm[:], in1=b1_t[:],
                                            op=OP.add)
                    s1 = wp.tile([P, 1], f32, tag="ps1c")
                    nc.scalar.activation(h1[:], h1[:], AF.Relu, accum_out=s1[:])
                    scr = wp.tile([P, HID], bf16, tag="pscr")
                    ss2 = wp.tile([P, 1], f32, tag="pss2")
                    nc.scalar.activation(scr[:], h1[:], AF.Square,
                                         accum_out=ss2[:])
                    sw1 = wp.tile([P, 1], f32, tag="psw1")
                    nc.vector.tensor_tensor_reduce(
                        out=scr[:], in0=h1[:], in1=wl_t[:], scale=1.0,
                        scalar=0.0, op0=OP.mult, op1=OP.add, accum_out=sw1[:])
                    musq = wp.tile([P, 1], f32, tag="pmusq")
                    nc.vector.tensor_scalar(
                        out=musq[:], in0=s1[:], scalar1=s1[:],
                        scalar2=1.0 / (HID * HID), op0=OP.mult, op1=OP.mult)
                    v1 = wp.tile([P, 1], f32, tag="pv1")
                    nc.vector.tensor_scalar(out=v1[:], in0=ss2[:],
                                            scalar1=1.0 / HID, scalar2=1e-5,
                                            op0=OP.mult, op1=OP.add)
                    nc.vector.tensor_tensor(out=varst[:, t:t + 1], in0=v1[:],
                                            in1=musq[:], op=OP.subtract)
                    amu = wp.tile([P, 1], f32, tag="pamu")
                    nc.vector.tensor_scalar(out=amu[:], in0=s1[:],
                                            scalar1=woh_t[:], scalar2=None,
                                            op0=OP.mult)
                    nc.vector.tensor_tensor(out=Ast[:, t:t + 1], in0=sw1[:],
                                            in1=amu[:], op=OP.subtract)

        if maxphase >= 4:
            mlp_pass((0, ETILES // 2))

        # ======================================================== phase 3: GAT
        if maxphase >= 3:
            with tc.tile_pool(name="gatp", bufs=1) as gp, \
                 tc.tile_pool(name="wg", bufs=4) as wg, \
                 tc.tile_pool(name="gbuf", bufs=3) as gbuf, \
                 tc.tile_pool(name="psg", bufs=3, space="PSUM") as psg:
                gatb_t = cget("gatb_row")
                scoreWl_t = cget("scoreW_l")
                swl_bf = gp.tile([P, 5, H], bf16, tag="swl_bf")
                nc.vector.tensor_copy(swl_bf[:], scoreWl_t[:])
                xlo = GatherList("GAT_Xlo", XWtab[:GHALF, :], 384, gbuf, gp)
                xhi = GatherList("GAT_Xhi", XWtab[GHALF:, :], 384, gbuf, gp)
                adlo = GatherList("GAT_ADlo", ad_loc[:, :], 128, gbuf, gp,
                                  idx_kind="GAT_Xlo")
                adhi = GatherList("GAT_ADhi", ad_loc[:, :], 128, gbuf, gp,
                                  idx_kind="GAT_Xhi")

                cnt_lo = meta["shared_cnt"]["GAT_Xlo"]
                cnt_hi = meta["shared_cnt"]["GAT_Xhi"]
                ex_pid, ex_tile = {}, {}
                for ot in range(VTILES):
                    qtmp3 = wg.tile([P, 2], f32, tag="qtmp3")
                    acc = psg.tile([P, 260], f32, tag="gatacc")
                    total = int(cnt_lo[ot]) + int(cnt_hi[ot])
                    done = 0
                    for gl, adl, cnt in ((xlo, adlo, int(cnt_lo[ot])),
                                         (xhi, adhi, int(cnt_hi[ot]))):
                        for _ in range(cnt):
                            rows, segc = gl.chunk()
                            adrows, _ = adl.chunk()
                            pid = (gl.k - 1) // CPP
                            if ex_pid.get(gl.kind) != pid:
                                ex_pid[gl.kind] = pid
                                ext = wg.tile([P, CPP, HEADS], f32,
                                              tag=f"ex_{gl.kind}")
                                nc.vector.tensor_tensor(
                                    out=ext[:], in0=gl.cur[:, :, 256:260],
                                    in1=adl.cur[:, :, 0:HEADS], op=OP.add)
                                t02 = wg.tile([P, CPP, HEADS], f32, tag="t02")
                                nc.vector.tensor_scalar(out=t02[:], in0=ext[:],
                                                        scalar1=0.2,
                                                        scalar2=None,
                                                        op0=OP.mult)
                                nc.vector.tensor_tensor(out=ext[:], in0=t02[:],
                                                        in1=ext[:], op=OP.max)
                                nc.scalar.activation(ext[:], ext[:], AF.Exp)
                                ex_tile[gl.kind] = ext
                            s = (gl.k - 1) % CPP
                            ex = ex_tile[gl.kind][:, s, :]
                            rhs = wg.tile([P, 260], bf16, tag="gatrhs")
                            for h in range(2):
                                nc.vector.tensor_scalar(
                                    out=rhs[:, h * 64:(h + 1) * 64],
                                    in0=rows[:, h * 64:(h + 1) * 64],
                                    scalar1=ex[:, h:h + 1], scalar2=None,
                                    op0=OP.mult)
                            for h in range(2, HEADS):
                                nc.scalar.activation(
                                    rhs[:, h * 64:(h + 1) * 64],
                                    rows[:, h * 64:(h + 1) * 64],
                                    AF.Copy, scale=ex[:, h:h + 1])
                            nc.vector.tensor_copy(rhs[:, 256:260], ex[:])
                            oh = onehot(wg, segc, tag="ohgat")
                            nc.tensor.matmul(acc[:], lhsT=oh[:], rhs=rhs[:],
                                             start=(done == 0),
                                             stop=(done == total - 1))
                            done += 1
                    dene = wg.tile([P, HEADS], f32, tag="dene")
                    nc.vector.tensor_scalar(out=dene[:], in0=acc[:, 256:260],
                                            scalar1=1e-20, scalar2=None,
                                            op0=OP.add)
                    rc = wg.tile([P, HEADS], f32, tag="gatrc")
                    nc.vector.reciprocal(rc[:], dene[:])
                    xl2 = wg.tile([P, H], f32, tag="xl2")
                    for h in range(HEADS):
                        nc.vector.tensor_scalar(
                            out=xl2[:, h * 64:(h + 1) * 64],
                            in0=acc[:, h * 64:(h + 1) * 64],
                            scalar1=rc[:, h:h + 1], scalar2=None, op0=OP.mult)
                    nc.vector.tensor_tensor(out=xl2[:], in0=xl2[:], in1=gatb_t[:],
                                            op=OP.add)
                    nc.vector.tensor_tensor(out=xl2[:], in0=xl2[:],
                                            in1=xl1ns[:, ot, :], op=OP.add)
                    xlst = wg.tile([P, 384], bf16, tag="xlst")
                    nc.gpsimd.memset(xlst[:, 258:384], 0.0)
                    nc.scalar.activation(xlst[:, :H], xl2[:], AF.Tanh)
                    pq = wg.tile([P, H], bf16, tag="pq3")
                    scr3 = wg.tile([P, H], bf16, tag="scr3")
                    nc.vector.tensor_tensor(out=pq[:], in0=xlst[:, :H],
                                            in1=swl_bf[:, 0, :], op=OP.mult)
                    ttracc(scr3[:], pq[:], xlst[:, :H], qtmp3[:, 0:1])
                    nc.vector.tensor_tensor(out=pq[:], in0=xlst[:, :H],
                                            in1=swl_bf[:, 1, :], op=OP.mult)
                    ttracc(scr3[:], pq[:], xlst[:, :H], qtmp3[:, 1:2])
                    nc.vector.tensor_copy(xlst[:, H:258], qtmp3[:])
                    nc.sync.dma_start(l_in[:, ot, :], xlst[:])
                nc.gpsimd.collective_compute(
                    "AllGather", OP.bypass, replica_groups=RG,
                    ins=[l_in[:, :, :]], outs=[Ltab[:, :]])

        mid.close()

        # ====================================================== phase 5: score
        def score_pass(br, lists, sw_bf, dw_bf, zst, nst, nnst, ws, psacc,
                       psep, rcols_t):
            cnts = meta["shared_cnt"]
            for ot in range(ETILES):
                accs = {}
                for side in "TH":
                    acc = psacc.tile([P, 258], f32, tag=f"acc{side}{br}")
                    kinds = ([f"{side}_G"] if br == "G"
                             else [f"{side}_Llo", f"{side}_Lhi"])
                    total = sum(int(cnts[k][ot]) for k in kinds)
                    done = 0
                    for k in kinds:
                        for _ in range(int(cnts[k][ot])):
                            rows, segc = lists[k].chunk()
                            oh = onehot(ws, segc, tag="ohs")
                            nc.tensor.matmul(
                                acc[:], lhsT=oh[:], rhs=rows[:, :258],
                                start=(done == 0), stop=(done == total - 1))
                            done += 1
                    accs[side] = acc
                accT, accH = accs["T"], accs["H"]
                smT = ws.tile([P, H], bf16, tag="smT")
                nc.scalar.copy(smT[:], accT[:, :H])
                smH = ws.tile([P, H], bf16, tag="smH")
                nc.scalar.copy(smH[:], accH[:, :H])
                zbuf = ws.tile([P, 5], f32, tag="zbuf")
                pq = ws.tile([P, H], bf16, tag="pqs")
                scr = ws.tile([P, H], bf16, tag="scrs")
                nc.vector.tensor_tensor(out=pq[:], in0=smT[:],
                                        in1=sw_bf[:, 2, :], op=OP.mult)
                nc.vector.tensor_tensor_reduce(
                    out=scr[:], in0=pq[:], in1=smT[:], scale=1.0, scalar=0.0,
                    op0=OP.mult, op1=OP.add, accum_out=zbuf[:, 0:1])
                nc.vector.tensor_tensor(out=pq[:], in0=smH[:],
                                        in1=sw_bf[:, 3, :], op=OP.mult)
                nc.vector.tensor_tensor_reduce(
                    out=scr[:], in0=pq[:], in1=smH[:], scale=1.0, scalar=0.0,
                    op0=OP.mult, op1=OP.add, accum_out=zbuf[:, 1:2])
                nc.vector.tensor_tensor(out=pq[:], in0=smT[:],
                                        in1=sw_bf[:, 4, :], op=OP.mult)
                nc.vector.tensor_tensor_reduce(
                    out=scr[:], in0=pq[:], in1=smH[:], scale=1.0, scalar=0.0,
                    op0=OP.mult, op1=OP.add, accum_out=zbuf[:, 2:3])
                nc.vector.tensor_copy(zbuf[:, 3:4], accT[:, 256:257])
                nc.vector.tensor_copy(zbuf[:, 4:5], accH[:, 257:258])
                scr5 = ws.tile([P, 5], f32, tag="scr5")
                ttracc(scr5[:], zbuf[:], rcols_t[:, :, ot], zst[:, ot:ot + 1])
                # De: a = smT @ DW.T (scale-invariant)
                mtT = ws.tile([P, 2, P], bf16, tag="mtT")
                for k in range(2):
                    tp = psep.tile([P, P], bf16, tag="mtTps")
                    nc.tensor.transpose(tp[:], smT[:, k * P:(k + 1) * P],
                                        ident_bf[:])
                    nc.vector.tensor_copy(mtT[:, k, :], tp[:])
                a_ps = psep.tile([P, H], f32, tag="aps")
                for k in range(2):
                    nc.tensor.matmul(a_ps[:], lhsT=mtT[:, k, :],
                                     rhs=dw_bf[:, k, :],
                                     start=(k == 0), stop=(k == 1))
                nc.vector.tensor_tensor_reduce(
                    out=scr[:], in0=a_ps[:], in1=smH[:], scale=1.0, scalar=0.0,
                    op0=OP.mult, op1=OP.add, accum_out=nst[:, ot:ot + 1])
                na2 = ws.tile([P, 1], f32, tag="na2")
                nc.scalar.activation(scr[:], a_ps[:], AF.Square,
                                     accum_out=na2[:])
                nh2 = ws.tile([P, 1], f32, tag="nh2")
                nc.scalar.activation(scr[:], smH[:], AF.Square,
                                     accum_out=nh2[:])
                nn = ws.tile([P, 1], f32, tag="nn")
                nc.vector.tensor_tensor(out=nn[:], in0=na2[:], in1=nh2[:],
                                        op=OP.mult)
                nc.vector.tensor_scalar(out=nnst[:, ot:ot + 1], in0=nn[:],
                                        scalar1=1e-16, scalar2=None, op0=OP.max)

        if maxphase >= 5:
            rcols_t = cget("rcols")
            # ---- G pass (overlaps Ltab AllGather)
            with tc.tile_pool(name="scoG", bufs=1) as scoG, \
                 tc.tile_pool(name="gbufG", bufs=3) as gbufG, \
                 tc.tile_pool(name="wsG", bufs=3) as wsG, \
                 tc.tile_pool(name="psaG", bufs=2, space="PSUM") as psaG, \
                 tc.tile_pool(name="pseG", bufs=2, space="PSUM") as pseG:
                swg_bf2 = scoG.tile([P, 5, H], bf16, tag="swg_bf2")
                nc.vector.tensor_copy(swg_bf2[:], cget("scoreW_g")[:])
                dwg_bf = scoG.tile([P, 2, H], bf16, tag="dwg_bf")
                nc.vector.tensor_copy(dwg_bf[:], cget("DWTg")[:])
                listsG = {}
                for side in "TH":
                    listsG[f"{side}_G"] = GatherList(f"{side}_G", Gtab[:, :],
                                                     384, gbufG, scoG)
                score_pass("G", listsG, swg_bf2, dwg_bf, zstG, numG, nnG,
                           wsG, psaG, pseG, rcols_t)
            mlp_pass((ETILES // 2, ETILES))
            # ---- L pass
            with tc.tile_pool(name="scoL", bufs=1) as scoL, \
                 tc.tile_pool(name="gbufL", bufs=3) as gbufL, \
                 tc.tile_pool(name="wsL", bufs=3) as wsL, \
                 tc.tile_pool(name="psaL", bufs=2, space="PSUM") as psaL, \
                 tc.tile_pool(name="pseL", bufs=2, space="PSUM") as pseL:
                swl_bf2 = scoL.tile([P, 5, H], bf16, tag="swl_bf2")
                nc.vector.tensor_copy(swl_bf2[:], cget("scoreW_l")[:])
                dwl_bf = scoL.tile([P, 2, H], bf16, tag="dwl_bf")
                nc.vector.tensor_copy(dwl_bf[:], cget("DWTl")[:])
                listsL = {}
                for side in "TH":
                    listsL[f"{side}_Llo"] = GatherList(
                        f"{side}_Llo", Ltab[:GHALF, :], 384, gbufL, scoL)
                    listsL[f"{side}_Lhi"] = GatherList(
                        f"{side}_Lhi", Ltab[GHALF:, :], 384, gbufL, scoL)
                score_pass("L", listsL, swl_bf2, dwl_bf, zstL, numL, nnL,
                           wsL, psaL, pseL, rcols_t)

            # ---- batched finish
            with tc.tile_pool(name="fin", bufs=1) as fin:
                Qc = cfg.Q
                sqG = fin.tile([P, ETILES], f32, tag="sqG")
                nc.scalar.activation(sqG[:], nnG[:], AF.Sqrt)
                sqL = fin.tile([P, ETILES], f32, tag="sqL")
                nc.scalar.activation(sqL[:], nnL[:], AF.Sqrt)
                sqV = fin.tile([P, ETILES], f32, tag="sqV")
                nc.scalar.activation(sqV[:], varst[:], AF.Sqrt)
                rG = fin.tile([P, ETILES], f32, tag="rG")
                nc.vector.reciprocal(rG[:], sqG[:])
                rL = fin.tile([P, ETILES], f32, tag="rL")
                nc.vector.reciprocal(rL[:], sqL[:])
                r4 = fin.tile([P, ETILES], f32, tag="r4")
                nc.vector.reciprocal(r4[:], sqV[:])
                cosG = fin.tile([P, ETILES], f32, tag="cosG")
                nc.vector.tensor_tensor(out=cosG[:], in0=numG[:], in1=rG[:],
                                        op=OP.mult)
                cosL = fin.tile([P, ETILES], f32, tag="cosL")
                nc.vector.tensor_tensor(out=cosL[:], in0=numL[:], in1=rL[:],
                                        op=OP.mult)
                nc.vector.tensor_scalar(out=cosG[:], in0=cosG[:],
                                        scalar1=0.5 * Qc, scalar2=None,
                                        op0=OP.mult)
                nc.vector.tensor_scalar(out=cosL[:], in0=cosL[:],
                                        scalar1=0.5 * (1.0 - Qc), scalar2=None,
                                        op0=OP.mult)
                nc.vector.tensor_tensor(out=cosG[:], in0=cosG[:], in1=cosL[:],
                                        op=OP.add)
                nc.vector.tensor_scalar(out=de_stage[:], in0=cosG[:],
                                        scalar1=0.5, scalar2=None, op0=OP.add)
                zp = fin.tile([P, ETILES], f32, tag="zp")
                nc.vector.tensor_tensor(out=zp[:], in0=Ast[:], in1=r4[:],
                                        op=OP.mult)
                nc.scalar.activation(pe_stage[:], zp[:], AF.Sigmoid,
                                     bias=cget("c1b2_col")[:])
                seG = fin.tile([P, ETILES], f32, tag="seG")
                nc.scalar.activation(seG[:], zstG[:], AF.Sigmoid,
                                     bias=cget("sbg_col")[:])
                seL = fin.tile([P, ETILES], f32, tag="seL")
                nc.scalar.activation(seL[:], zstL[:], AF.Sigmoid,
                                     bias=cget("sbl_col")[:])
                nc.vector.tensor_scalar(out=seG[:], in0=seG[:], scalar1=Qc,
                                        scalar2=None, op0=OP.mult)
                nc.vector.tensor_scalar(out=seL[:], in0=seL[:],
                                        scalar1=1.0 - Qc, scalar2=None,
                                        op0=OP.mult)
                nc.vector.tensor_tensor(out=se_stage[:], in0=seG[:],
                                        in1=seL[:], op=OP.add)

        nc.sync.dma_start(Pe_out[:, :], pe_stage[:])
        nc.sync.dma_start(Se_out[:, :], se_stage[:])
        nc.sync.dma_start(De_out[:, :], de_stage[:])

    nc.compile()
    return nc


# ------------------------------------------------------------------- driver

_CACHE = {}

_JIT_CACHE = {}


def _pjrt_run(nc, in_maps, n_cores, repeats=1, chain=0):
    """Execute the compiled Bass module on n_cores via PJRT (axon), caching
    the jitted executable, optionally timing pipelined executions."""
    import time as _time
    import jax
    import concourse.mybir as mybir
    from concourse import bass2jax
    from jax.experimental.shard_map import shard_map
    from jax.sharding import Mesh, PartitionSpec

    bass2jax.install_neuronx_cc_hook()

    partition_name = (nc.partition_id_tensor.name
                      if nc.partition_id_tensor else None)
    in_names, out_names, out_avals = [], [], []
    for alloc in nc.m.functions[0].allocations:
        if not isinstance(alloc, bass2jax.mybir.MemoryLocationSet):
            continue
        name = alloc.memorylocations[0].name
        if alloc.kind == "ExternalInput":
            if name != partition_name:
                in_names.append(name)
        elif alloc.kind == "ExternalOutput":
            out_names.append(name)
            out_avals.append(jax.core.ShapedArray(
                tuple(alloc.tensor_shape), mybir.dt.np(alloc.dtype)))
    n_params = len(in_names)
    all_in = list(in_names) + list(out_names)
    if partition_name is not None:
        all_in.append(partition_name)

    key = (id(nc), chain)
    if key not in _JIT_CACHE:
        def _body(*args):
            operands = list(args)
            if partition_name is not None:
                operands.append(bass2jax.partition_id_tensor())
            outs = bass2jax._bass_exec_p.bind(
                *operands, out_avals=tuple(out_avals),
                in_names=tuple(all_in), out_names=tuple(out_names),
                lowering_input_output_aliases=(),
                sim_require_finite=True, sim_require_nnan=True, nc=nc)
            return tuple(outs)

        n_outs = len(out_names)
        nrep = max(1, chain)

        def _chained(*args):
            outs = None
            for i in range(nrep):
                zs = args[n_params + i * n_outs:n_params + (i + 1) * n_outs]
                outs = _body(*args[:n_params], *zs)
            return outs

        devices = jax.devices()[:n_cores]
        mesh = Mesh(np.asarray(devices), ("core",))
        donate = tuple(range(n_params, n_params + nrep * n_outs))
        _JIT_CACHE[key] = jax.jit(
            shard_map(_chained, mesh=mesh,
                      in_specs=(PartitionSpec("core"),) * (n_params +
                                                           nrep * n_outs),
                      out_specs=(PartitionSpec("core"),) * n_outs,
                      check_rep=False),
            donate_argnums=donate, keep_unused=True)
    fn = _JIT_CACHE[key]
    nrep = max(1, chain)

    from jax.sharding import NamedSharding
    devices = jax.devices()[:n_cores]
    mesh = Mesh(np.asarray(devices), ("core",))
    shd = NamedSharding(mesh, PartitionSpec("core"))
    concat_in = [jax.device_put(
        np.concatenate([np.asarray(in_maps[c][nm]) for c in range(n_cores)],
                       axis=0), shd) for nm in in_names]

    def zeros():
        return [jax.device_put(
            np.zeros((n_cores * a.shape[0], *a.shape[1:]), a.dtype), shd)
            for a in out_avals for _ in range(1)]

    def zchain():
        out = []
        for _ in range(nrep):
            out.extend(zeros())
        return out

    staged = [zchain() for _ in range(repeats)]
    out_arrs = jax.block_until_ready(fn(*concat_in, *staged[0]))
    best_ns = None
    if repeats > 1:
        t0 = _time.perf_counter()
        pend = [fn(*concat_in, *staged[r]) for r in range(1, repeats)]
        jax.block_until_ready(pend)
        best_ns = (_time.perf_counter() - t0) * 1e9 / (repeats - 1)
        out_arrs = pend[-1]
    results = [{nm: np.asarray(out_arrs[i]).reshape(
        n_cores, *out_avals[i].shape)[c] for i, nm in enumerate(out_names)}
        for c in range(n_cores)]
    return results, best_ns


def _run(cfg, inputs, sim=False, trace=False, maxphase=99, repeats=1,
         chain=0, reps=1):
    meta = preprocess(cfg, inputs["C_vertex"], inputs["C_edge"],
                      inputs["T_vertex"], inputs["H_vertex"],
                      inputs["T_edge"], inputs["H_edge"], inputs["e_index"])
    in_maps = prepare_inputs(cfg, inputs, meta)

    key = (cfg.NV, cfg.NE, cfg.NG,
           hash(np.asarray(inputs["T_edge"]).tobytes()) ^
           hash(np.asarray(inputs["e_index"]).tobytes()) ^
           hash(np.asarray(inputs["C_edge"]).tobytes()) ^
           hash(np.asarray(inputs["C_vertex"]).tobytes()) ^
           hash(np.asarray(inputs["T_vertex"]).tobytes()) ^
           hash(np.asarray(inputs["H_vertex"]).tobytes()) ^
           hash(np.asarray(inputs["H_edge"]).tobytes()))
    key = key + (maxphase, reps)
    if key not in _CACHE:
        _CACHE[key] = build(cfg, meta, maxphase, reps)
    nc = _CACHE[key]

    if sim:
        from concourse.bass_interp import MultiCoreSim
        s = MultiCoreSim(nc, cfg.NCORES, num_workers=cfg.NCORES,
                         ignore_data_errors=True)
        for c in range(cfg.NCORES):
            for k, v in in_maps[c].items():
                s.cores[c].tensor(k)[:] = v
        s.simulate()
        results = [{n: np.asarray(s.cores[c].tensor(n))
                    for n in ("Pe_out", "Se_out", "De_out")}
                   for c in range(cfg.NCORES)]
        exec_ns = None
    else:
        results, exec_ns = _pjrt_run(nc, in_maps, cfg.NCORES,
                                     repeats=repeats, chain=chain)

    def unpack(name):
        full = np.zeros((cfg.NE, 1), np.float32)
        for c in range(cfg.NCORES):
            vals = np.asarray(results[c][name])      # (128, ETILES)
            flat = vals.T.reshape(-1)[:cfg.ER]
            full[c * cfg.ER:(c + 1) * cfg.ER, 0] = flat
        return full

    return (unpack("Pe_out"), unpack("Se_out"), unpack("De_out")), exec_ns


def kernel(**inputs):
    (Pe, Se, De), _ = _run(FULL, inputs, sim=False)
    return Pe, Se, De


if __name__ == "__main__":
    pass


# revision 28
# speedup vs baseline: 2.0933x; 1.0323x over previous
"""Trainium2 Bass kernel for the DHMNN gnn_message_passing problem.

kernel(**inputs) takes FULL unsharded inputs, shards across 8 NeuronCores,
runs one SPMD Bass/Tile program, returns full (Pe, Se, De).

v2 design (vs baseline):
- Tables carry [x | q_u | q_v] (258 cols bf16) instead of [x | x^2] (512):
  the Se terms sum(u*x^2) are precomputed per vertex before the AllGather,
  halving AllGather bytes, gather DMA and score matmul width.
- Se computed via z = rT*A + rH*B + rT^2*C + rH^2*D + rT*rH*E with
  host-combined weight rows (u=sw0+sw2, v=sw1+sw3, w=sw2+sw3).
- De cosine is scale-invariant: segment sums used directly (no recip).
- Sqrt/Sigmoid batched at the end -> ~5 activation-table loads total.
- GAT att_src/att_dst folded into the gatW matmul (264-col weights).
- MLP LayerNorm folded algebraically into two staged columns.
- Score G-branch runs before/during the Ltab AllGather; CPP=12.
"""

import math
import numpy as np

P = 128


class Cfg:
    def __init__(self, NG, NV, NE, E, NT, NH, NGRAPH):
        self.NG, self.NV, self.NE, self.E, self.NT, self.NH = NG, NV, NE, E, NT, NH
        self.NGRAPH = NGRAPH
        self.DG, self.DL, self.H, self.HEADS, self.HID = 256, 128, 256, 4, 256
        self.D = self.H // self.HEADS
        self.Q = 0.5
        self.NCORES = 8
        self.VR = NV // self.NCORES
        self.ER = NE // self.NCORES
        self.GR = NG // self.NCORES
        self.VTILES = math.ceil(self.VR / P)
        self.ETILES = math.ceil(self.ER / P)
        self.GTILES = math.ceil(self.GR / P)
        self.VPAD = self.VTILES * P
        self.GHALF = (self.NCORES // 2) * self.VPAD
        self.NGT = NG // P            # global seq tiles (MHA keys)
        self.QT = self.GR // P        # own query tiles
        self.CPP = 8                  # gather chunks per dma_gather piece


FULL = Cfg(NG=4096, NV=50000, NE=50000, E=300000, NT=300000, NH=300000,
           NGRAPH=64)
SMALL = Cfg(NG=1024, NV=10000, NE=10000, E=60000, NT=60000, NH=60000,
            NGRAPH=64)

# ------------------------------------------------------------- layout helpers


def vhat(cfg, v):
    v = np.asarray(v)
    c, r = v // cfg.VR, v % cfg.VR
    p, t = r % P, r // P
    return c * cfg.VPAD + p * cfg.VTILES + t


def ghat(cfg, g):
    g = np.asarray(g)
    c, r = g // cfg.GR, g % cfg.GR
    p, t = r % P, r // P
    return c * cfg.GR + p * cfg.GTILES + t


def col_layout_f32(vals, lo, n_rows, ntiles):
    """(128, ntiles) f32: [p, t] = vals[lo + t*128 + p] (pad 0)."""
    arr = np.zeros(ntiles * P, np.float32)
    v = np.asarray(vals[lo:lo + n_rows], np.float32)
    arr[:len(v)] = v
    return np.ascontiguousarray(arr.reshape(ntiles, P).T)


def row_bcast(vec, n=P):
    return np.ascontiguousarray(
        np.tile(np.asarray(vec, np.float32).reshape(1, -1), (n, 1)))


def col_of(vec, nblk):
    v = np.asarray(vec, np.float32).reshape(nblk, P)
    return np.ascontiguousarray(v.T)


# --------------------------------------------------------------- preprocessing


def _tile_groups(seg, core_lo, core_hi, ntiles):
    order = np.argsort(seg, kind="stable")
    seg_s = seg[order]
    groups, rebased = [], []
    for o in range(ntiles):
        s0 = core_lo + P * o
        s1 = min(core_lo + P * (o + 1), core_hi)
        j0 = np.searchsorted(seg_s, s0)
        j1 = np.searchsorted(seg_s, s1)
        groups.append(order[j0:j1])
        rebased.append((seg_s[j0:j1] - s0).astype(np.float32))
    return groups, rebased


def _assemble(groups_per_core, shared_cnt, idx_fn, cpp):
    ntiles = len(shared_cnt)
    total_chunks = int(shared_cnt.sum())
    npieces = max(1, math.ceil(total_chunks / cpp))
    tot_rows = npieces * cpp * P
    out = []
    for rows_tiles, seg_tiles in groups_per_core:
        idx = np.zeros(tot_rows, np.int64)
        seg = np.full(tot_rows, -1.0, np.float32)
        pos = 0
        for o in range(ntiles):
            rows, sg = rows_tiles[o], seg_tiles[o]
            n = len(rows)
            idx[pos:pos + n] = idx_fn(rows)
            seg[pos:pos + n] = sg
            pos += int(shared_cnt[o]) * P
        out.append(dict(idx=idx, seg=seg))
    return out, npieces


def idx_i16(idx):
    """(128, n/16) int16 wrapped layout, replicated across partition groups."""
    idx = np.asarray(idx, np.int64)
    n = len(idx)
    assert n % 16 == 0
    arr = np.zeros((16, n // 16), np.int64)
    arr[np.arange(n) % 16, np.arange(n) // 16] = idx
    assert arr.max() < 32768 and arr.min() >= 0
    return np.ascontiguousarray(np.tile(arr.astype(np.int16), (8, 1)))


def seg_cols(seg):
    nch = len(seg) // P
    return np.ascontiguousarray(seg.reshape(nch, P).T)


def preprocess(cfg, C_vertex, C_edge, T_vertex, H_vertex, T_edge, H_edge,
               e_index):
    c64 = lambda x: np.asarray(x, np.int64)
    C_vertex, C_edge = c64(C_vertex), c64(C_edge)
    T_vertex, H_vertex = c64(T_vertex), c64(H_vertex)
    T_edge, H_edge, e_index = c64(T_edge), c64(H_edge), c64(e_index)

    cnt_T = np.bincount(T_edge, minlength=cfg.NE)
    cnt_H = np.bincount(H_edge, minlength=cfg.NE)
    recip_T = (1.0 / np.maximum(cnt_T, 1)).astype(np.float32)
    recip_H = (1.0 / np.maximum(cnt_H, 1)).astype(np.float32)
    gcnt = np.bincount(C_edge, minlength=cfg.NGRAPH)
    recip_G = (1.0 / np.maximum(gcnt, 1)).astype(np.float32)

    src2 = np.concatenate([e_index[0], np.arange(cfg.NV)])
    dst2 = np.concatenate([e_index[1], np.arange(cfg.NV)])

    meta = dict(lists={}, npieces={}, shared_cnt={})

    def groups_for(seg_arr, core_n, ntiles):
        out = []
        for c in range(cfg.NCORES):
            out.append(_tile_groups(seg_arr, c * core_n, (c + 1) * core_n,
                                    ntiles))
        return out

    def split_groups(groups, tab_fn, half, ntiles):
        res = []
        for c in range(cfg.NCORES):
            g_rows, g_segs = [], []
            for o in range(ntiles):
                rows, sg = groups[c][0][o], groups[c][1][o]
                tab = tab_fn(rows)
                sel = (tab < cfg.GHALF) if half == 0 else (tab >= cfg.GHALF)
                g_rows.append(rows[sel])
                g_segs.append(sg[sel])
            res.append((g_rows, g_segs))
        return res

    def shared_counts(groups, ntiles):
        cnts = np.zeros((cfg.NCORES, ntiles), np.int64)
        for c in range(cfg.NCORES):
            for o in range(ntiles):
                cnts[c, o] = math.ceil(len(groups[c][0][o]) / P)
        return cnts.max(0)

    def emit(name, groups, idx_fn, ntiles):
        shared = shared_counts(groups, ntiles)
        lists, npieces = _assemble(groups, shared, idx_fn, cfg.CPP)
        meta["lists"][name] = lists
        meta["npieces"][name] = npieces
        meta["shared_cnt"][name] = shared

    for side, (seg, gl) in dict(T=(T_edge, T_vertex),
                                H=(H_edge, H_vertex)).items():
        gg = C_vertex[gl]
        groups = groups_for(seg, cfg.ER, cfg.ETILES)
        emit(f"{side}_G", groups, lambda rows, gg=gg: ghat(cfg, gg[rows]),
             cfg.ETILES)
        tab_fn = lambda rows, gl=gl: vhat(cfg, gl[rows])
        for half, sfx in ((0, "lo"), (1, "hi")):
            sub = split_groups(groups, tab_fn, half, cfg.ETILES)
            off = 0 if half == 0 else cfg.GHALF
            emit(f"{side}_L{sfx}", sub,
                 lambda rows, off=off, tab_fn=tab_fn: tab_fn(rows) - off,
                 cfg.ETILES)

    groups = groups_for(dst2, cfg.VR, cfg.VTILES)
    tab_fn = lambda rows: vhat(cfg, src2[rows])
    for half, sfx in ((0, "lo"), (1, "hi")):
        sub = split_groups(groups, tab_fn, half, cfg.VTILES)
        off = 0 if half == 0 else cfg.GHALF
        emit(f"GAT_X{sfx}", sub,
             lambda rows, off=off: vhat(cfg, src2[rows]) - off, cfg.VTILES)
        ad_fn = lambda rows: vhat(cfg, dst2[rows])
        shared = meta["shared_cnt"][f"GAT_X{sfx}"]
        ad_lists, _ = _assemble(sub, shared, ad_fn, cfg.CPP)
        for c in range(cfg.NCORES):
            ad_lists[c]["idx"] = np.maximum(
                ad_lists[c]["idx"] - c * cfg.VPAD, 0)
        meta["lists"][f"GAT_AD{sfx}"] = ad_lists
        meta["npieces"][f"GAT_AD{sfx}"] = meta["npieces"][f"GAT_X{sfx}"]

    percore = []
    for c in range(cfg.NCORES):
        md = {}
        rT = col_layout_f32(recip_T, c * cfg.ER, cfg.ER, cfg.ETILES)
        rH = col_layout_f32(recip_H, c * cfg.ER, cfg.ER, cfg.ETILES)
        # rcols order matches zbuf [C, D, E, A, B]
        md["rcols"] = np.ascontiguousarray(
            np.stack([rT * rT, rH * rH, rT * rH, rT, rH], axis=1))
        ce = np.full(cfg.VPAD, -1.0, np.float32)
        ce[:cfg.VR] = C_edge[c * cfg.VR:(c + 1) * cfg.VR]
        md["cedge"] = np.ascontiguousarray(ce.reshape(cfg.VTILES, P).T)
        percore.append(md)
    meta["percore"] = percore
    meta["recip_G"] = recip_G
    return meta


# ----------------------------------------------------------- input preparation

LIST_KINDS = ["T_G", "T_Llo", "T_Lhi", "H_G", "H_Llo", "H_Lhi",
              "GAT_Xlo", "GAT_Xhi"]


def prepare_inputs(cfg, inputs, meta):
    f32 = np.float32
    H, DG, DL, HEADS, D = cfg.H, cfg.DG, cfg.DL, cfg.HEADS, cfg.D
    Xg = np.asarray(inputs["Xg"], f32)
    Xl = np.asarray(inputs["Xl"], f32)
    Xe = np.asarray(inputs["Xe"], f32)

    s = {}
    s["XgT"] = np.ascontiguousarray(Xg.T.reshape(2, P, -1).transpose(1, 0, 2))
    s["iota"] = row_bcast(np.arange(P, dtype=f32))
    s["ident"] = np.eye(P, dtype=f32)
    s["lingWT"] = np.ascontiguousarray(
        np.asarray(inputs["ling_W"], f32).T.reshape(2, P, H).transpose(1, 0, 2))
    s["lingb_col"] = col_of(inputs["ling_b"], 2)
    s["WinT"] = np.ascontiguousarray(
        np.asarray(inputs["mha_in_W"], f32).T.reshape(2, P, 3 * H)
        .transpose(1, 0, 2))
    s["Winb_col"] = col_of(inputs["mha_in_b"], 6)
    s["WoutT"] = np.ascontiguousarray(
        np.asarray(inputs["mha_out_W"], f32).T.reshape(2, P, H).transpose(1, 0, 2))
    s["Woutb_row"] = row_bcast(inputs["mha_out_b"])
    s["normgw_row"] = row_bcast(inputs["normg_w"])
    s["normgb_row"] = row_bcast(inputs["normg_b"])
    s["linlWT"] = np.ascontiguousarray(np.asarray(inputs["linl_W"], f32).T)
    s["linlb_row"] = row_bcast(inputs["linl_b"])
    ms = np.asarray(inputs["gn_ms"], f32)
    s["ms_row"] = row_bcast(ms)
    s["ms2_row"] = row_bcast(2.0 * ms - ms * ms)
    s["gnw_row"] = row_bcast(inputs["gn_w"])
    s["gnb_row"] = row_bcast(inputs["gn_b"])
    rg = np.zeros((P, 1), f32)
    rg[:cfg.NGRAPH, 0] = meta["recip_G"]
    s["recipG_col"] = rg

    # GAT weights extended with folded attention vectors:
    # xw = x @ W.T ; a_s = sum_d xw[h*64+d]*att_src[h,d] = x @ WAs[:,h]
    gw = np.asarray(inputs["gat_W"], f32)            # [H, H] (out, in)
    asrc = np.asarray(inputs["gat_att_src"], f32)    # [HEADS, D]
    adst = np.asarray(inputs["gat_att_dst"], f32)
    WAs = np.einsum("hdk,hd->kh", gw.reshape(HEADS, D, H), asrc)  # [H, HEADS]
    WAd = np.einsum("hdk,hd->kh", gw.reshape(HEADS, D, H), adst)
    gatx = np.concatenate([gw.T, WAs, WAd], axis=1)  # [H(in), 264]
    s["gatWTx"] = np.ascontiguousarray(
        gatx.reshape(2, P, 264).transpose(1, 0, 2))
    s["gatb_row"] = row_bcast(inputs["gat_b"])

    # score weight rows, order [u, v, wu, wv, w2m] per branch
    def score_rows(SW):
        sw = np.asarray(SW, f32).reshape(4, H)
        u, v, w = sw[0] + sw[2], sw[1] + sw[3], sw[2] + sw[3]
        return np.ascontiguousarray(np.stack(
            [row_bcast(u), row_bcast(v), row_bcast(w - u), row_bcast(w - v),
             row_bcast(-2.0 * w)], axis=1))
    s["scoreW_g"] = score_rows(inputs["lingS_W"])
    s["scoreW_l"] = score_rows(inputs["linlS_W"])
    s["sbg_col"] = np.full((P, 1), np.asarray(inputs["lingS_b"], f32)
                           .reshape(-1)[0], f32)
    s["sbl_col"] = np.full((P, 1), np.asarray(inputs["linlS_b"], f32)
                           .reshape(-1)[0], f32)
    s["DWTg"] = np.ascontiguousarray(
        np.asarray(inputs["lingD_W"], f32).T.reshape(2, P, H).transpose(1, 0, 2))
    s["DWTl"] = np.ascontiguousarray(
        np.asarray(inputs["linlD_W"], f32).T.reshape(2, P, H).transpose(1, 0, 2))

    # MLP folds
    s["W1T"] = np.ascontiguousarray(
        np.asarray(inputs["mlp_W1"], f32).T.reshape(2, P, cfg.HID).transpose(1, 0, 2))
    s["b1_row"] = row_bcast(inputs["mlp_b1"])
    lnw = np.asarray(inputs["mlp_ln_w"], f32)
    lnb = np.asarray(inputs["mlp_ln_b"], f32)
    w2 = np.asarray(inputs["mlp_W2"], f32).reshape(-1)
    wl = lnw * w2
    s["wl_row"] = row_bcast(wl)
    s["woh_col"] = np.full((P, 1), float(wl.sum()) / cfg.HID, f32)
    c1 = float((w2 * lnb).sum()) + float(np.asarray(inputs["mlp_b2"], f32)
                                         .reshape(-1)[0])
    s["c1b2_col"] = np.full((P, 1), c1, f32)

    in_maps = []
    for c in range(cfg.NCORES):
        md = meta["percore"][c]
        m = dict(s)
        m["XgT_own"] = np.ascontiguousarray(
            Xg[c * cfg.GR:(c + 1) * cfg.GR].T.reshape(2, P, -1)
            .transpose(1, 0, 2))
        xlp = np.zeros((cfg.VPAD, DL), f32)
        xlp[:cfg.VR] = Xl[c * cfg.VR:(c + 1) * cfg.VR]
        m["XlT"] = np.ascontiguousarray(xlp.T)
        xep = np.zeros((cfg.VPAD, DG), f32)
        xep[:cfg.ER] = Xe[c * cfg.ER:(c + 1) * cfg.ER]
        m["XeT"] = np.ascontiguousarray(xep.T.reshape(2, P, -1).transpose(1, 0, 2))
        m["rcols"] = md["rcols"]
        m["cedge"] = md["cedge"]
        for kind in LIST_KINDS:
            lst = meta["lists"][kind][c]
            m[f"idx_{kind}"] = idx_i16(lst["idx"])
            m[f"seg_{kind}"] = seg_cols(lst["seg"])
        for sfx in ("lo", "hi"):
            m[f"idx_GAT_AD{sfx}"] = idx_i16(
                meta["lists"][f"GAT_AD{sfx}"][c]["idx"])
        in_maps.append(m)
    return in_maps


# --------------------------------------------------------------------- builder


def build(cfg, meta, maxphase=99, reps=1):
    from contextlib import ExitStack
    import concourse.bacc as bacc
    import concourse.mybir as mybir
    import concourse.tile as tile

    f32 = mybir.dt.float32
    bf16 = mybir.dt.bfloat16
    i16 = mybir.dt.int16
    AF = mybir.ActivationFunctionType
    OP = mybir.AluOpType
    AX = mybir.AxisListType

    H, DG, DL, HEADS = cfg.H, cfg.DG, cfg.DL, cfg.HEADS
    NGRAPH, VTILES, ETILES, CPP = cfg.NGRAPH, cfg.VTILES, cfg.ETILES, cfg.CPP
    NGT, QT, GR, HID = cfg.NGT, cfg.QT, cfg.GR, cfg.HID

    nc = bacc.Bacc("TRN2", target_bir_lowering=False, debug=False,
                   num_devices=cfg.NCORES, dynamic_dma_scratch_size=32768)
    RG = [list(range(cfg.NCORES))]

    def din(name, shape, dtype=f32):
        return nc.dram_tensor(name, list(shape), dtype, kind="ExternalInput")

    D_IN = {}
    for name, shape, dt in [
        ("XgT", (P, 2, cfg.NG), f32), ("XgT_own", (P, 2, GR), f32),
        ("XlT", (DL, cfg.VPAD), f32), ("XeT", (P, 2, cfg.VPAD), f32),
        ("iota", (P, P), f32), ("ident", (P, P), f32),
        ("lingWT", (P, 2, H), f32), ("lingb_col", (P, 2), f32),
        ("WinT", (P, 2, 3 * H), f32), ("Winb_col", (P, 6), f32),
        ("WoutT", (P, 2, H), f32), ("Woutb_row", (P, H), f32),
        ("normgw_row", (P, H), f32), ("normgb_row", (P, H), f32),
        ("linlWT", (DL, H), f32), ("linlb_row", (P, H), f32),
        ("ms_row", (P, H), f32), ("ms2_row", (P, H), f32),
        ("gnw_row", (P, H), f32), ("gnb_row", (P, H), f32),
        ("recipG_col", (P, 1), f32),
        ("gatWTx", (P, 2, 264), f32), ("gatb_row", (P, H), f32),
        ("scoreW_g", (P, 5, H), f32), ("scoreW_l", (P, 5, H), f32),
        ("sbg_col", (P, 1), f32), ("sbl_col", (P, 1), f32),
        ("DWTg", (P, 2, H), f32), ("DWTl", (P, 2, H), f32),
        ("W1T", (P, 2, HID), f32), ("b1_row", (P, HID), f32),
        ("wl_row", (P, HID), f32), ("woh_col", (P, 1), f32),
        ("c1b2_col", (P, 1), f32),
        ("rcols", (P, 5, ETILES), f32),
        ("cedge", (P, VTILES), f32),
    ]:
        D_IN[name] = din(name, shape, dt)
    npieces = meta["npieces"]
    for kind in LIST_KINDS:
        npc = npieces[kind]
        D_IN[f"idx_{kind}"] = din(f"idx_{kind}", (P, npc * CPP * P // 16), i16)
        D_IN[f"seg_{kind}"] = din(f"seg_{kind}", (P, npc * CPP), f32)
    for sfx in ("lo", "hi"):
        npc = npieces[f"GAT_X{sfx}"]
        D_IN[f"idx_GAT_AD{sfx}"] = din(f"idx_GAT_AD{sfx}",
                                       (P, npc * CPP * P // 16), i16)

    Pe_out = nc.dram_tensor("Pe_out", [P, ETILES], f32, kind="ExternalOutput")
    Se_out = nc.dram_tensor("Se_out", [P, ETILES], f32, kind="ExternalOutput")
    De_out = nc.dram_tensor("De_out", [P, ETILES], f32, kind="ExternalOutput")

    gn_in = nc.dram_tensor("gn_in", [NGRAPH, 2 * H], f32)
    gn_out = nc.dram_tensor("gn_out", [NGRAPH, 2 * H], f32, addr_space="Shared")
    xw_in = nc.dram_tensor("xw_in", [P, VTILES, 384], bf16)
    XWtab = nc.dram_tensor("XWtab", [cfg.NCORES * cfg.VPAD, 384], bf16,
                           addr_space="Shared")
    ad_loc = nc.dram_tensor("ad_loc", [cfg.VPAD, 128], bf16)
    g_in = nc.dram_tensor("g_in", [P, cfg.GTILES, 384], bf16)
    Gtab = nc.dram_tensor("Gtab", [cfg.NG, 384], bf16, addr_space="Shared")
    l_in = nc.dram_tensor("l_in", [P, VTILES, 384], bf16)
    Ltab = nc.dram_tensor("Ltab", [cfg.NCORES * cfg.VPAD, 384], bf16,
                          addr_space="Shared")

    GHALF = cfg.GHALF

    with tile.TileContext(nc) as tc, ExitStack() as top:
        const = top.enter_context(tc.tile_pool(name="const", bufs=1))
        persist = top.enter_context(tc.tile_pool(name="persist", bufs=1))

        mid = ExitStack()
        loc = mid.enter_context(tc.tile_pool(name="loc", bufs=1))

        CT = {}

        def cget(name, pool=None):
            if name not in CT:
                d = D_IN[name]
                t = (pool or const).tile(list(d.shape), d.dtype, tag=name)
                nc.sync.dma_start(t[:], d[:])
                CT[name] = t
            return CT[name]

        iota_t = cget("iota")
        ident_t = cget("ident")
        iota_bf = const.tile([P, P], bf16, tag="iota_bf")
        nc.vector.tensor_copy(iota_bf[:], iota_t[:])
        ident_bf = const.tile([P, P], bf16, tag="ident_bf")
        nc.vector.tensor_copy(ident_bf[:], ident_t[:])

        pe_stage = persist.tile([P, ETILES], f32, tag="pe_stage")
        se_stage = persist.tile([P, ETILES], f32, tag="se_stage")
        de_stage = persist.tile([P, ETILES], f32, tag="de_stage")
        nc.vector.memset(pe_stage[:], 0.0)
        nc.vector.memset(se_stage[:], 0.0)
        nc.vector.memset(de_stage[:], 0.0)
        zstG = persist.tile([P, ETILES], f32, tag="zstG")
        zstL = persist.tile([P, ETILES], f32, tag="zstL")
        numG = persist.tile([P, ETILES], f32, tag="numG")
        numL = persist.tile([P, ETILES], f32, tag="numL")
        nnG = persist.tile([P, ETILES], f32, tag="nnG")
        nnL = persist.tile([P, ETILES], f32, tag="nnL")
        Ast = persist.tile([P, ETILES], f32, tag="Ast")
        varst = persist.tile([P, ETILES], f32, tag="varst")
        eps_col = const.tile([P, 1], f32, tag="eps_col")
        nc.vector.memset(eps_col[:], 1e-5)

        def onehot(pool, segc, ncols=P, tag="oh"):
            oh = pool.tile([P, ncols], bf16, tag=tag)
            nc.vector.tensor_scalar(out=oh[:], in0=iota_bf[:, :ncols],
                                    scalar1=segc, scalar2=None,
                                    op0=OP.is_equal)
            return oh

        class GatherList:
            def __init__(self, kind, table_ap, elem, pool, cpool,
                         idx_kind=None):
                self.kind = kind
                self.table_ap = table_ap
                self.elem = elem
                self.pool = pool
                ik = idx_kind or kind
                self.npc = npieces[ik if ik in npieces else kind]
                d_idx = D_IN[f"idx_{kind}"]
                self.idx_t = cpool.tile(list(d_idx.shape), i16,
                                        tag=f"idx_{kind}")
                nc.sync.dma_start(self.idx_t[:], d_idx[:])
                if f"seg_{kind}" in D_IN:
                    d_seg = D_IN[f"seg_{kind}"]
                    self.seg_t = cpool.tile(list(d_seg.shape), f32,
                                            tag=f"seg_{kind}")
                    nc.sync.dma_start(self.seg_t[:], d_seg[:])
                else:
                    self.seg_t = None
                self.k = 0
                self.cur = None

            def chunk(self):
                p, s = divmod(self.k, CPP)
                if s == 0:
                    self.cur = self.pool.tile([P, CPP, self.elem], bf16,
                                              tag=f"gb_{self.kind}")
                    nidx = CPP * P
                    nc.gpsimd.dma_gather(
                        self.cur[:], self.table_ap,
                        self.idx_t[:, p * (nidx // 16):(p + 1) * (nidx // 16)],
                        nidx, nidx, self.elem)
                rows = self.cur[:, s, :]
                segc = (self.seg_t[:, self.k:self.k + 1]
                        if self.seg_t is not None else None)
                self.k += 1
                return rows, segc

        # ==================================================== phase 1: linl+gn
        if maxphase >= 1:
            xl1ns = loc.tile([P, VTILES, H], bf16, tag="xl1ns")
            from contextlib import ExitStack as _ES
            _ps_es = _ES()
            with tc.tile_pool(name="p1", bufs=1) as p1, \
                 tc.tile_pool(name="w1", bufs=3) as w1:
                ps1 = _ps_es.enter_context(
                    tc.tile_pool(name="ps1", bufs=1, space="PSUM"))
                ps1g = _ps_es.enter_context(
                    tc.tile_pool(name="ps1g", bufs=1, space="PSUM"))
                linlWT_t = cget("linlWT")
                linlb_t = cget("linlb_row")
                cedge_t = cget("cedge")
                xl1th = p1.tile([P, VTILES, H], bf16, tag="xl1th")
                gn_ps = ps1g.tile([NGRAPH, 2 * H], f32, tag="gn")
                for t in range(VTILES):
                    xlc = w1.tile([DL, P], f32, tag="xlc")
                    nc.sync.dma_start(xlc[:], D_IN["XlT"][:, t * P:(t + 1) * P])
                    mm = ps1.tile([P, H], f32, tag="mm1")
                    nc.tensor.matmul(mm[:], lhsT=xlc[:],
                                     rhs=linlWT_t[:], start=True, stop=True)
                    xl1r = w1.tile([P, H], f32, tag="xl1r")
                    nc.vector.tensor_tensor(out=xl1r[:], in0=mm[:],
                                            in1=linlb_t[:], op=OP.add)
                    sc2 = w1.tile([P, 2 * H], bf16, tag="sc2")
                    nc.scalar.activation(sc2[:, :H], xl1r[:], AF.Tanh)
                    nc.scalar.activation(sc2[:, H:], sc2[:, :H], AF.Square)
                    nc.vector.tensor_copy(xl1th[:, t, :], sc2[:, :H])
                    oh = onehot(w1, cedge_t[:, t:t + 1], NGRAPH, tag="ohg")
                    nc.tensor.matmul(gn_ps[:], lhsT=oh[:], rhs=sc2[:],
                                     start=(t == 0), stop=(t == VTILES - 1))
                gn_sb = w1.tile([NGRAPH, 2 * H], f32, tag="gnsb")
                nc.vector.tensor_copy(gn_sb[:], gn_ps[:])
                nc.sync.dma_start(gn_in[:, :], gn_sb[:])
                nc.gpsimd.collective_compute(
                    "AllReduce", OP.add, replica_groups=RG,
                    ins=[gn_in[:, :]], outs=[gn_out[:, :]])
                gn_glob = w1.tile([NGRAPH, 2 * H], f32, tag="gnglob")
                nc.sync.dma_start(gn_glob[:], gn_out[:, :])

                # per-graph affine: x*A + B
                rgc = cget("recipG_col")
                AB = p1.tile([NGRAPH, 2 * H], f32, tag="AB")
                mean_t = w1.tile([NGRAPH, H], f32, tag="gmean")
                nc.vector.tensor_scalar(out=mean_t[:], in0=gn_glob[:, :H],
                                        scalar1=rgc[:NGRAPH, :], scalar2=None,
                                        op0=OP.mult)
                ex2_t = w1.tile([NGRAPH, H], f32, tag="gex2")
                nc.vector.tensor_scalar(out=ex2_t[:], in0=gn_glob[:, H:],
                                        scalar1=rgc[:NGRAPH, :], scalar2=None,
                                        op0=OP.mult)
                var_t = w1.tile([NGRAPH, H], f32, tag="gvar")
                nc.vector.tensor_tensor(out=var_t[:], in0=mean_t[:],
                                        in1=mean_t[:], op=OP.mult)
                nc.vector.tensor_tensor(out=var_t[:], in0=var_t[:],
                                        in1=cget("ms2_row")[:NGRAPH, :],
                                        op=OP.mult)
                nc.vector.tensor_tensor(out=var_t[:], in0=ex2_t[:], in1=var_t[:],
                                        op=OP.subtract)
                sq_t = w1.tile([NGRAPH, H], f32, tag="gsq")
                nc.scalar.activation(sq_t[:], var_t[:], AF.Sqrt,
                                     bias=eps_col[:NGRAPH, :])
                rstd_t = w1.tile([NGRAPH, H], f32, tag="grstd")
                nc.vector.reciprocal(rstd_t[:], sq_t[:])
                nc.vector.tensor_tensor(out=AB[:, :H], in0=rstd_t[:],
                                        in1=cget("gnw_row")[:NGRAPH, :],
                                        op=OP.mult)
                t2 = w1.tile([NGRAPH, H], f32, tag="gt2")
                nc.vector.tensor_tensor(out=t2[:], in0=mean_t[:], in1=AB[:, :H],
                                        op=OP.mult)
                nc.vector.tensor_tensor(out=t2[:], in0=t2[:],
                                        in1=cget("ms_row")[:NGRAPH, :],
                                        op=OP.mult)
                nc.vector.tensor_tensor(out=AB[:, H:],
                                        in0=cget("gnb_row")[:NGRAPH, :],
                                        in1=t2[:], op=OP.subtract)
                AB_bf = p1.tile([NGRAPH, 2 * H], bf16, tag="AB_bf")
                nc.vector.tensor_copy(AB_bf[:], AB[:])

                # ------------------------------------------ phase 1b: gn apply, xw
                _ps_es.close()
                ps1b = _ps_es.enter_context(
                    tc.tile_pool(name="ps1b", bufs=2, space="PSUM"))
                gatWTx_t = cget("gatWTx")
                gatWTx_bf = p1.tile([P, 2, 264], bf16, tag="gatWTx_bf")
                nc.vector.tensor_copy(gatWTx_bf[:], gatWTx_t[:])
                for t in range(VTILES):
                    oh = onehot(w1, cedge_t[:, t:t + 1], NGRAPH, tag="ohg")
                    ohT_ps = ps1b.tile([NGRAPH, P], bf16, tag="ohTps")
                    nc.tensor.transpose(ohT_ps[:], oh[:], ident_bf[:])
                    ohT = w1.tile([NGRAPH, P], bf16, tag="ohT")
                    nc.vector.tensor_copy(ohT[:], ohT_ps[:])
                    abrows = ps1b.tile([P, 2 * H], f32, tag="abrows")
                    nc.tensor.matmul(abrows[:], lhsT=ohT[:], rhs=AB_bf[:],
                                     start=True, stop=True)
                    tmp = w1.tile([P, H], f32, tag="xl1tmp")
                    nc.vector.tensor_tensor(out=tmp[:], in0=xl1th[:, t, :],
                                            in1=abrows[:, :H], op=OP.mult)
                    nc.vector.tensor_tensor(out=xl1ns[:, t, :], in0=tmp[:],
                                            in1=abrows[:, H:], op=OP.add)
                    xnT_ps = ps1b.tile([P, 2, P], bf16, tag="xnTps")
                    nc.tensor.transpose(xnT_ps[:, 0, :], xl1ns[:, t, :P],
                                        ident_bf[:])
                    nc.tensor.transpose(xnT_ps[:, 1, :], xl1ns[:, t, P:],
                                        ident_bf[:])
                    xnT = w1.tile([P, 2, P], bf16, tag="xnT")
                    nc.vector.tensor_copy(xnT[:], xnT_ps[:])
                    xw_ps = ps1b.tile([P, 264], f32, tag="xwps")
                    for k in range(2):
                        nc.tensor.matmul(xw_ps[:], lhsT=xnT[:, k, :],
                                         rhs=gatWTx_bf[:, k, :],
                                         start=(k == 0), stop=(k == 1))
                    xwt = w1.tile([P, 384], bf16, tag="xwt")
                    nc.gpsimd.memset(xwt[:, 260:384], 0.0)
                    nc.scalar.copy(xwt[:, :260], xw_ps[:, :260])
                    adt = w1.tile([P, 128], bf16, tag="adt")
                    nc.gpsimd.memset(adt[:, 4:128], 0.0)
                    nc.vector.tensor_copy(adt[:, 0:4], xw_ps[:, 260:264])
                    nc.sync.dma_start(xw_in[:, t, :], xwt[:])
                    nc.sync.dma_start(
                        ad_loc[:].rearrange("(p t) d -> p t d", p=P)[:, t, :],
                        adt[:])
                nc.gpsimd.collective_compute(
                    "AllGather", OP.bypass, replica_groups=RG,
                    ins=[xw_in[:, :, :]], outs=[XWtab[:, :]])
            _ps_es.close()

        # ======================================================== phase 2: MHA
        if maxphase >= 2:
            with tc.tile_pool(name="mha", bufs=1) as mp, \
                 tc.tile_pool(name="wm", bufs=3) as wm:
                lingWT_t = cget("lingWT")
                lingb_t = cget("lingb_col")
                WinT_t = cget("WinT")
                WinT_bf = mp.tile([P, 2, 3 * H], bf16, tag="WinT_bf")
                nc.vector.tensor_copy(WinT_bf[:], WinT_t[:])
                Winb_t = cget("Winb_col")
                NCH = cfg.NG // 512

                xg1T = mp.tile([P, 2, cfg.NG], bf16, tag="xg1T")
                with tc.tile_pool(name="xgp", bufs=2) as xgp, \
                     tc.tile_pool(name="psx", bufs=2, space="PSUM") as psx:
                    for ch in range(NCH):
                        xgc = xgp.tile([P, 2, 512], f32, tag="xgc")
                        nc.sync.dma_start(
                            xgc[:], D_IN["XgT"][:, :, ch * 512:(ch + 1) * 512])
                        for fb in range(2):
                            mm = psx.tile([P, 512], f32, tag="mmg")
                            for k in range(2):
                                nc.tensor.matmul(
                                    mm[:],
                                    lhsT=lingWT_t[:, k, fb * P:(fb + 1) * P],
                                    rhs=xgc[:, k, :],
                                    start=(k == 0), stop=(k == 1))
                            nc.scalar.activation(
                                xg1T[:, fb, ch * 512:(ch + 1) * 512], mm[:],
                                AF.Tanh, bias=lingb_t[:, fb:fb + 1])
                    xg1T_own = mp.tile([P, 2, GR], f32, tag="xg1T_own")
                    xgTo_t = xgp.tile([P, 2, GR], f32, tag="xgTo")
                    nc.sync.dma_start(xgTo_t[:], D_IN["XgT_own"][:, :, :])
                    for fb in range(2):
                        mm = psx.tile([P, GR], f32, tag="mmgo")
                        for k in range(2):
                            nc.tensor.matmul(
                                mm[:], lhsT=lingWT_t[:, k, fb * P:(fb + 1) * P],
                                rhs=xgTo_t[:, k, :], start=(k == 0), stop=(k == 1))
                        nc.scalar.activation(xg1T_own[:, fb, :], mm[:], AF.Tanh,
                                             bias=lingb_t[:, fb:fb + 1])

                kT = mp.tile([P, 2, cfg.NG], bf16, tag="kT")
                vext = mp.tile([P, NGT, HEADS, 65], bf16, tag="vext")
                nc.vector.memset(vext[:], 1.0)
                qT_own = mp.tile([P, 2, GR], bf16, tag="qT_own")
                xg1own = mp.tile([P, QT, DG], f32, tag="xg1own")
                with tc.tile_pool(name="psk", bufs=1, space="PSUM") as psk:
                    for fb in range(2):
                        mm = psk.tile([P, GR], f32, tag="qmm")
                        for k in range(2):
                            nc.tensor.matmul(
                                mm[:], lhsT=WinT_t[:, k, fb * P:(fb + 1) * P],
                                rhs=xg1T_own[:, k, :], start=(k == 0),
                                stop=(k == 1))
                        nc.scalar.activation(qT_own[:, fb, :], mm[:], AF.Identity,
                                             bias=Winb_t[:, fb:fb + 1])
                    for fb in range(2):
                        for ch in range(NCH):
                            mm = psk.tile([P, 512], f32, tag="kmm")
                            for k in range(2):
                                nc.tensor.matmul(
                                    mm[:],
                                    lhsT=WinT_bf[:, k, (2 + fb) * P:(3 + fb) * P],
                                    rhs=xg1T[:, k, ch * 512:(ch + 1) * 512],
                                    start=(k == 0), stop=(k == 1))
                            nc.scalar.activation(
                                kT[:, fb, ch * 512:(ch + 1) * 512], mm[:],
                                AF.Identity, bias=Winb_t[:, 2 + fb:3 + fb])
                    for fb in range(2):
                        for ch in range(NCH):
                            mm = psk.tile([P, 512], f32, tag="vmm")
                            for k in range(2):
                                nc.tensor.matmul(
                                    mm[:],
                                    lhsT=WinT_bf[:, k, (4 + fb) * P:(5 + fb) * P],
                                    rhs=xg1T[:, k, ch * 512:(ch + 1) * 512],
                                    start=(k == 0), stop=(k == 1))
                            vsb = wm.tile([P, 512], f32, tag="vsb")
                            nc.scalar.activation(vsb[:], mm[:], AF.Identity,
                                                 bias=Winb_t[:, 4 + fb:5 + fb])
                            for hh in range(2):
                                h = fb * 2 + hh
                                for j in range(4):
                                    kt = ch * 4 + j
                                    tp = psk.tile([P, 64], f32, tag="vtp")
                                    nc.tensor.transpose(
                                        tp[:],
                                        vsb[hh * 64:(hh + 1) * 64,
                                            j * P:(j + 1) * P],
                                        ident_t[hh * 64:(hh + 1) * 64,
                                                hh * 64:(hh + 1) * 64])
                                    nc.vector.tensor_copy(
                                        vext[:, kt, h, :64], tp[:])
                    for qt in range(QT):
                        for k in range(2):
                            tp = psk.tile([P, P], f32, tag="xg1ownT")
                            nc.tensor.transpose(
                                tp[:], xg1T_own[:, k, qt * P:(qt + 1) * P],
                                ident_t[:])
                            nc.vector.tensor_copy(
                                xg1own[:, qt, k * P:(k + 1) * P], tp[:])

                attnout = mp.tile([P, QT, H], f32, tag="attnout")
                with tc.tile_pool(name="expp", bufs=1) as expp, \
                     tc.tile_pool(name="psS", bufs=2, space="PSUM") as psS, \
                     tc.tile_pool(name="psAV", bufs=1, space="PSUM") as psAV:
                    HNGT = NGT // 2
                    for h in range(HEADS):
                        kT_h = kT[64 * (h % 2):64 * (h % 2) + 64, h // 2, :]
                        qT_h = qT_own[64 * (h % 2):64 * (h % 2) + 64, h // 2, :]
                        avs = []
                        for qt in range(QT):
                            av_t = psAV.tile([P, 65], f32, tag=f"av{qt}")
                            avs.append(av_t)
                        for half in range(2):
                            expS = expp.tile([P, HNGT, GR], bf16, tag="expS")
                            for kt in range(HNGT):
                                ktg = half * HNGT + kt
                                s_ps = psS.tile([P, GR], f32, tag="sps")
                                nc.tensor.matmul(
                                    s_ps[:], lhsT=kT_h[:, ktg * P:(ktg + 1) * P],
                                    rhs=qT_h[:, :], start=True, stop=True)
                                nc.scalar.activation(expS[:, kt, :], s_ps[:],
                                                     AF.Exp, scale=0.125)
                            for qt in range(QT):
                                for kt in range(HNGT):
                                    ktg = half * HNGT + kt
                                    nc.tensor.matmul(
                                        avs[qt][:],
                                        lhsT=expS[:, kt, qt * P:(qt + 1) * P],
                                        rhs=vext[:, ktg, h, :],
                                        start=(ktg == 0), stop=(ktg == NGT - 1))
                        for qt in range(QT):
                            rc = wm.tile([P, 1], f32, tag="avrc")
                            nc.vector.reciprocal(rc[:], avs[qt][:, 64:65])
                            nc.vector.tensor_scalar(
                                out=attnout[:, qt, h * 64:(h + 1) * 64],
                                in0=avs[qt][:, :64], scalar1=rc[:], scalar2=None,
                                op0=OP.mult)

                # out proj + residual + LN (batched sqrt) + tanh -> staging
                WoutT_t = cget("WoutT")
                scoreWg_t = cget("scoreW_g")
                swg_bf = mp.tile([P, 5, H], bf16, tag="swg_bf")
                nc.vector.tensor_copy(swg_bf[:], scoreWg_t[:])
                gst = mp.tile([P, cfg.GTILES, 384], bf16, tag="gst")
                nc.gpsimd.memset(gst[:, :, 258:384], 0.0)
                xcst = mp.tile([P, QT, H], f32, tag="xcst")
                ss2st = mp.tile([P, QT], f32, tag="ss2st")
                with tc.tile_pool(name="pso", bufs=2, space="PSUM") as pso:
                    for qt in range(QT):
                        aT = wm.tile([P, 2, P], f32, tag="aT")
                        for k in range(2):
                            tp = pso.tile([P, P], f32, tag="aTps")
                            nc.tensor.transpose(
                                tp[:], attnout[:, qt, k * P:(k + 1) * P],
                                ident_t[:])
                            nc.vector.tensor_copy(aT[:, k, :], tp[:])
                        o_ps = pso.tile([P, H], f32, tag="ops")
                        for k in range(2):
                            nc.tensor.matmul(o_ps[:], lhsT=aT[:, k, :],
                                             rhs=WoutT_t[:, k, :],
                                             start=(k == 0), stop=(k == 1))
                        xs = wm.tile([P, H], f32, tag="xs")
                        nc.vector.tensor_tensor(out=xs[:], in0=o_ps[:],
                                                in1=cget("Woutb_row")[:],
                                                op=OP.add)
                        nc.vector.tensor_tensor(out=xs[:], in0=xs[:],
                                                in1=xg1own[:, qt, :], op=OP.add)
                        ssum = wm.tile([P, 1], f32, tag="ssum")
                        nc.vector.tensor_reduce(out=ssum[:], in_=xs[:], axis=AX.X,
                                                op=OP.add)
                        mu = wm.tile([P, 1], f32, tag="mu")
                        nc.vector.tensor_scalar(out=mu[:], in0=ssum[:],
                                                scalar1=1.0 / H, scalar2=None,
                                                op0=OP.mult)
                        nc.vector.tensor_scalar(out=xcst[:, qt, :], in0=xs[:],
                                                scalar1=mu[:], scalar2=None,
                                                op0=OP.subtract)
                        scr = wm.tile([P, H], bf16, tag="lnscr")
                        nc.scalar.activation(scr[:], xcst[:, qt, :], AF.Square,
                                             accum_out=ss2st[:, qt:qt + 1])
                    sqv = wm.tile([P, QT], f32, tag="sqv")
                    nc.scalar.activation(sqv[:], ss2st[:], AF.Sqrt,
                                         bias=eps_col[:], scale=1.0 / H)
                    rstd = wm.tile([P, QT], f32, tag="rstdq")
                    nc.vector.reciprocal(rstd[:], sqv[:])
                    for qt in range(QT):
                        qtmp = wm.tile([P, 2], f32, tag="qtmp")
                        xn = wm.tile([P, H], f32, tag="xn")
                        nc.vector.tensor_scalar(out=xn[:], in0=xcst[:, qt, :],
                                                scalar1=rstd[:, qt:qt + 1],
                                                scalar2=None, op0=OP.mult)
                        nc.vector.tensor_tensor(out=xn[:], in0=xn[:],
                                                in1=cget("normgw_row")[:],
                                                op=OP.mult)
                        nc.vector.tensor_tensor(out=xn[:], in0=xn[:],
                                                in1=cget("normgb_row")[:],
                                                op=OP.add)
                        nc.scalar.activation(gst[:, qt, :H], xn[:], AF.Tanh)
                        pq = wm.tile([P, H], bf16, tag="pq")
                        scr2 = wm.tile([P, H], bf16, tag="scr2")
                        nc.vector.tensor_tensor(out=pq[:], in0=gst[:, qt, :H],
                                                in1=swg_bf[:, 0, :], op=OP.mult)
                        ttracc(scr2[:], pq[:], gst[:, qt, :H], qtmp[:, 0:1])
                        nc.vector.tensor_tensor(out=pq[:], in0=gst[:, qt, :H],
                                                in1=swg_bf[:, 1, :], op=OP.mult)
                        ttracc(scr2[:], pq[:], gst[:, qt, :H], qtmp[:, 1:2])
                        nc.vector.tensor_copy(gst[:, qt, H:258], qtmp[:])
                nc.sync.dma_start(g_in[:, :, :], gst[:])
                nc.gpsimd.collective_compute(
                    "AllGather", OP.bypass, replica_groups=RG,
                    ins=[g_in[:, :, :]], outs=[Gtab[:, :]])

        # ======================================================== phase 4: MLP
        def mlp_pass(p4_range):
            with tc.tile_pool(name="mlpp", bufs=1) as mlpp, \
                 tc.tile_pool(name="wp", bufs=3) as wp, \
                 tc.tile_pool(name="psm", bufs=2, space="PSUM") as psm:
                W1T_t = cget("W1T")
                b1_t = cget("b1_row")
                wl_t = cget("wl_row")
                woh_t = cget("woh_col")
                xeT_t = mlpp.tile([P, 2, cfg.VPAD], f32, tag="xeT")
                nc.sync.dma_start(xeT_t[:], D_IN["XeT"][:, :, :])
                for t in range(*p4_range):
                    mm = psm.tile([P, HID], f32, tag="mmp")
                    for k in range(2):
                        nc.tensor.matmul(mm[:],
                                         lhsT=xeT_t[:, k, t * P:(t + 1) * P],
                                         rhs=W1T_t[:, k, :],
                                         start=(k == 0), stop=(k == 1))
                    h1 = wp.tile([P, HID], f32, tag="h1")
                    nc.vector.tensor_tensor(out=h1[:], in0=mm[:], in1=b1_t[:],
                                            op=OP.add)
                    s1 = wp.tile([P, 1], f32, tag="ps1c")
                    nc.scalar.activation(h1[:], h1[:], AF.Relu, accum_out=s1[:])
                    scr = wp.tile([P, HID], bf16, tag="pscr")
                    ss2 = wp.tile([P, 1], f32, tag="pss2")
                    nc.scalar.activation(scr[:], h1[:], AF.Square,
                                         accum_out=ss2[:])
                    sw1 = wp.tile([P, 1], f32, tag="psw1")
                    nc.vector.tensor_tensor_reduce(
                        out=scr[:], in0=h1[:], in1=wl_t[:], scale=1.0,
                        scalar=0.0, op0=OP.mult, op1=OP.add, accum_out=sw1[:])
                    musq = wp.tile([P, 1], f32, tag="pmusq")
                    nc.vector.tensor_scalar(
                        out=musq[:], in0=s1[:], scalar1=s1[:],
                        scalar2=1.0 / (HID * HID), op0=OP.mult, op1=OP.mult)
                    v1 = wp.tile([P, 1], f32, tag="pv1")
                    nc.vector.tensor_scalar(out=v1[:], in0=ss2[:],
                                            scalar1=1.0 / HID, scalar2=1e-5,
                                            op0=OP.mult, op1=OP.add)
                    nc.vector.tensor_tensor(out=varst[:, t:t + 1], in0=v1[:],
                                            in1=musq[:], op=OP.subtract)
                    amu = wp.tile([P, 1], f32, tag="pamu")
                    nc.vector.tensor_scalar(out=amu[:], in0=s1[:],
                                            scalar1=woh_t[:], scalar2=None,
                                            op0=OP.mult)
                    nc.vector.tensor_tensor(out=Ast[:, t:t + 1], in0=sw1[:],
                                            in1=amu[:], op=OP.subtract)

        if maxphase >= 4:
            mlp_pass((0, ETILES // 2))

        # ======================================================== phase 3: GAT
        if maxphase >= 3:
            with tc.tile_pool(name="gatp", bufs=1) as gp, \
                 tc.tile_pool(name="wg", bufs=4) as wg, \
                 tc.tile_pool(name="gbuf", bufs=3) as gbuf, \
                 tc.tile_pool(name="psg", bufs=3, space="PSUM") as psg:
                gatb_t = cget("gatb_row")
                scoreWl_t = cget("scoreW_l")
                swl_bf = gp.tile([P, 5, H], bf16, tag="swl_bf")
                nc.vector.tensor_copy(swl_bf[:], scoreWl_t[:])
                xlo = GatherList("GAT_Xlo", XWtab[:GHALF, :], 384, gbuf, gp)
                xhi = GatherList("GAT_Xhi", XWtab[GHALF:, :], 384, gbuf, gp)
                adlo = GatherList("GAT_ADlo", ad_loc[:, :], 128, gbuf, gp,
                                  idx_kind="GAT_Xlo")
                adhi = GatherList("GAT_ADhi", ad_loc[:, :], 128, gbuf, gp,
                                  idx_kind="GAT_Xhi")

                cnt_lo = meta["shared_cnt"]["GAT_Xlo"]
                cnt_hi = meta["shared_cnt"]["GAT_Xhi"]
                ex_pid, ex_tile = {}, {}
                for ot in range(VTILES):
                    qtmp3 = wg.tile([P, 2], f32, tag="qtmp3")
                    acc = psg.tile([P, 260], f32, tag="gatacc")
                    total = int(cnt_lo[ot]) + int(cnt_hi[ot])
                    done = 0
                    for gl, adl, cnt in ((xlo, adlo, int(cnt_lo[ot])),
                                         (xhi, adhi, int(cnt_hi[ot]))):
                        for _ in range(cnt):
                            rows, segc = gl.chunk()
                            adrows, _ = adl.chunk()
                            pid = (gl.k - 1) // CPP
                            if ex_pid.get(gl.kind) != pid:
                                ex_pid[gl.kind] = pid
                                ext = wg.tile([P, CPP, HEADS], f32,
                                              tag=f"ex_{gl.kind}")
                                nc.vector.tensor_tensor(
                                    out=ext[:], in0=gl.cur[:, :, 256:260],
                                    in1=adl.cur[:, :, 0:HEADS], op=OP.add)
                                t02 = wg.tile([P, CPP, HEADS], f32, tag="t02")
                                nc.vector.tensor_scalar(out=t02[:], in0=ext[:],
                                                        scalar1=0.2,
                                                        scalar2=None,
                                                        op0=OP.mult)
                                nc.vector.tensor_tensor(out=ext[:], in0=t02[:],
                                                        in1=ext[:], op=OP.max)
                                nc.scalar.activation(ext[:], ext[:], AF.Exp)
                                ex_tile[gl.kind] = ext
                            s = (gl.k - 1) % CPP
                            ex = ex_tile[gl.kind][:, s, :]
                            rhs = wg.tile([P, 260], bf16, tag="gatrhs")
                            for h in range(2):
                                nc.vector.tensor_scalar(
                                    out=rhs[:, h * 64:(h + 1) * 64],
                                    in0=rows[:, h * 64:(h + 1) * 64],
                                    scalar1=ex[:, h:h + 1], scalar2=None,
                                    op0=OP.mult)
                            for h in range(2, HEADS):
                                nc.scalar.activation(
                                    rhs[:, h * 64:(h + 1) * 64],
                                    rows[:, h * 64:(h + 1) * 64],
                                    AF.Copy, scale=ex[:, h:h + 1])
                            nc.vector.tensor_copy(rhs[:, 256:260], ex[:])
                            oh = onehot(wg, segc, tag="ohgat")
                            nc.tensor.matmul(acc[:], lhsT=oh[:], rhs=rhs[:],
                                             start=(done == 0),
                                             stop=(done == total - 1))
                            done += 1
                    dene = wg.tile([P, HEADS], f32, tag="dene")
                    nc.vector.tensor_scalar(out=dene[:], in0=acc[:, 256:260],
                                            scalar1=1e-20, scalar2=None,
                                            op0=OP.add)
                    rc = wg.tile([P, HEADS], f32, tag="gatrc")
                    nc.vector.reciprocal(rc[:], dene[:])
                    xl2 = wg.tile([P, H], f32, tag="xl2")
                    for h in range(HEADS):
                        nc.vector.tensor_scalar(
                            out=xl2[:, h * 64:(h + 1) * 64],
                            in0=acc[:, h * 64:(h + 1) * 64],
                            scalar1=rc[:, h:h + 1], scalar2=None, op0=OP.mult)
                    nc.vector.tensor_tensor(out=xl2[:], in0=xl2[:], in1=gatb_t[:],
                                            op=OP.add)
                    nc.vector.tensor_tensor(out=xl2[:], in0=xl2[:],
                                            in1=xl1ns[:, ot, :], op=OP.add)
                    xlst = wg.tile([P, 384], bf16, tag="xlst")
                    nc.gpsimd.memset(xlst[:, 258:384], 0.0)
                    nc.scalar.activation(xlst[:, :H], xl2[:], AF.Tanh)
                    pq = wg.tile([P, H], bf16, tag="pq3")
                    scr3 = wg.tile([P, H], bf16, tag="scr3")
                    nc.vector.tensor_tensor(out=pq[:], in0=xlst[:, :H],
                                            in1=swl_bf[:, 0, :], op=OP.mult)
                    ttracc(scr3[:], pq[:], xlst[:, :H], qtmp3[:, 0:1])
                    nc.vector.tensor_tensor(out=pq[:], in0=xlst[:, :H],
                                            in1=swl_bf[:, 1, :], op=OP.mult)
                    ttracc(scr3[:], pq[:], xlst[:, :H], qtmp3[:, 1:2])
                    nc.vector.tensor_copy(xlst[:, H:258], qtmp3[:])
                    nc.sync.dma_start(l_in[:, ot, :], xlst[:])
                nc.gpsimd.collective_compute(
                    "AllGather", OP.bypass, replica_groups=RG,
                    ins=[l_in[:, :, :]], outs=[Ltab[:, :]])

        mid.close()

        # ====================================================== phase 5: score
        def score_pass(br, lists, sw_bf, dw_bf, zst, nst, nnst, ws, psacc,
                       psep, rcols_t):
            cnts = meta["shared_cnt"]
            for ot in range(ETILES):
                accs = {}
                for side in "TH":
                    acc = psacc.tile([P, 258], f32, tag=f"acc{side}{br}")
                    kinds = ([f"{side}_G"] if br == "G"
                             else [f"{side}_Llo", f"{side}_Lhi"])
                    total = sum(int(cnts[k][ot]) for k in kinds)
                    done = 0
                    for k in kinds:
                        for _ in range(int(cnts[k][ot])):
                            rows, segc = lists[k].chunk()
                            oh = onehot(ws, segc, tag="ohs")
                            nc.tensor.matmul(
                                acc[:], lhsT=oh[:], rhs=rows[:, :258],
                                start=(done == 0), stop=(done == total - 1))
                            done += 1
                    accs[side] = acc
                accT, accH = accs["T"], accs["H"]
                smT = ws.tile([P, H], bf16, tag="smT")
                nc.scalar.copy(smT[:], accT[:, :H])
                smH = ws.tile([P, H], bf16, tag="smH")
                nc.scalar.copy(smH[:], accH[:, :H])
                zbuf = ws.tile([P, 5], f32, tag="zbuf")
                pq = ws.tile([P, H], bf16, tag="pqs")
                scr = ws.tile([P, H], bf16, tag="scrs")
                nc.vector.tensor_tensor(out=pq[:], in0=smT[:],
                                        in1=sw_bf[:, 2, :], op=OP.mult)
                nc.vector.tensor_tensor_reduce(
                    out=scr[:], in0=pq[:], in1=smT[:], scale=1.0, scalar=0.0,
                    op0=OP.mult, op1=OP.add, accum_out=zbuf[:, 0:1])
                nc.vector.tensor_tensor(out=pq[:], in0=smH[:],
                                        in1=sw_bf[:, 3, :], op=OP.mult)
                nc.vector.tensor_tensor_reduce(
                    out=scr[:], in0=pq[:], in1=smH[:], scale=1.0, scalar=0.0,
                    op0=OP.mult, op1=OP.add, accum_out=zbuf[:, 1:2])
                nc.vector.tensor_tensor(out=pq[:], in0=smT[:],
                                        in1=sw_bf[:, 4, :], op=OP.mult)
                nc.vector.tensor_tensor_reduce(
                    out=scr[:], in0=pq[:], in1=smH[:], scale=1.0, scalar=0.0,
                    op0=OP.mult, op1=OP.add, accum_out=zbuf[:, 2:3])
                nc.vector.tensor_copy(zbuf[:, 3:4], accT[:, 256:257])
                nc.vector.tensor_copy(zbuf[:, 4:5], accH[:, 257:258])
                scr5 = ws.tile([P, 5], f32, tag="scr5")
                ttracc(scr5[:], zbuf[:], rcols_t[:, :, ot], zst[:, ot:ot + 1])
                # De: a = smT @ DW.T (scale-invariant)
                mtT = ws.tile([P, 2, P], bf16, tag="mtT")
                for k in range(2):
                    tp = psep.tile([P, P], bf16, tag="mtTps")
                    nc.tensor.transpose(tp[:], smT[:, k * P:(k + 1) * P],
                                        ident_bf[:])
                    nc.vector.tensor_copy(mtT[:, k, :], tp[:])
                a_ps = psep.tile([P, H], f32, tag="aps")
                for k in range(2):
                    nc.tensor.matmul(a_ps[:], lhsT=mtT[:, k, :],
                                     rhs=dw_bf[:, k, :],
                                     start=(k == 0), stop=(k == 1))
                nc.vector.tensor_tensor_reduce(
                    out=scr[:], in0=a_ps[:], in1=smH[:], scale=1.0, scalar=0.0,
                    op0=OP.mult, op1=OP.add, accum_out=nst[:, ot:ot + 1])
                na2 = ws.tile([P, 1], f32, tag="na2")
                nc.scalar.activation(scr[:], a_ps[:], AF.Square,
                                     accum_out=na2[:])
                nh2 = ws.tile([P, 1], f32, tag="nh2")
                nc.scalar.activation(scr[:], smH[:], AF.Square,
                                     accum_out=nh2[:])
                nn = ws.tile([P, 1], f32, tag="nn")
                nc.vector.tensor_tensor(out=nn[:], in0=na2[:], in1=nh2[:],
                                        op=OP.mult)
                nc.vector.tensor_scalar(out=nnst[:, ot:ot + 1], in0=nn[:],
                                        scalar1=1e-16, scalar2=None, op0=OP.max)

        if maxphase >= 5:
            rcols_t = cget("rcols")
            # ---- G pass (overlaps Ltab AllGather)
            with tc.tile_pool(name="scoG", bufs=1) as scoG, \
                 tc.tile_pool(name="gbufG", bufs=3) as gbufG, \
                 tc.tile_pool(name="wsG", bufs=3) as wsG, \
                 tc.tile_pool(name="psaG", bufs=2, space="PSUM") as psaG, \
                 tc.tile_pool(name="pseG", bufs=2, space="PSUM") as pseG:
                swg_bf2 = scoG.tile([P, 5, H], bf16, tag="swg_bf2")
                nc.vector.tensor_copy(swg_bf2[:], cget("scoreW_g")[:])
                dwg_bf = scoG.tile([P, 2, H], bf16, tag="dwg_bf")
                nc.vector.tensor_copy(dwg_bf[:], cget("DWTg")[:])
                listsG = {}
                for side in "TH":
                    listsG[f"{side}_G"] = GatherList(f"{side}_G", Gtab[:, :],
                                                     384, gbufG, scoG)
                score_pass("G", listsG, swg_bf2, dwg_bf, zstG, numG, nnG,
                           wsG, psaG, pseG, rcols_t)
            mlp_pass((ETILES // 2, ETILES))
            # ---- L pass
            with tc.tile_pool(name="scoL", bufs=1) as scoL, \
                 tc.tile_pool(name="gbufL", bufs=3) as gbufL, \
                 tc.tile_pool(name="wsL", bufs=3) as wsL, \
                 tc.tile_pool(name="psaL", bufs=2, space="PSUM") as psaL, \
                 tc.tile_pool(name="pseL", bufs=2, space="PSUM") as pseL:
                swl_bf2 = scoL.tile([P, 5, H], bf16, tag="swl_bf2")
                nc.vector.tensor_copy(swl_bf2[:], cget("scoreW_l")[:])
                dwl_bf = scoL.tile([P, 2, H], bf16, tag="dwl_bf")
                nc.vector.tensor_copy(dwl_bf[:], cget("DWTl")[:])
                listsL = {}
                for side in "TH":
                    listsL[f"{side}_Llo"] = GatherList(
                        f"{side}_Llo", Ltab[:GHALF, :], 384, gbufL, scoL)
                    listsL[f"{side}_Lhi"] = GatherList(
                        f"{side}_Lhi", Ltab[GHALF:, :], 384, gbufL, scoL)
                score_pass("L", listsL, swl_bf2, dwl_bf, zstL, numL, nnL,
                           wsL, psaL, pseL, rcols_t)

            # ---- batched finish
            with tc.tile_pool(name="fin", bufs=1) as fin:
                Qc = cfg.Q
                sqG = fin.tile([P, ETILES], f32, tag="sqG")
                nc.scalar.activation(sqG[:], nnG[:], AF.Sqrt)
                sqL = fin.tile([P, ETILES], f32, tag="sqL")
                nc.scalar.activation(sqL[:], nnL[:], AF.Sqrt)
                sqV = fin.tile([P, ETILES], f32, tag="sqV")
                nc.scalar.activation(sqV[:], varst[:], AF.Sqrt)
                rG = fin.tile([P, ETILES], f32, tag="rG")
                nc.vector.reciprocal(rG[:], sqG[:])
                rL = fin.tile([P, ETILES], f32, tag="rL")
                nc.vector.reciprocal(rL[:], sqL[:])
                r4 = fin.tile([P, ETILES], f32, tag="r4")
                nc.vector.reciprocal(r4[:], sqV[:])
                cosG = fin.tile([P, ETILES], f32, tag="cosG")
                nc.vector.tensor_tensor(out=cosG[:], in0=numG[:], in1=rG[:],
                                        op=OP.mult)
                cosL = fin.tile([P, ETILES], f32, tag="cosL")
                nc.vector.tensor_tensor(out=cosL[:], in0=numL[:], in1=rL[:],
                                        op=OP.mult)
                nc.vector.tensor_scalar(out=cosG[:], in0=cosG[:],
                                        scalar1=0.5 * Qc, scalar2=None,
                                        op0=OP.mult)
                nc.vector.tensor_scalar(out=cosL[:], in0=cosL[:],
                                        scalar1=0.5 * (1.0 - Qc), scalar2=None,
                                        op0=OP.mult)
                nc.vector.tensor_tensor(out=cosG[:], in0=cosG[:], in1=cosL[:],
                                        op=OP.add)
                nc.vector.tensor_scalar(out=de_stage[:], in0=cosG[:],
                                        scalar1=0.5, scalar2=None, op0=OP.add)
                zp = fin.tile([P, ETILES], f32, tag="zp")
                nc.vector.tensor_tensor(out=zp[:], in0=Ast[:], in1=r4[:],
                                        op=OP.mult)
                nc.scalar.activation(pe_stage[:], zp[:], AF.Sigmoid,
                                     bias=cget("c1b2_col")[:])
                seG = fin.tile([P, ETILES], f32, tag="seG")
                nc.scalar.activation(seG[:], zstG[:], AF.Sigmoid,
                                     bias=cget("sbg_col")[:])
                seL = fin.tile([P, ETILES], f32, tag="seL")
                nc.scalar.activation(seL[:], zstL[:], AF.Sigmoid,
                                     bias=cget("sbl_col")[:])
                nc.vector.tensor_scalar(out=seG[:], in0=seG[:], scalar1=Qc,
                                        scalar2=None, op0=OP.mult)
                nc.vector.tensor_scalar(out=seL[:], in0=seL[:],
                                        scalar1=1.0 - Qc, scalar2=None,
                                        op0=OP.mult)
                nc.vector.tensor_tensor(out=se_stage[:], in0=seG[:],
                                        in1=seL[:], op=OP.add)

        nc.sync.dma_start(Pe_out[:, :], pe_stage[:])
        nc.sync.dma_start(Se_out[:, :], se_stage[:])
        nc.sync.dma_start(De_out[:, :], de_stage[:])

    nc.compile()
    return nc


# ------------------------------------------------------------------- driver

_CACHE = {}

_JIT_CACHE = {}


def _pjrt_run(nc, in_maps, n_cores, repeats=1, chain=0):
    """Execute the compiled Bass module on n_cores via PJRT (axon), caching
    the jitted executable, optionally timing pipelined executions."""
    import time as _time
    import jax
    import concourse.mybir as mybir
    from concourse import bass2jax
    from jax.experimental.shard_map import shard_map
    from jax.sharding import Mesh, PartitionSpec

    bass2jax.install_neuronx_cc_hook()

    partition_name = (nc.partition_id_tensor.name
                      if nc.partition_id_tensor else None)
    in_names, out_names, out_avals = [], [], []
    for alloc in nc.m.functions[0].allocations:
        if not isinstance(alloc, bass2jax.mybir.MemoryLocationSet):
            continue
        name = alloc.memorylocations[0].name
        if alloc.kind == "ExternalInput":
            if name != partition_name:
                in_names.append(name)
        elif alloc.kind == "ExternalOutput":
            out_names.append(name)
            out_avals.append(jax.core.ShapedArray(
                tuple(alloc.tensor_shape), mybir.dt.np(alloc.dtype)))
    n_params = len(in_names)
    all_in = list(in_names) + list(out_names)
    if partition_name is not None:
        all_in.append(partition_name)

    key = (id(nc), chain)
    if key not in _JIT_CACHE:
        def _body(*args):
            operands = list(args)
            if partition_name is not None:
                operands.append(bass2jax.partition_id_tensor())
            outs = bass2jax._bass_exec_p.bind(
                *operands, out_avals=tuple(out_avals),
                in_names=tuple(all_in), out_names=tuple(out_names),
                lowering_input_output_aliases=(),
                sim_require_finite=True, sim_require_nnan=True, nc=nc)
            return tuple(outs)

        n_outs = len(out_names)
        nrep = max(1, chain)

        def _chained(*args):
            outs = None
            for i in range(nrep):
                zs = args[n_params + i * n_outs:n_params + (i + 1) * n_outs]
                outs = _body(*args[:n_params], *zs)
            return outs

        devices = jax.devices()[:n_cores]
        mesh = Mesh(np.asarray(devices), ("core",))
        donate = tuple(range(n_params, n_params + nrep * n_outs))
        _JIT_CACHE[key] = jax.jit(
            shard_map(_chained, mesh=mesh,
                      in_specs=(PartitionSpec("core"),) * (n_params +
                                                           nrep * n_outs),
                      out_specs=(PartitionSpec("core"),) * n_outs,
                      check_rep=False),
            donate_argnums=donate, keep_unused=True)
    fn = _JIT_CACHE[key]
    nrep = max(1, chain)

    from jax.sharding import NamedSharding
    devices = jax.devices()[:n_cores]
    mesh = Mesh(np.asarray(devices), ("core",))
    shd = NamedSharding(mesh, PartitionSpec("core"))
    concat_in = [jax.device_put(
        np.concatenate([np.asarray(in_maps[c][nm]) for c in range(n_cores)],
                       axis=0), shd) for nm in in_names]

    def zeros():
        return [jax.device_put(
            np.zeros((n_cores * a.shape[0], *a.shape[1:]), a.dtype), shd)
            for a in out_avals for _ in range(1)]

    def zchain():
        out = []
        for _ in range(nrep):
            out.extend(zeros())
        return out

    staged = [zchain() for _ in range(repeats)]
    out_arrs = jax.block_until_ready(fn(*concat_in, *staged[0]))
    best_ns = None
    if repeats > 1:
        t0 = _time.perf_counter()
        pend = [fn(*concat_in, *staged[r]) for r in range(1, repeats)]
        jax.block_until_ready(pend)
        best_ns = (_time.perf_counter() - t0) * 1e9 / (repeats - 1)
        out_arrs = pend[-1]
    results = [{nm: np.asarray(out_arrs[i]).reshape(
        n_cores, *out_avals[i].shape)[c] for i, nm in enumerate(out_names)}
        for c in range(n_cores)]
    return results, best_ns


def _run(cfg, inputs, sim=False, trace=False, maxphase=99, repeats=1,
         chain=0, reps=1):
    meta = preprocess(cfg, inputs["C_vertex"], inputs["C_edge"],
                      inputs["T_vertex"], inputs["H_vertex"],
                      inputs["T_edge"], inputs["H_edge"], inputs["e_index"])
    in_maps = prepare_inputs(cfg, inputs, meta)

    key = (cfg.NV, cfg.NE, cfg.NG,
           hash(np.asarray(inputs["T_edge"]).tobytes()) ^
           hash(np.asarray(inputs["e_index"]).tobytes()) ^
           hash(np.asarray(inputs["C_edge"]).tobytes()) ^
           hash(np.asarray(inputs["C_vertex"]).tobytes()) ^
           hash(np.asarray(inputs["T_vertex"]).tobytes()) ^
           hash(np.asarray(inputs["H_vertex"]).tobytes()) ^
           hash(np.asarray(inputs["H_edge"]).tobytes()))
    key = key + (maxphase, reps)
    if key not in _CACHE:
        _CACHE[key] = build(cfg, meta, maxphase, reps)
    nc = _CACHE[key]

    if sim:
        from concourse.bass_interp import MultiCoreSim
        s = MultiCoreSim(nc, cfg.NCORES, num_workers=cfg.NCORES,
                         ignore_data_errors=True)
        for c in range(cfg.NCORES):
            for k, v in in_maps[c].items():
                s.cores[c].tensor(k)[:] = v
        s.simulate()
        results = [{n: np.asarray(s.cores[c].tensor(n))
                    for n in ("Pe_out", "Se_out", "De_out")}
                   for c in range(cfg.NCORES)]
        exec_ns = None
    else:
        results, exec_ns = _pjrt_run(nc, in_maps, cfg.NCORES,
                                     repeats=repeats, chain=chain)

    def unpack(name):
        full = np.zeros((cfg.NE, 1), np.float32)
        for c in range(cfg.NCORES):
            vals = np.asarray(results[c][name])      # (128, ETILES)
            flat = vals.T.reshape(-1)[:cfg.ER]
            full[c * cfg.ER:(c + 1) * cfg.ER, 0] = flat
        return full

    return (unpack("Pe_out"), unpack("Se_out"), unpack("De_out")), exec_ns


def kernel(**inputs):
    (Pe, Se, De), _ = _run(FULL, inputs, sim=False)
    return Pe, Se, De


if __name__ == "__main__":
    pass
